# revision 1
# baseline (speedup 1.0000x reference)
"""Trainium2 Bass kernel for nn_Net_76622216561354 (gnn_message_passing).

Self-contained: host-side sharding/index prep (numpy) + an 8-core SPMD
Bass/Tile kernel run via run_bass_kernel_spmd. Accepts FULL inputs, returns
the FULL pooled output [8192] float32.
"""
import numpy as np
import concourse.bass as bass
import concourse.mybir as mybir
import concourse.tile as tile
from concourse import bacc
from contextlib import ExitStack
import os

import numpy as np

NC = 8
N = 131072; E = 524288; F_IN = 16; DIM = 64; DNN = 16; BK = 4; NG = 8192
NL1 = 4; NL2 = 2
SUB = 2112
NPAD = 8 * SUB          # 16896
HALF = NPAD // 2        # 8448
ECH = 512
GSLOT = 192             # pooled graph slots per sub-chunk (padded)


def host_prep(inputs):
    ei = np.asarray(inputs["edge_index"])
    batch = np.asarray(inputs["batch"]).astype(np.int64)
    src, dst = ei[0].astype(np.int64), ei[1].astype(np.int64)

    # ---- graph spans ----
    # graphs may be empty; gstart[g] = first node of graph g (batch sorted)
    gsizes = np.bincount(batch, minlength=NG)
    gstart = np.concatenate([[0], np.cumsum(gsizes)])

    # ---- core cuts at graph boundaries ----
    cuts = [0]
    for c in range(1, NC):
        t = c * (N // NC)
        while t < N and batch[t] == batch[t - 1]:
            t += 1
        cuts.append(t)
    cuts.append(N)
    cuts = np.array(cuts, np.int64)

    # ---- per-core: pack graphs into 8 graph-aligned sub-chunks ----
    g2l = np.full(N, -1, np.int64)      # global node -> local slot (within its core)
    node_core = np.zeros(N, np.int64)
    l2g = [np.full(NPAD, -1, np.int64) for _ in range(NC)]
    # pooling bookkeeping: per core, per sub-chunk: list of (graph_id, end_pos)
    pool_graphs = [[[] for _ in range(8)] for _ in range(NC)]
    pool_mask = [np.zeros((8, SUB), np.float32) for _ in range(NC)]

    for c in range(NC):
        lo, hi = cuts[c], cuts[c + 1]
        glo, ghi = batch[lo], (batch[hi - 1] + 1 if hi > lo else batch[lo])
        s = 0; pos = 0
        for g in range(glo, ghi):
            sz = int(gsizes[g])
            if sz == 0:
                continue
            if pos + sz > SUB:
                s += 1; pos = 0
                assert s < 8, f"core {c}: sub-chunk overflow"
                assert sz <= SUB
            nodes = np.arange(gstart[g], gstart[g] + sz)
            slots = s * SUB + pos + np.arange(sz)
            g2l[nodes] = slots
            node_core[nodes] = c
            l2g[c][slots] = nodes
            pool_mask[c][s, pos + 1: pos + sz] = 1.0  # same-graph continuation
            # pos of graph end within sub-chunk stream
            pool_graphs[c][s].append((g, pos + sz - 1))
            pos += sz
        assert hi == lo or batch[hi - 1] + 1 == ghi

    # ---- per (core, block) edge streams ----
    # count first to get EP
    counts = np.zeros((NC, NC), np.int64)
    dst_core = node_core[dst]; src_core = node_core[src]
    for c in range(NC):
        for b in range(NC):
            counts[c, b] = np.count_nonzero((dst_core == c) & (src_core == b))
    maxcnt = int(counts.max())
    EP = ((maxcnt + 1 + ECH - 1) // ECH) * ECH
    nchunk = EP // ECH

    dstslot = g2l[dst]; srcslot = g2l[src]

    indeg = np.bincount(dst, minlength=N).astype(np.float64)
    inv = 1.0 / np.maximum(indeg, 1.0)
    ea_all = np.asarray(inputs["edge_attr"])

    per_core = []
    for c in range(NC):
        gidx = np.zeros((8, EP), np.int64)       # src local slot per stream pos
        craw = np.zeros((8, 6, EP), np.float32)  # inv, ea*4, mask
        ends = np.zeros((8, NPAD), np.int64)
        for b in range(NC):
            m = (dst_core == c) & (src_core == b)
            eids = np.nonzero(m)[0]
            order = np.argsort(dstslot[eids], kind="stable")
            eids = eids[order]
            k = len(eids)
            ps = 1 + np.arange(k)               # positions (0 = dummy)
            gidx[b, ps] = srcslot[eids]
            craw[b, 0, ps] = inv[dst[eids]].astype(np.float32)
            craw[b, 1:5, ps] = ea_all[eids].T if False else 0
            for q in range(4):
                craw[b, 1 + q, ps] = ea_all[eids, q]
            dsl = dstslot[eids]
            same = np.zeros(k, bool)
            if k > 0:
                same[1:] = dsl[1:] == dsl[:-1]
                craw[b, 5, ps] = same.astype(np.float32)
                last = np.zeros(NPAD, np.int64)
                last[dsl] = ps                  # dsl sorted -> last write wins
                ends[b] = last
        per_core.append(dict(gidx=gidx, craw=craw, ends=ends))

    # ---- wrap helper: seq -> [16, L/16] with idx[p, s] = seq[s*16+p] ----
    def wrap16(seq):
        L = len(seq)
        assert L % 16 == 0
        return np.asarray(seq).reshape(L // 16, 16).T.copy()

    ECHUNKS = [4096, 4096, 4096, 4096, 512]
    assert sum(ECHUNKS) == NPAD

    in_maps = []
    for c in range(NC):
        pc = per_core[c]
        # gather idx: [128, EP/16] int16, wrapped per gather call
        # (super-chunks of 4*ECH, remainder as one final call)
        GCH = 4 * ECH
        gidx_t = np.zeros((128, EP // 16), np.int16)
        for b in range(NC):
            off = 0
            while off < EP:
                L = min(GCH, EP - off)
                seq = pc["gidx"][b, off:off + L]
                gidx_t[16 * b:16 * (b + 1), off // 16:(off + L) // 16] = \
                    wrap16(seq).astype(np.int16)
                off += L
        # ends idx: [128, NPAD/16] int16, wrapped per ends-chunk
        eidx_t = np.zeros((128, NPAD // 16), np.int16)
        off = 0
        for L in ECHUNKS:
            for b in range(NC):
                seq = pc["ends"][b, off:off + L]
                eidx_t[16 * b:16 * (b + 1), off // 16:(off + L) // 16] = \
                    wrap16(seq).astype(np.int16)
            off += L
        # c compact replicated x16: [128, 6, EP] fp16
        cexp = np.repeat(pc["craw"].astype(np.float16), 16, axis=0)
        # pooling
        pmask = np.repeat(pool_mask[c], 16, axis=0).astype(np.float16)
        pidx_t = np.zeros((128, GSLOT // 16), np.int16)
        for s in range(8):
            seq = np.zeros(GSLOT, np.int64)
            gl = pool_graphs[c][s]
            assert len(gl) <= GSLOT, f"GSLOT overflow: {len(gl)}"
            for i, (g, endpos) in enumerate(gl):
                seq[i] = endpos
            pidx_t[16 * s:16 * (s + 1), :] = wrap16(seq).astype(np.int16)
        # x slab transposed [16, NPAD] f32
        xT = np.zeros((16, NPAD), np.float32)
        real = l2g[c] >= 0
        xT[:, real] = np.asarray(inputs["x"])[l2g[c][real]].T
        in_maps.append(dict(xT=xT, gidx=gidx_t, eidx=eidx_t, craw=cexp,
                            pmask=pmask, pidx=pidx_t))

    meta = dict(EP=EP, nchunk=nchunk, ECHUNKS=ECHUNKS, cuts=cuts,
                pool_graphs=pool_graphs, l2g=l2g)
    return in_maps, meta


def fold_weights_host(inputs):
    """float64 weight folds -> shipped stationaries/biases (per-core identical)."""
    dt = np.float64
    lin0_w = np.asarray(inputs["lin0_w"], dt); lin0_b = np.asarray(inputs["lin0_b"], dt)
    lin1_w = np.asarray(inputs["lin1_w"], dt); lin1_b = np.asarray(inputs["lin1_b"], dt)
    lin2_w = np.asarray(inputs["lin2_w"], dt)
    root_w = np.asarray(inputs["root_w"], dt); conv_b = np.asarray(inputs["conv_b"], dt)
    nn1_w = np.asarray(inputs["nn1_w"], dt); nn1_b = np.asarray(inputs["nn1_b"], dt)
    gw_ih = np.asarray(inputs["gru_w_ih"], dt); gw_hh = np.asarray(inputs["gru_w_hh"], dt)
    gb_ih = np.asarray(inputs["gru_b_ih"], dt); gb_hh = np.asarray(inputs["gru_b_hh"], dt)

    Bm = nn1_b.reshape(DNN, DNN)
    Ak = nn1_w.reshape(BK, DNN, DNN)
    M = np.concatenate([Bm[None], Ak], axis=0)            # [5,16,16]

    w = {}
    # slab stationaries: [8, 128, 128] f16
    wslab = np.zeros((8, 128, 128), np.float32)
    for b in range(8):
        r0 = 64 * (b // 4)
        wslab[b, r0:r0 + 64, 16 * b:16 * (b + 1)] = lin1_w
    w["wslab"] = wslab.astype(np.float16)
    # M stationaries: block-diag-8 [5, 128, 128]
    wM = np.zeros((5, 128, 128), np.float32)
    for p in range(5):
        for b in range(8):
            wM[p, 16 * b:16 * (b + 1), 16 * b:16 * (b + 1)] = M[p]
    w["wM"] = wM.astype(np.float16)
    # gates h-side and folds per layer
    whs = np.zeros((NL1, 4, 128, 128), np.float32)
    wfold = np.zeros((NL1, 3, 128, 64), np.float32)
    biases = np.zeros((128, 17), np.float32)
    for j in range(NL1):
        P = lin1_w @ root_w @ gw_ih[j].T                  # [64,192]
        W_rz = P[:, :2 * DIM] + gw_hh[j].T[:, :2 * DIM]
        W_ni = P[:, 2 * DIM:]
        W_nh = gw_hh[j].T[:, 2 * DIM:]
        grp_w = [W_rz[:, :64], W_rz[:, 64:], W_ni, W_nh]
        for g in range(4):
            whs[j, g, 0:64, 0:64] = grp_w[g]
            whs[j, g, 64:128, 64:128] = grp_w[g]
        wihT = gw_ih[j].T                                  # [16,192]
        for g in range(3):
            blk = wihT[:, 64 * g:64 * (g + 1)]
            wfold[j, g] = np.tile(blk, (8, 1))
        b_base = (lin1_b @ root_w + conv_b) @ gw_ih[j].T   # [192]
        b_rz = b_base[:2 * DIM] + gb_ih[j][:2 * DIM] + gb_hh[j][:2 * DIM]
        b_ni = b_base[2 * DIM:] + gb_ih[j][2 * DIM:]
        b_hn = gb_hh[j][2 * DIM:]
        vec = [b_rz[:64], b_rz[64:], b_ni, b_hn]
        for g in range(4):
            biases[0:64, 4 * j + g] = vec[g]
            biases[64:128, 4 * j + g] = vec[g]
    w["whs"] = whs.astype(np.float16)
    w["wfold"] = wfold.astype(np.float16)
    biases[0:64, 16] = lin0_b
    biases[64:128, 16] = lin0_b
    w["biases"] = biases.astype(np.float32)
    # lin0 stationaries [2, 16, 128] f32
    wlin0 = np.zeros((2, 16, 128), np.float32)
    wlin0[0, :, 0:64] = lin0_w
    wlin0[1, :, 64:128] = lin0_w
    w["wlin0"] = wlin0
    # y stationary [128, 2] f16
    wy = np.zeros((128, 2), np.float32)
    wy[0:64, 0] = lin2_w[:, 0]
    wy[64:128, 1] = lin2_w[:, 0]
    w["wy"] = wy.astype(np.float16)
    return w


# ================= kernel builder =================

import concourse.bass as bass
import concourse.mybir as mybir
import concourse.tile as tile
from concourse import bacc
from contextlib import ExitStack

NITER = 8

f32 = mybir.dt.float32
f16 = mybir.dt.float16
i16 = mybir.dt.int16
AF = mybir.ActivationFunctionType
OP = mybir.AluOpType


def pieces(total, step):
    out = []
    off = 0
    while off < total:
        out.append((off, min(step, total - off)))
        off += step
    return out


def ends_pieces(c0, L):
    out = []
    while L > 0:
        ch = c0 // SUB
        off = c0 % SUB
        ln = min(L, SUB - off)
        out.append((ch, off, ln))
        c0 += ln
        L -= ln
    return out


def build(EP, fake_collective=False, niter=NITER):
    nchunk = EP // ECH
    nc = bacc.Bacc("TRN2", target_bir_lowering=False, debug=False, num_devices=NC)

    xT_d = nc.dram_tensor("xT", [16, NPAD], f32, kind="ExternalInput")
    gidx_d = nc.dram_tensor("gidx", [128, EP // 16], i16, kind="ExternalInput")
    eidx_d = nc.dram_tensor("eidx", [128, NPAD // 16], i16, kind="ExternalInput")
    craw_d = nc.dram_tensor("craw", [128, 6, EP], f16, kind="ExternalInput")
    pmask_d = nc.dram_tensor("pmask", [128, SUB], f16, kind="ExternalInput")
    pidx_d = nc.dram_tensor("pidx", [128, GSLOT // 16], i16, kind="ExternalInput")
    wslab_d = nc.dram_tensor("wslab", [8, 128, 128], f16, kind="ExternalInput")
    wM_d = nc.dram_tensor("wM", [5, 128, 128], f16, kind="ExternalInput")
    whs_d = nc.dram_tensor("whs", [NL1, 4, 128, 128], f16, kind="ExternalInput")
    wfold_d = nc.dram_tensor("wfold", [NL1, 3, 128, 64], f16, kind="ExternalInput")
    biases_d = nc.dram_tensor("biases", [128, 17], f32, kind="ExternalInput")
    wlin0_d = nc.dram_tensor("wlin0", [2, 16, 128], f32, kind="ExternalInput")
    wy_d = nc.dram_tensor("wy", [128, 2], f16, kind="ExternalInput")
    out_d = nc.dram_tensor("pooled", [8, GSLOT], f32, kind="ExternalOutput")

    PIECES_H = pieces(HALF, 512)
    PIECES_S = pieces(SUB, 512)

    with tile.TileContext(nc) as tc, ExitStack() as ex:
        pp = ex.enter_context(tc.tile_pool(name="persist", bufs=1))
        wk = ex.enter_context(tc.tile_pool(name="work", bufs=2))
        wk2 = ex.enter_context(tc.tile_pool(name="work2", bufs=2))
        ps = ex.enter_context(tc.tile_pool(name="psum", bufs=8, space="PSUM"))
        dr = ex.enter_context(tc.tile_pool(name="dram", bufs=1, space="DRAM"))

        BUFA = dict(tag="bufA")   # >= 8.25KB slots
        BUFB = dict(tag="bufB")
        GGT = dict(tag="gg")      # 2KB slots

        hT = pp.tile([128, HALF], f16, tag="hT")
        table = pp.tile([128, NPAD, 2], f16, tag="table")
        cum = pp.tile([128, EP, 2], f16, tag="cum")
        nc.vector.memset(cum[:], 0)
        gidx = pp.tile([128, EP // 16], i16, tag="gidx")
        eidx = pp.tile([128, NPAD // 16], i16, tag="eidx")
        pmask = pp.tile([128, SUB], f16, tag="pmask")
        pidx = pp.tile([128, GSLOT // 16], i16, tag="pidx")
        biases = pp.tile([128, 17], f32, tag="biases")
        wy = pp.tile([128, 2], f16, tag="wy")

        nc.sync.dma_start(out=gidx[:], in_=gidx_d[:])
        nc.sync.dma_start(out=eidx[:], in_=eidx_d[:])
        nc.sync.dma_start(out=pmask[:], in_=pmask_d[:])
        nc.sync.dma_start(out=pidx[:], in_=pidx_d[:])
        nc.sync.dma_start(out=biases[:], in_=biases_d[:])
        nc.sync.dma_start(out=wy[:], in_=wy_d[:])

        wslab_s = pp.tile([128, 8, 128], f16, tag="wslab_s")
        nc.sync.dma_start(
            out=wslab_s[:],
            in_=bass.AP(wslab_d, 0, [(128, 128), (128 * 128, 8), (1, 128)]))
        wM_s = pp.tile([128, 5, 128], f16, tag="wM_s")
        nc.sync.dma_start(
            out=wM_s[:], in_=bass.AP(wM_d, 0, [(128, 128), (128 * 128, 5), (1, 128)]))
        whs_s = pp.tile([128, NL1 * 4, 128], f16, tag="whs_s")
        nc.sync.dma_start(
            out=whs_s[:],
            in_=bass.AP(whs_d, 0, [(128, 128), (128 * 128, NL1 * 4), (1, 128)]))
        wfold_s = pp.tile([128, NL1 * 3, 64], f16, tag="wfold_s")
        nc.sync.dma_start(
            out=wfold_s[:],
            in_=bass.AP(wfold_d, 0, [(64, 128), (128 * 64, NL1 * 3), (1, 64)]))

        slab_dram = dr.tile([128, SUB * 2], f16)
        ag_dram = dr.tile([NC, 128, SUB * 2], f16)
        cexp_dram = dr.tile([nchunk, 128, 6 * ECH], f16)
        y_dram = dr.tile([2, HALF], f32)

        # ================= INIT =================
        # c-expansion
        for k in range(nchunk):
            cc = wk.tile([128, 6, ECH], f16, **BUFA)
            nc.sync.dma_start(
                out=cc[:],
                in_=bass.AP(craw_d, k * ECH, [(6 * EP, 128), (EP, 6), (1, ECH)]))
            ce = wk.tile([128, 6, ECH], f16, **BUFB)
            nc.vector.tensor_copy(out=ce[:, 0, :], in_=cc[:, 0, :])
            in0 = bass.AP(cc.tensor, cc[:].offset, [cc[:].ap[0], (0, 4), (1, ECH)])
            nc.vector.tensor_tensor(out=ce[:, 1:5, :], in0=in0,
                                    in1=cc[:, 1:5, :], op=OP.mult)
            nc.vector.tensor_copy(out=ce[:, 5, :], in_=cc[:, 5, :])
            nc.sync.dma_start(out=cexp_dram[k], in_=ce[:])

        # lin0 -> hT
        wlin0 = pp.tile([16, 2, 128], f32, tag="wlin0")
        nc.sync.dma_start(
            out=wlin0[:],
            in_=bass.AP(wlin0_d, 0, [(128, 16), (16 * 128, 2), (1, 128)]))
        for c0, L in PIECES_H:
            xa = wk2.tile([16, 512], f32, **GGT)
            nc.sync.dma_start(out=xa[:, :L], in_=xT_d[:, c0:c0 + L])
            xb = wk2.tile([16, 512], f32, **GGT)
            nc.sync.dma_start(out=xb[:, :L], in_=xT_d[:, HALF + c0:HALF + c0 + L])
            p0 = ps.tile([128, 512], f32, tag="ps")
            nc.tensor.matmul(p0[:, :L], wlin0[:, 0, :], xa[:, :L],
                             start=True, stop=False)
            nc.tensor.matmul(p0[:, :L], wlin0[:, 1, :], xb[:, :L],
                             start=False, stop=True)
            nc.scalar.activation(out=hT[:, c0:c0 + L], in_=p0[:, :L],
                                 func=AF.Relu, bias=biases[:, 16:17], scale=1.0)

        # ================= ITERATIONS =================
        for it in range(niter):
            j = (it // 2) % NL1
            # ---- A: slab + exchange ----
            for c0, L in PIECES_S:
                p0 = ps.tile([128, 512], f32, tag="ps")
                for b in range(8):
                    rc0 = (b % 4) * SUB + c0
                    nc.tensor.matmul(p0[:, :L], wslab_s[:, b, :],
                                     hT[:, rc0:rc0 + L],
                                     start=(b == 0), stop=(b == 7))
                stg = wk2.tile([128, 1024], f16, tag="slabstg")
                for dup in range(2):
                    dst = bass.AP(stg.tensor, stg[:].offset + dup,
                                  [stg[:].ap[0], (2, L)])
                    nc.vector.tensor_copy(out=dst, in_=p0[:, :L])
                nc.sync.dma_start(out=slab_dram[:, c0 * 2:(c0 + L) * 2],
                                  in_=stg[:, :2 * L])
            if fake_collective:
                for cc_ in range(NC):
                    nc.sync.dma_start(out=ag_dram[cc_], in_=slab_dram[:])
            else:
                nc.gpsimd.collective_compute(
                    "AllGather", OP.bypass,
                    replica_groups=[list(range(NC))],
                    ins=[slab_dram[:].opt()], outs=[ag_dram[:].opt()])
            for s in range(8):
                src = bass.AP(ag_dram.tensor,
                              ag_dram[:].offset + (16 * s) * (SUB * 2),
                              [(128 * SUB * 2, 8), (SUB * 2, 16), (1, SUB * 2)])
                dst = bass.AP(table.tensor, table[:].offset + s * SUB * 2,
                              [table[:].ap[0], (1, SUB * 2)])
                nc.sync.dma_start(out=dst, in_=src)

            # ---- C: edge chunks (gathers batched 4x to amortize ap_gather) ----
            GCH = 4 * ECH
            gbuf = None
            for k in range(nchunk):
                cc = wk.tile([128, 6, ECH], f16, **BUFA)
                nc.sync.dma_start(out=cc[:], in_=cexp_dram[k])
                if k % 4 == 0:
                    G0 = k * ECH
                    GL = min(GCH, EP - G0)
                    gbuf = pp.tile([128, GCH, 2], f16, tag="gbuf")
                    nc.gpsimd.ap_gather(
                        out_ap=gbuf[:, :GL, :], in_ap=table[:],
                        idxs_ap=gidx[:, G0 // 16:(G0 + GL) // 16],
                        channels=128, num_elems=NPAD, d=2, num_idxs=GL)
                sc = wk.tile([128, 5, ECH], f16, **BUFB)
                g_in0 = bass.AP(gbuf.tensor,
                                gbuf[:].offset + (k % 4) * ECH * 2,
                                [gbuf[:].ap[0], (0, 5), (2, ECH)])
                nc.vector.tensor_tensor(out=sc[:], in0=g_in0, in1=cc[:, 0:5, :],
                                        op=OP.mult)
                msg = ps.tile([128, 512], f32, tag="ps")
                for p in range(5):
                    nc.tensor.matmul(msg[:, :ECH], wM_s[:, p, :], sc[:, p, :],
                                     start=(p == 0), stop=(p == 4))
                cum_out = bass.AP(cum.tensor, cum[:].offset + k * ECH * 2,
                                  [cum[:].ap[0], (2, ECH)])
                if k == 0:
                    init = 0.0
                else:
                    init = bass.AP(cum.tensor, cum[:].offset + (k * ECH - 1) * 2,
                                   [cum[:].ap[0], (1, 1)])
                nc.vector.tensor_tensor_scan(out=cum_out, data0=cc[:, 5, :],
                                             data1=msg[:, :ECH], initial=init,
                                             op0=OP.mult, op1=OP.add)

            # ---- D+E: ends + gates ----
            ends_tiles = {}

            def get_ends(ch, _et=ends_tiles):
                if ch in _et:
                    return _et[ch]
                eb = wk.tile([128, SUB, 2], f16, **(BUFA if ch < 4 else BUFB))
                nc.gpsimd.ap_gather(
                    out_ap=eb[:], in_ap=cum[:],
                    idxs_ap=eidx[:, ch * (SUB // 16):(ch + 1) * (SUB // 16)],
                    channels=128, num_elems=EP, d=2, num_idxs=SUB)
                for o in list(_et):
                    if o != ch and (o < 4) == (ch < 4):
                        del _et[o]
                _et[ch] = eb
                return eb

            for c0, L in PIECES_H:
                pr = ps.tile([128, 512], f32, tag="ps")
                pz = ps.tile([128, 512], f32, tag="ps")
                pn = ps.tile([128, 512], f32, tag="ps")
                ph = ps.tile([128, 512], f32, tag="ps")
                for g, pst in enumerate([pr, pz, pn, ph]):
                    nc.tensor.matmul(pst[:, :L], whs_s[:, 4 * j + g, :],
                                     hT[:, c0:c0 + L],
                                     start=True, stop=True)
                for g, pst in enumerate([pr, pz, pn]):
                    for half in range(2):
                        pcs = ends_pieces(half * HALF + c0, L)
                        for i, (ch, eoff, eln) in enumerate(pcs):
                            eb = get_ends(ch)
                            rhs = bass.AP(eb.tensor, eb[:].offset + eoff * 2,
                                          [eb[:].ap[0], (2, eln)])
                            oo = (eoff + ch * SUB) - (half * HALF + c0)
                            out = bass.AP(
                                pst.tensor,
                                pst[:].offset + 64 * half * pst[:].ap[0][0] + oo,
                                [(pst[:].ap[0][0], 64), (1, eln)])
                            tp = (0, 64) if half == 1 else None
                            nc.tensor.matmul(out, wfold_s[:, 3 * j + g, :], rhs,
                                             start=False, stop=False,
                                             skip_group_check=True,
                                             tile_position=tp)
                r16 = wk2.tile([128, 512], f16, tag="g_r")
                z16 = wk2.tile([128, 512], f16, tag="g_z")
                nc.scalar.activation(out=r16[:, :L], in_=pr[:, :L], func=AF.Sigmoid,
                                     bias=biases[:, 4 * j:4 * j + 1], scale=1.0)
                nc.scalar.activation(out=z16[:, :L], in_=pz[:, :L], func=AF.Sigmoid,
                                     bias=biases[:, 4 * j + 1:4 * j + 2], scale=1.0)
                t16 = wk2.tile([128, 512], f16, tag="g_t")
                nc.vector.scalar_tensor_tensor(
                    out=t16[:, :L], in0=ph[:, :L],
                    scalar=biases[:, 4 * j + 3:4 * j + 4], in1=r16[:, :L],
                    op0=OP.add, op1=OP.mult)
                u16 = wk2.tile([128, 512], f16, tag="g_u")
                nc.vector.tensor_tensor(out=u16[:, :L], in0=t16[:, :L],
                                        in1=pn[:, :L], op=OP.add)
                n16 = wk2.tile([128, 512], f16, tag="g_n")
                nc.scalar.activation(out=n16[:, :L], in_=u16[:, :L], func=AF.Tanh,
                                     bias=biases[:, 4 * j + 2:4 * j + 3], scale=1.0)
                v16 = wk2.tile([128, 512], f16, tag="g_t")
                nc.vector.tensor_tensor(out=v16[:, :L], in0=hT[:, c0:c0 + L],
                                        in1=n16[:, :L], op=OP.subtract)
                w16 = wk2.tile([128, 512], f16, tag="g_u")
                nc.vector.tensor_tensor(out=w16[:, :L], in0=z16[:, :L],
                                        in1=v16[:, :L], op=OP.mult)
                nc.vector.tensor_tensor(out=hT[:, c0:c0 + L], in0=n16[:, :L],
                                        in1=w16[:, :L], op=OP.add)

        # ================= FINAL: y + pooling =================
        for c0, L in PIECES_H:
            py = ps.tile([2, 512], f32, tag="ps")
            nc.tensor.matmul(py[:, :L], wy[:], hT[:, c0:c0 + L],
                             start=True, stop=True)
            ystg = wk2.tile([2, 512], f32, **GGT)
            nc.vector.tensor_copy(out=ystg[:, :L], in_=py[:, :L])
            nc.sync.dma_start(out=y_dram[:, c0:c0 + L], in_=ystg[:, :L])
        ypool = wk.tile([128, SUB], f32, **BUFA)
        for half in range(2):
            src = bass.AP(y_dram.tensor, y_dram[:].offset + half * HALF,
                          [(SUB, 4), (0, 16), (1, SUB)])
            nc.sync.dma_start(out=ypool[:][64 * half:64 * (half + 1)], in_=src)
        ycum = wk.tile([128, SUB], f32, **BUFB)
        nc.vector.tensor_tensor_scan(out=ycum[:], data0=pmask[:],
                                     data1=ypool[:], initial=0.0,
                                     op0=OP.mult, op1=OP.add)
        pooled = wk2.tile([128, GSLOT], f32, tag="g_r")
        nc.gpsimd.ap_gather(out_ap=pooled[:], in_ap=ycum[:], idxs_ap=pidx[:],
                            channels=128, num_elems=SUB, d=1, num_idxs=GSLOT)
        nc.sync.dma_start(out=out_d[:], in_=pooled[:][0::16])

    nc.compile()
    return nc


# ================= driver =================
_CACHE = {}


def kernel(**inputs):
    inputs = {k: np.asarray(v) for k, v in inputs.items()}
    in_maps_data, meta = host_prep(inputs)
    w = fold_weights_host(inputs)
    EP = meta["EP"]
    if EP not in _CACHE:
        _CACHE[EP] = build(EP)
    nc = _CACHE[EP]
    from concourse.bass_utils import run_bass_kernel_spmd
    in_maps = []
    for c in range(NC):
        m = dict(in_maps_data[c])
        m.update(w)
        in_maps.append(m)
    trace = os.environ.get("KERNEL_PROFILE", "0") == "1"
    br = run_bass_kernel_spmd(nc, in_maps, list(range(NC)), trace=trace)
    if trace and br.exec_time_ns is not None:
        print(f"HW exec time: {br.exec_time_ns} ns")
    got = np.zeros(NG, np.float32)
    for c in range(NC):
        pooled = br.results[c]["pooled"]
        for s in range(8):
            for i, (g, endpos) in enumerate(meta["pool_graphs"][c][s]):
                got[g] = pooled[s, i]
    return got



# revision 5
# speedup vs baseline: 1.2546x; 1.2546x over previous
"""Trainium2 Bass kernel for nn_Net_76622216561354 (gnn_message_passing).

Self-contained: host-side sharding/index prep (numpy) + an 8-core SPMD
Bass/Tile kernel run via run_bass_kernel_spmd. Accepts FULL inputs, returns
the FULL pooled output [8192] float32.

v2: dst-side aggregation uses gpsimd.local_scatter (streaming, ~4cyc/idx)
instead of ap_gather over run-end positions (~70cyc/idx); edge streams are
re-sorted by (src-block, dst sub-chunk) with per-segment padding so each
scatter call covers a static stream window; per-edge scalar tables and
index tensors ship compact ([8,...]) and are partition-replicated x16 on
device via 0-stride DMA, cutting shipped input bytes ~5x.
"""
import numpy as np
import os

NC = 8
N = 131072; E = 524288; F_IN = 16; DIM = 64; DNN = 16; BK = 4; NG = 8192
NL1 = 4; NL2 = 2
SUB = 2112
NPAD = 8 * SUB          # 16896
HALF = NPAD // 2        # 8448
ECH = 512
GSLOT = 192             # pooled graph slots per sub-chunk (padded)
EBCAP = 2046            # local_scatter num_elems cap (num_elems*32 < 2^16)
EBX = SUB - EBCAP       # 66 tail slots per sub-chunk -> cleanup lane


def host_prep(inputs):
    ei = np.asarray(inputs["edge_index"])
    batch = np.asarray(inputs["batch"]).astype(np.int64)
    src, dst = ei[0].astype(np.int64), ei[1].astype(np.int64)

    # ---- graph spans ----
    gsizes = np.bincount(batch, minlength=NG)
    gstart = np.concatenate([[0], np.cumsum(gsizes)])

    # ---- core cuts at graph boundaries ----
    cuts = [0]
    for c in range(1, NC):
        t = c * (N // NC)
        while t < N and batch[t] == batch[t - 1]:
            t += 1
        cuts.append(t)
    cuts.append(N)
    cuts = np.array(cuts, np.int64)

    # ---- per-core: pack graphs into 8 graph-aligned sub-chunks ----
    g2l = np.full(N, -1, np.int64)      # global node -> local slot (within its core)
    node_core = np.zeros(N, np.int64)
    l2g = [np.full(NPAD, -1, np.int64) for _ in range(NC)]
    pool_graphs = [[[] for _ in range(8)] for _ in range(NC)]
    pool_mask = [np.zeros((8, SUB), np.float32) for _ in range(NC)]

    for c in range(NC):
        lo, hi = cuts[c], cuts[c + 1]
        glo, ghi = batch[lo], (batch[hi - 1] + 1 if hi > lo else batch[lo])
        s = 0; pos = 0
        for g in range(glo, ghi):
            sz = int(gsizes[g])
            if sz == 0:
                continue
            if pos + sz > SUB:
                s += 1; pos = 0
                assert s < 8, f"core {c}: sub-chunk overflow"
                assert sz <= SUB
            nodes = np.arange(gstart[g], gstart[g] + sz)
            slots = s * SUB + pos + np.arange(sz)
            g2l[nodes] = slots
            node_core[nodes] = c
            l2g[c][slots] = nodes
            pool_mask[c][s, pos + 1: pos + sz] = 1.0
            pool_graphs[c][s].append((g, pos + sz - 1))
            pos += sz
        assert hi == lo or batch[hi - 1] + 1 == ghi

    dst_core = node_core[dst]; src_core = node_core[src]
    dstslot = g2l[dst]; srcslot = g2l[src]
    dsub = dstslot // SUB               # dst sub-chunk

    # ---- segment capacity: max edges per (dst core, src block, dst sub) ----
    key = (dst_core * NC + src_core) * 8 + dsub
    segcnt = np.bincount(key, minlength=NC * NC * 8)
    SEGCAP = int(-(-int(segcnt.max()) // 64) * 64)
    EP = 8 * SEGCAP
    nchunk = EP // ECH

    indeg = np.bincount(dst, minlength=N).astype(np.float64)
    inv = 1.0 / np.maximum(indeg, 1.0)
    ea_all = np.asarray(inputs["edge_attr"])

    per_core = []
    for c in range(NC):
        gidx = np.zeros((8, EP), np.int64)       # src slot per stream pos
        craw = np.zeros((8, 6, EP), np.float32)  # inv, ea*4, same-mask
        sidx = np.full((8, 2, EP), -1, np.int64)  # run-end targets: main, clean
        for b in range(NC):
            m = (dst_core == c) & (src_core == b)
            eids = np.nonzero(m)[0]
            order = np.lexsort((dstslot[eids],))
            eids = eids[order]                   # sorted by dstslot => by (dsub, slot)
            dsl = dstslot[eids]
            ds = dsl // SUB
            # position within stream: segment base + rank within segment
            seg_off = np.zeros(len(eids), np.int64)
            for s in range(8):
                sm = ds == s
                k = int(sm.sum())
                assert k <= SEGCAP
                seg_off[sm] = s * SEGCAP + np.arange(k)
            gidx[b, seg_off] = srcslot[eids]
            craw[b, 0, seg_off] = inv[dst[eids]]
            for q in range(BK):
                craw[b, 1 + q, seg_off] = ea_all[eids, q]
            # same-dst continuation mask (within segment; runs never span segs)
            same = np.zeros(len(eids), bool)
            if len(eids) > 0:
                same[1:] = (dsl[1:] == dsl[:-1])
                craw[b, 5, seg_off] = same.astype(np.float32)
                # run ends: last edge of each dst run
                is_end = np.ones(len(eids), bool)
                is_end[:-1] = dsl[1:] != dsl[:-1]
                epos = seg_off[is_end]
                eslot = dsl[is_end] % SUB        # slot within sub-chunk
                main = eslot < EBCAP
                sidx[b, 0, epos[main]] = eslot[main]
                sidx[b, 1, epos[~main]] = eslot[~main] - EBCAP
        per_core.append(dict(gidx=gidx, craw=craw, sidx=sidx))

    # ---- wrap helper: seq -> [16, L/16] with idx[p, s] = seq[s*16+p] ----
    def wrap16(seq):
        L = len(seq)
        assert L % 16 == 0
        return np.asarray(seq).reshape(L // 16, 16).T.copy()

    in_maps = []
    for c in range(NC):
        pc = per_core[c]
        # gather idx: [128, EP/16] int16, wrapped per gather-call window (GCH)
        GCH = 4 * ECH
        gidx_t = np.zeros((128, EP // 16), np.int16)
        for b in range(NC):
            off = 0
            while off < EP:
                L = min(GCH, EP - off)
                seq = pc["gidx"][b, off:off + L]
                gidx_t[16 * b:16 * (b + 1), off // 16:(off + L) // 16] = \
                    wrap16(seq).astype(np.int16)
                off += L
        pidx_t = np.zeros((128, GSLOT // 16), np.int16)
        for s in range(8):
            seq = np.zeros(GSLOT, np.int64)
            gl = pool_graphs[c][s]
            assert len(gl) <= GSLOT, f"GSLOT overflow: {len(gl)}"
            for i, (g, endpos) in enumerate(gl):
                seq[i] = endpos
            pidx_t[16 * s:16 * (s + 1), :] = wrap16(seq).astype(np.int16)
        # x slab transposed [16, NPAD] f16
        xT = np.zeros((16, NPAD), np.float16)
        real = l2g[c] >= 0
        xT[:, real] = np.asarray(inputs["x"])[l2g[c][real]].T.astype(np.float16)
        in_maps.append(dict(
            xT=xT,
            gidx=gidx_t,
            craw=pc["craw"].astype(np.float16),
            sidx=pc["sidx"].astype(np.int16),
            pmask=pool_mask[c].astype(np.float16),
            pidx=pidx_t))

    meta = dict(EP=EP, SEGCAP=SEGCAP, nchunk=nchunk, cuts=cuts,
                pool_graphs=pool_graphs, l2g=l2g)
    return in_maps, meta


def fold_weights_host(inputs):
    """float64 weight folds -> shipped stationaries/biases (per-core identical)."""
    dt = np.float64
    lin0_w = np.asarray(inputs["lin0_w"], dt); lin0_b = np.asarray(inputs["lin0_b"], dt)
    lin1_w = np.asarray(inputs["lin1_w"], dt); lin1_b = np.asarray(inputs["lin1_b"], dt)
    lin2_w = np.asarray(inputs["lin2_w"], dt)
    root_w = np.asarray(inputs["root_w"], dt); conv_b = np.asarray(inputs["conv_b"], dt)
    nn1_w = np.asarray(inputs["nn1_w"], dt); nn1_b = np.asarray(inputs["nn1_b"], dt)
    gw_ih = np.asarray(inputs["gru_w_ih"], dt); gw_hh = np.asarray(inputs["gru_w_hh"], dt)
    gb_ih = np.asarray(inputs["gru_b_ih"], dt); gb_hh = np.asarray(inputs["gru_b_hh"], dt)

    Bm = nn1_b.reshape(DNN, DNN)
    Ak = nn1_w.reshape(BK, DNN, DNN)
    M = np.concatenate([Bm[None], Ak], axis=0)            # [5,16,16]

    w = {}
    # slab stationaries: [8, 128, 128] f16
    wslab = np.zeros((8, 128, 128), np.float32)
    for b in range(8):
        r0 = 64 * (b // 4)
        wslab[b, r0:r0 + 64, 16 * b:16 * (b + 1)] = lin1_w
    w["wslab"] = wslab.astype(np.float16)
    # M stationaries: block-diag-8 [5, 128, 128]
    wM = np.zeros((5, 128, 128), np.float32)
    for p in range(5):
        for b in range(8):
            wM[p, 16 * b:16 * (b + 1), 16 * b:16 * (b + 1)] = M[p]
    w["wM"] = wM.astype(np.float16)
    # gates h-side and folds per layer
    whs = np.zeros((NL1, 4, 128, 128), np.float32)
    wfold = np.zeros((NL1, 3, 128, 64), np.float32)
    biases = np.zeros((128, 17), np.float32)
    for j in range(NL1):
        P = lin1_w @ root_w @ gw_ih[j].T                  # [64,192]
        W_rz = P[:, :2 * DIM] + gw_hh[j].T[:, :2 * DIM]
        W_ni = P[:, 2 * DIM:]
        W_nh = gw_hh[j].T[:, 2 * DIM:]
        grp_w = [W_rz[:, :64], W_rz[:, 64:], W_ni, W_nh]
        for g in range(4):
            whs[j, g, 0:64, 0:64] = grp_w[g]
            whs[j, g, 64:128, 64:128] = grp_w[g]
        wihT = gw_ih[j].T                                  # [16,192]
        for g in range(3):
            blk = wihT[:, 64 * g:64 * (g + 1)]
            wfold[j, g] = np.tile(blk, (8, 1))
        b_base = (lin1_b @ root_w + conv_b) @ gw_ih[j].T   # [192]
        b_rz = b_base[:2 * DIM] + gb_ih[j][:2 * DIM] + gb_hh[j][:2 * DIM]
        b_ni = b_base[2 * DIM:] + gb_ih[j][2 * DIM:]
        b_hn = gb_hh[j][2 * DIM:]
        vec = [b_rz[:64], b_rz[64:], b_ni, b_hn]
        for g in range(4):
            biases[0:64, 4 * j + g] = vec[g]
            biases[64:128, 4 * j + g] = vec[g]
    w["whs"] = whs.astype(np.float16)
    w["wfold"] = wfold.astype(np.float16)
    biases[0:64, 16] = lin0_b
    biases[64:128, 16] = lin0_b
    w["biases"] = biases.astype(np.float32)
    # lin0 stationaries [2, 16, 128] f16
    wlin0 = np.zeros((2, 16, 128), np.float32)
    wlin0[0, :, 0:64] = lin0_w
    wlin0[1, :, 64:128] = lin0_w
    w["wlin0"] = wlin0.astype(np.float16)
    # y stationary [128, 2] f16
    wy = np.zeros((128, 2), np.float32)
    wy[0:64, 0] = lin2_w[:, 0]
    wy[64:128, 1] = lin2_w[:, 0]
    w["wy"] = wy.astype(np.float16)
    return w


# ================= kernel builder =================

import concourse.bass as bass
import concourse.mybir as mybir
import concourse.tile as tile
from concourse import bacc
from contextlib import ExitStack

NITER = 8

f32 = mybir.dt.float32
f16 = mybir.dt.float16
i16 = mybir.dt.int16
AF = mybir.ActivationFunctionType
OP = mybir.AluOpType


def pieces(total, step):
    out = []
    off = 0
    while off < total:
        out.append((off, min(step, total - off)))
        off += step
    return out


def ends_pieces(c0, L):
    """Split node-column window [c0, c0+L) into (sub-chunk, offset, len)."""
    out = []
    while L > 0:
        ch = c0 // SUB
        off = c0 % SUB
        ln = min(L, SUB - off)
        out.append((ch, off, ln))
        c0 += ln
        L -= ln
    return out


def build(EP, fake_collective=False, niter=NITER):
    SEGCAP = EP // 8
    nchunk = EP // ECH
    nc = bacc.Bacc("TRN2", target_bir_lowering=False, debug=False, num_devices=NC)

    xT_d = nc.dram_tensor("xT", [16, NPAD], f16, kind="ExternalInput")
    gidx_d = nc.dram_tensor("gidx", [128, EP // 16], i16, kind="ExternalInput")
    craw_d = nc.dram_tensor("craw", [8, 6, EP], f16, kind="ExternalInput")
    sidx_d = nc.dram_tensor("sidx", [8, 2, EP], i16, kind="ExternalInput")
    pmask_d = nc.dram_tensor("pmask", [8, SUB], f16, kind="ExternalInput")
    pidx_d = nc.dram_tensor("pidx", [128, GSLOT // 16], i16, kind="ExternalInput")
    wslab_d = nc.dram_tensor("wslab", [8, 128, 128], f16, kind="ExternalInput")
    wM_d = nc.dram_tensor("wM", [5, 128, 128], f16, kind="ExternalInput")
    whs_d = nc.dram_tensor("whs", [NL1, 4, 128, 128], f16, kind="ExternalInput")
    wfold_d = nc.dram_tensor("wfold", [NL1, 3, 128, 64], f16, kind="ExternalInput")
    biases_d = nc.dram_tensor("biases", [128, 17], f32, kind="ExternalInput")
    wlin0_d = nc.dram_tensor("wlin0", [2, 16, 128], f16, kind="ExternalInput")
    wy_d = nc.dram_tensor("wy", [128, 2], f16, kind="ExternalInput")
    out_d = nc.dram_tensor("pooled", [8, GSLOT], f32, kind="ExternalOutput")

    PIECES_H = pieces(HALF, 512)
    PIECES_S = pieces(SUB, 512)

    with tile.TileContext(nc) as tc, ExitStack() as ex:
        pp = ex.enter_context(tc.tile_pool(name="persist", bufs=1))
        wk = ex.enter_context(tc.tile_pool(name="work", bufs=2))
        wk2 = ex.enter_context(tc.tile_pool(name="work2", bufs=2))
        ps = ex.enter_context(tc.tile_pool(name="psum", bufs=8, space="PSUM"))
        dr = ex.enter_context(tc.tile_pool(name="dram", bufs=1, space="DRAM"))

        BUFA = dict(tag="bufA")
        BUFB = dict(tag="bufB")
        GGT = dict(tag="gg")

        hT = pp.tile([128, HALF], f16, tag="hT")
        table = pp.tile([128, NPAD, 2], f16, tag="table")
        cum = pp.tile([128, EP], f16, tag="cum")
        nc.vector.memset(cum[:], 0)
        gidx = pp.tile([128, EP // 16], i16, tag="gidx")
        pmask = pp.tile([128, SUB], f16, tag="pmask")
        pidx = pp.tile([128, GSLOT // 16], i16, tag="pidx")
        biases = pp.tile([128, 17], f32, tag="biases")
        wy = pp.tile([128, 2], f16, tag="wy")

        nc.sync.dma_start(out=gidx[:], in_=gidx_d[:])
        nc.sync.dma_start(
            out=pmask[:],
            in_=bass.AP(pmask_d, 0, [(SUB, 8), (0, 16), (1, SUB)]))
        nc.sync.dma_start(out=pidx[:], in_=pidx_d[:])
        nc.sync.dma_start(out=biases[:], in_=biases_d[:])
        nc.sync.dma_start(out=wy[:], in_=wy_d[:])

        wslab_s = pp.tile([128, 8, 128], f16, tag="wslab_s")
        nc.sync.dma_start(
            out=wslab_s[:],
            in_=bass.AP(wslab_d, 0, [(128, 128), (128 * 128, 8), (1, 128)]))
        wM_s = pp.tile([128, 5, 128], f16, tag="wM_s")
        nc.sync.dma_start(
            out=wM_s[:], in_=bass.AP(wM_d, 0, [(128, 128), (128 * 128, 5), (1, 128)]))
        whs_s = pp.tile([128, NL1 * 4, 128], f16, tag="whs_s")
        nc.sync.dma_start(
            out=whs_s[:],
            in_=bass.AP(whs_d, 0, [(128, 128), (128 * 128, NL1 * 4), (1, 128)]))
        wfold_s = pp.tile([128, NL1 * 3, 64], f16, tag="wfold_s")
        nc.sync.dma_start(
            out=wfold_s[:],
            in_=bass.AP(wfold_d, 0, [(64, 128), (128 * 64, NL1 * 3), (1, 64)]))

        slab_dram = dr.tile([128, SUB * 2], f16)
        ag_dram = dr.tile([NC, 128, SUB * 2], f16)
        cexp_dram = dr.tile([nchunk, 128, 6 * ECH], f16)
        sidx_dram = dr.tile([128, 2, EP], i16)
        y_dram = dr.tile([2, HALF], f32)

        # ================= INIT =================
        # sidx expand x16 (DRAM -> DRAM), one DMA per lane (3-dim AP limit)
        for t in range(2):
            dstap = bass.AP(sidx_dram.tensor,
                            sidx_dram[:].offset + t * EP,
                            [sidx_dram[:].ap[0], (1, EP)])
            nc.sync.dma_start(
                out=dstap,
                in_=bass.AP(sidx_d, t * EP, [(2 * EP, 8), (0, 16), (1, EP)]))
        # c-expansion: replicate x16 across partitions, fold inv into ea
        for k in range(nchunk):
            cc = wk.tile([128, 6, ECH], f16, **BUFA)
            for j in range(6):
                nc.sync.dma_start(
                    out=cc[:, j, :],
                    in_=bass.AP(craw_d, j * EP + k * ECH,
                                [(6 * EP, 8), (0, 16), (1, ECH)]))
            ce = wk.tile([128, 6, ECH], f16, **BUFB)
            nc.vector.tensor_copy(out=ce[:, 0, :], in_=cc[:, 0, :])
            in0 = bass.AP(cc.tensor, cc[:].offset, [cc[:].ap[0], (0, 4), (1, ECH)])
            nc.vector.tensor_tensor(out=ce[:, 1:5, :], in0=in0,
                                    in1=cc[:, 1:5, :], op=OP.mult)
            nc.vector.tensor_copy(out=ce[:, 5, :], in_=cc[:, 5, :])
            nc.sync.dma_start(out=cexp_dram[k], in_=ce[:])

        # lin0 -> hT
        wlin0 = pp.tile([16, 2, 128], f16, tag="wlin0")
        nc.sync.dma_start(
            out=wlin0[:],
            in_=bass.AP(wlin0_d, 0, [(128, 16), (16 * 128, 2), (1, 128)]))
        for c0, L in PIECES_H:
            xa = wk2.tile([16, 512], f16, **GGT)
            nc.sync.dma_start(out=xa[:, :L], in_=xT_d[:, c0:c0 + L])
            xb = wk2.tile([16, 512], f16, **GGT)
            nc.sync.dma_start(out=xb[:, :L], in_=xT_d[:, HALF + c0:HALF + c0 + L])
            p0 = ps.tile([128, 512], f32, tag="ps")
            nc.tensor.matmul(p0[:, :L], wlin0[:, 0, :], xa[:, :L],
                             start=True, stop=False)
            nc.tensor.matmul(p0[:, :L], wlin0[:, 1, :], xb[:, :L],
                             start=False, stop=True)
            nc.scalar.activation(out=hT[:, c0:c0 + L], in_=p0[:, :L],
                                 func=AF.Relu, bias=biases[:, 16:17], scale=1.0)

        # ================= ITERATIONS =================
        for it in range(niter):
            j = (it // 2) % NL1
            # ---- A: slab + exchange ----
            for c0, L in PIECES_S:
                p0 = ps.tile([128, 512], f32, tag="ps")
                for b in range(8):
                    rc0 = (b % 4) * SUB + c0
                    nc.tensor.matmul(p0[:, :L], wslab_s[:, b, :],
                                     hT[:, rc0:rc0 + L],
                                     start=(b == 0), stop=(b == 7))
                stg = wk2.tile([128, 1024], f16, tag="slabstg")
                for dup in range(2):
                    dst = bass.AP(stg.tensor, stg[:].offset + dup,
                                  [stg[:].ap[0], (2, L)])
                    nc.vector.tensor_copy(out=dst, in_=p0[:, :L])
                nc.sync.dma_start(out=slab_dram[:, c0 * 2:(c0 + L) * 2],
                                  in_=stg[:, :2 * L])
            if fake_collective:
                for cc_ in range(NC):
                    nc.sync.dma_start(out=ag_dram[cc_], in_=slab_dram[:])
            else:
                nc.gpsimd.collective_compute(
                    "AllGather", OP.bypass,
                    replica_groups=[list(range(NC))],
                    ins=[slab_dram[:].opt()], outs=[ag_dram[:].opt()])
            for s in range(8):
                src = bass.AP(ag_dram.tensor,
                              ag_dram[:].offset + (16 * s) * (SUB * 2),
                              [(128 * SUB * 2, 8), (SUB * 2, 16), (1, SUB * 2)])
                dst = bass.AP(table.tensor, table[:].offset + s * SUB * 2,
                              [table[:].ap[0], (1, SUB * 2)])
                nc.sync.dma_start(out=dst, in_=src)

            # ---- C: edge chunks (gathers batched 4x to amortize ap_gather) ----
            GCH = 4 * ECH
            gbuf = None
            for k in range(nchunk):
                cc = wk.tile([128, 6, ECH], f16, **BUFA)
                nc.sync.dma_start(out=cc[:], in_=cexp_dram[k])
                if k % 4 == 0:
                    G0 = k * ECH
                    GL = min(GCH, EP - G0)
                    gbuf = pp.tile([128, GCH, 2], f16, tag="gbuf")
                    nc.gpsimd.ap_gather(
                        out_ap=gbuf[:, :GL, :], in_ap=table[:],
                        idxs_ap=gidx[:, G0 // 16:(G0 + GL) // 16],
                        channels=128, num_elems=NPAD, d=2, num_idxs=GL)
                sc = wk.tile([128, 5, ECH], f16, **BUFB)
                g_in0 = bass.AP(gbuf.tensor,
                                gbuf[:].offset + (k % 4) * ECH * 2,
                                [gbuf[:].ap[0], (0, 5), (2, ECH)])
                nc.vector.tensor_tensor(out=sc[:], in0=g_in0, in1=cc[:, 0:5, :],
                                        op=OP.mult)
                msg = ps.tile([128, 512], f32, tag="ps")
                for p in range(5):
                    nc.tensor.matmul(msg[:, :ECH], wM_s[:, p, :], sc[:, p, :],
                                     start=(p == 0), stop=(p == 4))
                if k == 0:
                    init = 0.0
                else:
                    init = bass.AP(cum.tensor, cum[:].offset + (k * ECH - 1),
                                   [cum[:].ap[0], (1, 1)])
                nc.vector.tensor_tensor_scan(out=cum[:, k * ECH:(k + 1) * ECH],
                                             data0=cc[:, 5, :],
                                             data1=msg[:, :ECH], initial=init,
                                             op0=OP.mult, op1=OP.add)

            # ---- D: per-segment local_scatter -> eb tiles ----
            ebx = pp.tile([128, 8 * EBX], f16, tag="ebx")
            ebs = pp.tile([128, 8, EBCAP], f16, tag="ebs")
            for s in range(8):
                st = wk.tile([128, 2, SEGCAP], i16, **BUFB)
                nc.sync.dma_start(
                    out=st[:],
                    in_=bass.AP(sidx_dram.tensor,
                                sidx_dram[:].offset + s * SEGCAP,
                                [sidx_dram[:].ap[0], (EP, 2), (1, SEGCAP)]))
                nc.gpsimd.local_scatter(
                    out_ap=ebs[:, s, :],
                    data_ap=cum[:, s * SEGCAP:(s + 1) * SEGCAP],
                    idxs_ap=st[:, 0, :],
                    channels=128, num_elems=EBCAP, num_idxs=SEGCAP)
                nc.gpsimd.local_scatter(
                    out_ap=ebx[:, s * EBX:(s + 1) * EBX],
                    data_ap=cum[:, s * SEGCAP:(s + 1) * SEGCAP],
                    idxs_ap=st[:, 1, :],
                    channels=128, num_elems=EBX, num_idxs=SEGCAP)

            # ---- E: gates ----
            for c0, L in PIECES_H:
                pr = ps.tile([128, 512], f32, tag="ps")
                pz = ps.tile([128, 512], f32, tag="ps")
                pn = ps.tile([128, 512], f32, tag="ps")
                ph = ps.tile([128, 512], f32, tag="ps")
                for g, pst in enumerate([pr, pz, pn, ph]):
                    nc.tensor.matmul(pst[:, :L], whs_s[:, 4 * j + g, :],
                                     hT[:, c0:c0 + L],
                                     start=True, stop=True)
                for g, pst in enumerate([pr, pz, pn]):
                    for half in range(2):
                        pcs = ends_pieces(half * HALF + c0, L)
                        for (ch, eoff, eln) in pcs:
                            # split at the EBCAP boundary within the sub-chunk
                            parts = []
                            if eoff < EBCAP:
                                ln1 = min(eln, EBCAP - eoff)
                                parts.append((ebs, ch * EBCAP + eoff, ln1, eoff))
                            if eoff + eln > EBCAP:
                                o2 = max(eoff, EBCAP)
                                parts.append((ebx, ch * EBX + (o2 - EBCAP),
                                              eoff + eln - o2, o2))
                            for (tile_, toff, tln, nodeoff) in parts:
                                rhs = bass.AP(tile_.tensor,
                                              tile_[:].offset + toff,
                                              [tile_[:].ap[0], (1, tln)])
                                oo = (nodeoff + ch * SUB) - (half * HALF + c0)
                                out = bass.AP(
                                    pst.tensor,
                                    pst[:].offset + 64 * half * pst[:].ap[0][0] + oo,
                                    [(pst[:].ap[0][0], 64), (1, tln)])
                                tp = (0, 64) if half == 1 else None
                                nc.tensor.matmul(out, wfold_s[:, 3 * j + g, :], rhs,
                                                 start=False, stop=False,
                                                 skip_group_check=True,
                                                 tile_position=tp)
                r16 = wk2.tile([128, 512], f16, tag="g_r")
                z16 = wk2.tile([128, 512], f16, tag="g_z")
                nc.scalar.activation(out=r16[:, :L], in_=pr[:, :L], func=AF.Sigmoid,
                                     bias=biases[:, 4 * j:4 * j + 1], scale=1.0)
                nc.scalar.activation(out=z16[:, :L], in_=pz[:, :L], func=AF.Sigmoid,
                                     bias=biases[:, 4 * j + 1:4 * j + 2], scale=1.0)
                t16 = wk2.tile([128, 512], f16, tag="g_t")
                nc.vector.scalar_tensor_tensor(
                    out=t16[:, :L], in0=ph[:, :L],
                    scalar=biases[:, 4 * j + 3:4 * j + 4], in1=r16[:, :L],
                    op0=OP.add, op1=OP.mult)
                u16 = wk2.tile([128, 512], f16, tag="g_u")
                nc.vector.tensor_tensor(out=u16[:, :L], in0=t16[:, :L],
                                        in1=pn[:, :L], op=OP.add)
                n16 = wk2.tile([128, 512], f16, tag="g_n")
                nc.scalar.activation(out=n16[:, :L], in_=u16[:, :L], func=AF.Tanh,
                                     bias=biases[:, 4 * j + 2:4 * j + 3], scale=1.0)
                v16 = wk2.tile([128, 512], f16, tag="g_t")
                nc.vector.tensor_tensor(out=v16[:, :L], in0=hT[:, c0:c0 + L],
                                        in1=n16[:, :L], op=OP.subtract)
                w16 = wk2.tile([128, 512], f16, tag="g_u")
                nc.vector.tensor_tensor(out=w16[:, :L], in0=z16[:, :L],
                                        in1=v16[:, :L], op=OP.mult)
                nc.vector.tensor_tensor(out=hT[:, c0:c0 + L], in0=n16[:, :L],
                                        in1=w16[:, :L], op=OP.add)

        # ================= FINAL: y + pooling =================
        for c0, L in PIECES_H:
            py = ps.tile([2, 512], f32, tag="ps")
            nc.tensor.matmul(py[:, :L], wy[:], hT[:, c0:c0 + L],
                             start=True, stop=True)
            ystg = wk2.tile([2, 512], f32, **GGT)
            nc.vector.tensor_copy(out=ystg[:, :L], in_=py[:, :L])
            nc.sync.dma_start(out=y_dram[:, c0:c0 + L], in_=ystg[:, :L])
        ypool = wk.tile([128, SUB], f32, **BUFA)
        for half in range(2):
            src = bass.AP(y_dram.tensor, y_dram[:].offset + half * HALF,
                          [(SUB, 4), (0, 16), (1, SUB)])
            nc.sync.dma_start(out=ypool[:][64 * half:64 * (half + 1)], in_=src)
        ycum = wk.tile([128, SUB], f32, **BUFB)
        nc.vector.tensor_tensor_scan(out=ycum[:], data0=pmask[:],
                                     data1=ypool[:], initial=0.0,
                                     op0=OP.mult, op1=OP.add)
        pooled = wk2.tile([128, GSLOT], f32, tag="g_r")
        nc.gpsimd.ap_gather(out_ap=pooled[:], in_ap=ycum[:], idxs_ap=pidx[:],
                            channels=128, num_elems=SUB, d=1, num_idxs=GSLOT)
        nc.sync.dma_start(out=out_d[:], in_=pooled[:][0::16])

    nc.compile()
    return nc


# ================= driver =================
_CACHE = {}


def kernel(**inputs):
    inputs = {k: np.asarray(v) for k, v in inputs.items()}
    in_maps_data, meta = host_prep(inputs)
    w = fold_weights_host(inputs)
    EP = meta["EP"]
    if EP not in _CACHE:
        _CACHE[EP] = build(EP)
    nc = _CACHE[EP]
    from concourse.bass_utils import run_bass_kernel_spmd
    in_maps = []
    for c in range(NC):
        m = dict(in_maps_data[c])
        m.update(w)
        in_maps.append(m)
    trace = os.environ.get("KERNEL_PROFILE", "0") == "1"
    br = run_bass_kernel_spmd(nc, in_maps, list(range(NC)), trace=trace)
    if trace and br.exec_time_ns is not None:
        print(f"HW exec time: {br.exec_time_ns} ns")
    got = np.zeros(NG, np.float32)
    for c in range(NC):
        pooled = br.results[c]["pooled"]
        for s in range(8):
            for i, (g, endpos) in enumerate(meta["pool_graphs"][c][s]):
                got[g] = pooled[s, i]
    return got


# revision 7
# speedup vs baseline: 1.3056x; 1.0406x over previous
"""Trainium2 Bass kernel for nn_Net_76622216561354 (gnn_message_passing).

Self-contained: host-side sharding/index prep (numpy) + an 8-core SPMD
Bass/Tile kernel run via run_bass_kernel_spmd. Accepts FULL inputs, returns
the FULL pooled output [8192] float32.

v2: dst-side aggregation uses gpsimd.local_scatter (streaming, ~4cyc/idx)
instead of ap_gather over run-end positions (~70cyc/idx); edge streams are
re-sorted by (src-block, dst sub-chunk) with per-segment padding so each
scatter call covers a static stream window; per-edge scalar tables and
index tensors ship compact ([8,...]) and are partition-replicated x16 on
device via 0-stride DMA, cutting shipped input bytes ~5x.
"""
import numpy as np
import os

NC = 8
N = 131072; E = 524288; F_IN = 16; DIM = 64; DNN = 16; BK = 4; NG = 8192
NL1 = 4; NL2 = 2
SUB = 2112
NPAD = 8 * SUB          # 16896
HALF = NPAD // 2        # 8448
ECH = 512
GSLOT = 192             # pooled graph slots per sub-chunk (padded)
EBCAP = 2046            # local_scatter num_elems cap (num_elems*32 < 2^16)
EBX = SUB - EBCAP       # 66 tail slots per sub-chunk -> cleanup lane
# dst sub-chunk segment order in the edge stream: half-pairs (q, 4+q)
# complete early so gates can start before the full scan pipeline drains
SEG_ORDER = [0, 4, 1, 5, 2, 6, 3, 7]
SEG_RANK = {s: r for r, s in enumerate(SEG_ORDER)}


def host_prep(inputs):
    ei = np.asarray(inputs["edge_index"])
    batch = np.asarray(inputs["batch"]).astype(np.int64)
    src, dst = ei[0].astype(np.int64), ei[1].astype(np.int64)

    # ---- graph spans ----
    gsizes = np.bincount(batch, minlength=NG)
    gstart = np.concatenate([[0], np.cumsum(gsizes)])

    # ---- core cuts at graph boundaries ----
    cuts = [0]
    for c in range(1, NC):
        t = c * (N // NC)
        while t < N and batch[t] == batch[t - 1]:
            t += 1
        cuts.append(t)
    cuts.append(N)
    cuts = np.array(cuts, np.int64)

    # ---- per-core: pack graphs into 8 graph-aligned sub-chunks ----
    g2l = np.full(N, -1, np.int64)      # global node -> local slot (within its core)
    node_core = np.zeros(N, np.int64)
    l2g = [np.full(NPAD, -1, np.int64) for _ in range(NC)]
    pool_graphs = [[[] for _ in range(8)] for _ in range(NC)]
    pool_mask = [np.zeros((8, SUB), np.float32) for _ in range(NC)]

    for c in range(NC):
        lo, hi = cuts[c], cuts[c + 1]
        glo, ghi = batch[lo], (batch[hi - 1] + 1 if hi > lo else batch[lo])
        s = 0; pos = 0
        for g in range(glo, ghi):
            sz = int(gsizes[g])
            if sz == 0:
                continue
            if pos + sz > SUB:
                s += 1; pos = 0
                assert s < 8, f"core {c}: sub-chunk overflow"
                assert sz <= SUB
            nodes = np.arange(gstart[g], gstart[g] + sz)
            slots = s * SUB + pos + np.arange(sz)
            g2l[nodes] = slots
            node_core[nodes] = c
            l2g[c][slots] = nodes
            pool_mask[c][s, pos + 1: pos + sz] = 1.0
            pool_graphs[c][s].append((g, pos + sz - 1))
            pos += sz
        assert hi == lo or batch[hi - 1] + 1 == ghi

    dst_core = node_core[dst]; src_core = node_core[src]
    dstslot = g2l[dst]; srcslot = g2l[src]
    dsub = dstslot // SUB               # dst sub-chunk

    # ---- segment capacity: max edges per (dst core, src block, dst sub) ----
    key = (dst_core * NC + src_core) * 8 + dsub
    segcnt = np.bincount(key, minlength=NC * NC * 8)
    SEGCAP = int(-(-int(segcnt.max()) // 64) * 64)
    EP = 8 * SEGCAP
    nchunk = EP // ECH

    indeg = np.bincount(dst, minlength=N).astype(np.float64)
    inv = 1.0 / np.maximum(indeg, 1.0)
    ea_all = np.asarray(inputs["edge_attr"])

    per_core = []
    for c in range(NC):
        gidx = np.zeros((8, EP), np.int64)       # src slot per stream pos
        craw = np.zeros((8, 6, EP), np.float32)  # inv, ea*4, same-mask
        sidx = np.full((8, 2, EP), -1, np.int64)  # run-end targets: main, clean
        for b in range(NC):
            m = (dst_core == c) & (src_core == b)
            eids = np.nonzero(m)[0]
            order = np.lexsort((dstslot[eids],))
            eids = eids[order]                   # sorted by dstslot => by (dsub, slot)
            dsl = dstslot[eids]
            ds = dsl // SUB
            # position within stream: segment base + rank within segment
            seg_off = np.zeros(len(eids), np.int64)
            for s in range(8):
                sm = ds == s
                k = int(sm.sum())
                assert k <= SEGCAP
                seg_off[sm] = SEG_RANK[s] * SEGCAP + np.arange(k)
            gidx[b, seg_off] = srcslot[eids]
            craw[b, 0, seg_off] = inv[dst[eids]]
            for q in range(BK):
                craw[b, 1 + q, seg_off] = ea_all[eids, q]
            # same-dst continuation mask (within segment; runs never span segs)
            same = np.zeros(len(eids), bool)
            if len(eids) > 0:
                same[1:] = (dsl[1:] == dsl[:-1])
                craw[b, 5, seg_off] = same.astype(np.float32)
                # run ends: last edge of each dst run
                is_end = np.ones(len(eids), bool)
                is_end[:-1] = dsl[1:] != dsl[:-1]
                epos = seg_off[is_end]
                eslot = dsl[is_end] % SUB        # slot within sub-chunk
                main = eslot < EBCAP
                sidx[b, 0, epos[main]] = eslot[main]
                sidx[b, 1, epos[~main]] = eslot[~main] - EBCAP
        per_core.append(dict(gidx=gidx, craw=craw, sidx=sidx))

    # ---- wrap helper: seq -> [16, L/16] with idx[p, s] = seq[s*16+p] ----
    def wrap16(seq):
        L = len(seq)
        assert L % 16 == 0
        return np.asarray(seq).reshape(L // 16, 16).T.copy()

    in_maps = []
    for c in range(NC):
        pc = per_core[c]
        # gather idx: [128, EP/16] int16, wrapped per gather-call window (GCH)
        GCH = 4 * ECH
        gidx_t = np.zeros((128, EP // 16), np.int16)
        for b in range(NC):
            off = 0
            while off < EP:
                L = min(GCH, EP - off)
                seq = pc["gidx"][b, off:off + L]
                gidx_t[16 * b:16 * (b + 1), off // 16:(off + L) // 16] = \
                    wrap16(seq).astype(np.int16)
                off += L
        pidx_t = np.zeros((128, GSLOT // 16), np.int16)
        for s in range(8):
            seq = np.zeros(GSLOT, np.int64)
            gl = pool_graphs[c][s]
            assert len(gl) <= GSLOT, f"GSLOT overflow: {len(gl)}"
            for i, (g, endpos) in enumerate(gl):
                seq[i] = endpos
            pidx_t[16 * s:16 * (s + 1), :] = wrap16(seq).astype(np.int16)
        # x slab transposed [16, NPAD] f16
        xT = np.zeros((16, NPAD), np.float16)
        real = l2g[c] >= 0
        xT[:, real] = np.asarray(inputs["x"])[l2g[c][real]].T.astype(np.float16)
        in_maps.append(dict(
            xT=xT,
            gidx=gidx_t,
            craw=pc["craw"].astype(np.float16),
            sidx=pc["sidx"].astype(np.int16),
            pmask=pool_mask[c].astype(np.float16),
            pidx=pidx_t))

    meta = dict(EP=EP, SEGCAP=SEGCAP, nchunk=nchunk, cuts=cuts,
                pool_graphs=pool_graphs, l2g=l2g)
    return in_maps, meta


def fold_weights_host(inputs):
    """float64 weight folds -> shipped stationaries/biases (per-core identical)."""
    dt = np.float64
    lin0_w = np.asarray(inputs["lin0_w"], dt); lin0_b = np.asarray(inputs["lin0_b"], dt)
    lin1_w = np.asarray(inputs["lin1_w"], dt); lin1_b = np.asarray(inputs["lin1_b"], dt)
    lin2_w = np.asarray(inputs["lin2_w"], dt)
    root_w = np.asarray(inputs["root_w"], dt); conv_b = np.asarray(inputs["conv_b"], dt)
    nn1_w = np.asarray(inputs["nn1_w"], dt); nn1_b = np.asarray(inputs["nn1_b"], dt)
    gw_ih = np.asarray(inputs["gru_w_ih"], dt); gw_hh = np.asarray(inputs["gru_w_hh"], dt)
    gb_ih = np.asarray(inputs["gru_b_ih"], dt); gb_hh = np.asarray(inputs["gru_b_hh"], dt)

    Bm = nn1_b.reshape(DNN, DNN)
    Ak = nn1_w.reshape(BK, DNN, DNN)
    M = np.concatenate([Bm[None], Ak], axis=0)            # [5,16,16]

    w = {}
    # slab stationaries: [8, 128, 128] f16
    wslab = np.zeros((8, 128, 128), np.float32)
    for b in range(8):
        r0 = 64 * (b // 4)
        wslab[b, r0:r0 + 64, 16 * b:16 * (b + 1)] = lin1_w
    w["wslab"] = wslab.astype(np.float16)
    # M stationaries: block-diag-8 [5, 128, 128]
    wM = np.zeros((5, 128, 128), np.float32)
    for p in range(5):
        for b in range(8):
            wM[p, 16 * b:16 * (b + 1), 16 * b:16 * (b + 1)] = M[p]
    w["wM"] = wM.astype(np.float16)
    # gates h-side and folds per layer
    whs = np.zeros((NL1, 4, 128, 128), np.float32)
    wfold = np.zeros((NL1, 3, 128, 64), np.float32)
    biases = np.zeros((128, 17), np.float32)
    for j in range(NL1):
        P = lin1_w @ root_w @ gw_ih[j].T                  # [64,192]
        W_rz = P[:, :2 * DIM] + gw_hh[j].T[:, :2 * DIM]
        W_ni = P[:, 2 * DIM:]
        W_nh = gw_hh[j].T[:, 2 * DIM:]
        grp_w = [W_rz[:, :64], W_rz[:, 64:], W_ni, W_nh]
        for g in range(4):
            whs[j, g, 0:64, 0:64] = grp_w[g]
            whs[j, g, 64:128, 64:128] = grp_w[g]
        wihT = gw_ih[j].T                                  # [16,192]
        for g in range(3):
            blk = wihT[:, 64 * g:64 * (g + 1)]
            wfold[j, g] = np.tile(blk, (8, 1))
        b_base = (lin1_b @ root_w + conv_b) @ gw_ih[j].T   # [192]
        b_rz = b_base[:2 * DIM] + gb_ih[j][:2 * DIM] + gb_hh[j][:2 * DIM]
        b_ni = b_base[2 * DIM:] + gb_ih[j][2 * DIM:]
        b_hn = gb_hh[j][2 * DIM:]
        vec = [b_rz[:64], b_rz[64:], b_ni, b_hn]
        for g in range(4):
            biases[0:64, 4 * j + g] = vec[g]
            biases[64:128, 4 * j + g] = vec[g]
    w["whs"] = whs.astype(np.float16)
    w["wfold"] = wfold.astype(np.float16)
    biases[0:64, 16] = lin0_b
    biases[64:128, 16] = lin0_b
    w["biases"] = biases.astype(np.float32)
    # lin0 stationaries [2, 16, 128] f16
    wlin0 = np.zeros((2, 16, 128), np.float32)
    wlin0[0, :, 0:64] = lin0_w
    wlin0[1, :, 64:128] = lin0_w
    w["wlin0"] = wlin0.astype(np.float16)
    # y stationary [128, 2] f16
    wy = np.zeros((128, 2), np.float32)
    wy[0:64, 0] = lin2_w[:, 0]
    wy[64:128, 1] = lin2_w[:, 0]
    w["wy"] = wy.astype(np.float16)
    return w


# ================= kernel builder =================

import concourse.bass as bass
import concourse.mybir as mybir
import concourse.tile as tile
from concourse import bacc
from contextlib import ExitStack

NITER = 8

f32 = mybir.dt.float32
f16 = mybir.dt.float16
i16 = mybir.dt.int16
AF = mybir.ActivationFunctionType
OP = mybir.AluOpType


def pieces(total, step):
    out = []
    off = 0
    while off < total:
        out.append((off, min(step, total - off)))
        off += step
    return out


def ends_pieces(c0, L):
    """Split node-column window [c0, c0+L) into (sub-chunk, offset, len)."""
    out = []
    while L > 0:
        ch = c0 // SUB
        off = c0 % SUB
        ln = min(L, SUB - off)
        out.append((ch, off, ln))
        c0 += ln
        L -= ln
    return out


def build(EP, fake_collective=False, niter=NITER, skip=()):
    SEGCAP = EP // 8
    nchunk = EP // ECH
    nc = bacc.Bacc("TRN2", target_bir_lowering=False, debug=False, num_devices=NC)

    xT_d = nc.dram_tensor("xT", [16, NPAD], f16, kind="ExternalInput")
    gidx_d = nc.dram_tensor("gidx", [128, EP // 16], i16, kind="ExternalInput")
    craw_d = nc.dram_tensor("craw", [8, 6, EP], f16, kind="ExternalInput")
    sidx_d = nc.dram_tensor("sidx", [8, 2, EP], i16, kind="ExternalInput")
    pmask_d = nc.dram_tensor("pmask", [8, SUB], f16, kind="ExternalInput")
    pidx_d = nc.dram_tensor("pidx", [128, GSLOT // 16], i16, kind="ExternalInput")
    wslab_d = nc.dram_tensor("wslab", [8, 128, 128], f16, kind="ExternalInput")
    wM_d = nc.dram_tensor("wM", [5, 128, 128], f16, kind="ExternalInput")
    whs_d = nc.dram_tensor("whs", [NL1, 4, 128, 128], f16, kind="ExternalInput")
    wfold_d = nc.dram_tensor("wfold", [NL1, 3, 128, 64], f16, kind="ExternalInput")
    biases_d = nc.dram_tensor("biases", [128, 17], f32, kind="ExternalInput")
    wlin0_d = nc.dram_tensor("wlin0", [2, 16, 128], f16, kind="ExternalInput")
    wy_d = nc.dram_tensor("wy", [128, 2], f16, kind="ExternalInput")
    out_d = nc.dram_tensor("pooled", [8, GSLOT], f32, kind="ExternalOutput")

    PIECES_H = pieces(HALF, 512)
    PIECES_S = pieces(SUB, 512)

    with tile.TileContext(nc) as tc, ExitStack() as ex:
        pp = ex.enter_context(tc.tile_pool(name="persist", bufs=1))
        wk = ex.enter_context(tc.tile_pool(name="work", bufs=2))
        wk2 = ex.enter_context(tc.tile_pool(name="work2", bufs=2))
        ps = ex.enter_context(tc.tile_pool(name="psum", bufs=8, space="PSUM"))
        dr = ex.enter_context(tc.tile_pool(name="dram", bufs=1, space="DRAM"))

        BUFA = dict(tag="bufA")
        BUFB = dict(tag="bufB")
        GGT = dict(tag="gg")

        hT = pp.tile([128, HALF], f16, tag="hT")
        table = pp.tile([128, NPAD, 2], f16, tag="table")
        cum = pp.tile([128, EP], f16, tag="cum")
        nc.vector.memset(cum[:], 0)
        gidx = pp.tile([128, EP // 16], i16, tag="gidx")
        pmask = pp.tile([128, SUB], f16, tag="pmask")
        pidx = pp.tile([128, GSLOT // 16], i16, tag="pidx")
        biases = pp.tile([128, 17], f32, tag="biases")
        wy = pp.tile([128, 2], f16, tag="wy")

        nc.sync.dma_start(out=gidx[:], in_=gidx_d[:])
        nc.sync.dma_start(
            out=pmask[:],
            in_=bass.AP(pmask_d, 0, [(SUB, 8), (0, 16), (1, SUB)]))
        nc.sync.dma_start(out=pidx[:], in_=pidx_d[:])
        nc.sync.dma_start(out=biases[:], in_=biases_d[:])
        nc.sync.dma_start(out=wy[:], in_=wy_d[:])

        wslab_s = pp.tile([128, 8, 128], f16, tag="wslab_s")
        nc.sync.dma_start(
            out=wslab_s[:],
            in_=bass.AP(wslab_d, 0, [(128, 128), (128 * 128, 8), (1, 128)]))
        wM_s = pp.tile([128, 5, 128], f16, tag="wM_s")
        nc.sync.dma_start(
            out=wM_s[:], in_=bass.AP(wM_d, 0, [(128, 128), (128 * 128, 5), (1, 128)]))
        whs_s = pp.tile([128, NL1 * 4, 128], f16, tag="whs_s")
        nc.sync.dma_start(
            out=whs_s[:],
            in_=bass.AP(whs_d, 0, [(128, 128), (128 * 128, NL1 * 4), (1, 128)]))
        wfold_s = pp.tile([128, NL1 * 3, 64], f16, tag="wfold_s")
        nc.sync.dma_start(
            out=wfold_s[:],
            in_=bass.AP(wfold_d, 0, [(64, 128), (128 * 64, NL1 * 3), (1, 64)]))

        slab_dram = dr.tile([128, SUB * 2], f16)
        ag_dram = dr.tile([NC, 128, SUB * 2], f16)
        cexp_dram = dr.tile([nchunk, 128, 6 * ECH], f16)
        sidx_dram = dr.tile([128, 2, EP], i16)
        y_dram = dr.tile([2, HALF], f32)

        # ================= INIT =================
        # sidx expand x16 (DRAM -> DRAM), one DMA per lane (3-dim AP limit)
        for t in range(2):
            dstap = bass.AP(sidx_dram.tensor,
                            sidx_dram[:].offset + t * EP,
                            [sidx_dram[:].ap[0], (1, EP)])
            nc.sync.dma_start(
                out=dstap,
                in_=bass.AP(sidx_d, t * EP, [(2 * EP, 8), (0, 16), (1, EP)]))
        # c-expansion: replicate x16 across partitions, fold inv into ea
        for k in range(nchunk):
            cc = wk.tile([128, 6, ECH], f16, **BUFA)
            for j in range(6):
                nc.sync.dma_start(
                    out=cc[:, j, :],
                    in_=bass.AP(craw_d, j * EP + k * ECH,
                                [(6 * EP, 8), (0, 16), (1, ECH)]))
            ce = wk.tile([128, 6, ECH], f16, **BUFB)
            nc.vector.tensor_copy(out=ce[:, 0, :], in_=cc[:, 0, :])
            in0 = bass.AP(cc.tensor, cc[:].offset, [cc[:].ap[0], (0, 4), (1, ECH)])
            nc.vector.tensor_tensor(out=ce[:, 1:5, :], in0=in0,
                                    in1=cc[:, 1:5, :], op=OP.mult)
            nc.vector.tensor_copy(out=ce[:, 5, :], in_=cc[:, 5, :])
            nc.sync.dma_start(out=cexp_dram[k], in_=ce[:])

        # lin0 -> hT
        wlin0 = pp.tile([16, 2, 128], f16, tag="wlin0")
        nc.sync.dma_start(
            out=wlin0[:],
            in_=bass.AP(wlin0_d, 0, [(128, 16), (16 * 128, 2), (1, 128)]))
        for c0, L in PIECES_H:
            xa = wk2.tile([16, 512], f16, **GGT)
            nc.sync.dma_start(out=xa[:, :L], in_=xT_d[:, c0:c0 + L])
            xb = wk2.tile([16, 512], f16, **GGT)
            nc.sync.dma_start(out=xb[:, :L], in_=xT_d[:, HALF + c0:HALF + c0 + L])
            p0 = ps.tile([128, 512], f32, tag="ps")
            nc.tensor.matmul(p0[:, :L], wlin0[:, 0, :], xa[:, :L],
                             start=True, stop=False)
            nc.tensor.matmul(p0[:, :L], wlin0[:, 1, :], xb[:, :L],
                             start=False, stop=True)
            nc.scalar.activation(out=hT[:, c0:c0 + L], in_=p0[:, :L],
                                 func=AF.Relu, bias=biases[:, 16:17], scale=1.0)

        # ================= ITERATIONS =================
        for it in range(niter):
            j = (it // 2) % NL1
            # ---- A: slab + exchange ----
            for c0, L in PIECES_S:
                p0 = ps.tile([128, 512], f32, tag="ps")
                for b in range(8):
                    rc0 = (b % 4) * SUB + c0
                    nc.tensor.matmul(p0[:, :L], wslab_s[:, b, :],
                                     hT[:, rc0:rc0 + L],
                                     start=(b == 0), stop=(b == 7))
                stg = wk2.tile([128, 1024], f16, tag="slabstg")
                for dup in range(2):
                    dst = bass.AP(stg.tensor, stg[:].offset + dup,
                                  [stg[:].ap[0], (2, L)])
                    nc.vector.tensor_copy(out=dst, in_=p0[:, :L])
                nc.sync.dma_start(out=slab_dram[:, c0 * 2:(c0 + L) * 2],
                                  in_=stg[:, :2 * L])
            if fake_collective:
                for cc_ in range(NC):
                    nc.sync.dma_start(out=ag_dram[cc_], in_=slab_dram[:])
            else:
                nc.gpsimd.collective_compute(
                    "AllGather", OP.bypass,
                    replica_groups=[list(range(NC))],
                    ins=[slab_dram[:].opt()], outs=[ag_dram[:].opt()])
            for s in range(8):
                src = bass.AP(ag_dram.tensor,
                              ag_dram[:].offset + (16 * s) * (SUB * 2),
                              [(128 * SUB * 2, 8), (SUB * 2, 16), (1, SUB * 2)])
                dst = bass.AP(table.tensor, table[:].offset + s * SUB * 2,
                              [table[:].ap[0], (1, SUB * 2)])
                nc.sync.dma_start(out=dst, in_=src)

            # ---- C: edge chunks (gathers batched 4x to amortize ap_gather) ----
            GCH = 4 * ECH
            gbuf = None
            for k in range(nchunk):
                cc = wk.tile([128, 6, ECH], f16, **BUFA)
                nc.sync.dma_start(out=cc[:], in_=cexp_dram[k])
                if k % 4 == 0:
                    G0 = k * ECH
                    GL = min(GCH, EP - G0)
                    gbuf = pp.tile([128, GCH, 2], f16, tag="gbuf")
                    if "gather" not in skip:
                        nc.gpsimd.ap_gather(
                            out_ap=gbuf[:, :GL, :], in_ap=table[:],
                            idxs_ap=gidx[:, G0 // 16:(G0 + GL) // 16],
                            channels=128, num_elems=NPAD, d=2, num_idxs=GL)
                    else:
                        nc.vector.memset(gbuf[:, 0:4, :], 0)
                sc = wk.tile([128, 5, ECH], f16, **BUFB)
                g_in0 = bass.AP(gbuf.tensor,
                                gbuf[:].offset + (k % 4) * ECH * 2,
                                [gbuf[:].ap[0], (0, 5), (2, ECH)])
                nc.vector.tensor_tensor(out=sc[:], in0=g_in0, in1=cc[:, 0:5, :],
                                        op=OP.mult)
                msg = ps.tile([128, 512], f32, tag="ps")
                for p in range(5):
                    nc.tensor.matmul(msg[:, :ECH], wM_s[:, p, :], sc[:, p, :],
                                     start=(p == 0), stop=(p == 4))
                if k == 0:
                    init = 0.0
                else:
                    init = bass.AP(cum.tensor, cum[:].offset + (k * ECH - 1),
                                   [cum[:].ap[0], (1, 1)])
                nc.vector.tensor_tensor_scan(out=cum[:, k * ECH:(k + 1) * ECH],
                                             data0=cc[:, 5, :],
                                             data1=msg[:, :ECH], initial=init,
                                             op0=OP.mult, op1=OP.add)

            # ---- D: per-segment local_scatter -> eb tiles ----
            ebx = pp.tile([128, 8 * EBX], f16, tag="ebx")
            ebs = pp.tile([128, 8, EBCAP], f16, tag="ebs")
            for r, s in enumerate(SEG_ORDER):
                st = wk.tile([128, 2, SEGCAP], i16, **BUFB)
                nc.sync.dma_start(
                    out=st[:],
                    in_=bass.AP(sidx_dram.tensor,
                                sidx_dram[:].offset + r * SEGCAP,
                                [sidx_dram[:].ap[0], (EP, 2), (1, SEGCAP)]))
                if "scatter" not in skip:
                    nc.gpsimd.local_scatter(
                        out_ap=ebs[:, s, :],
                        data_ap=cum[:, r * SEGCAP:(r + 1) * SEGCAP],
                        idxs_ap=st[:, 0, :],
                        channels=128, num_elems=EBCAP, num_idxs=SEGCAP)
                    nc.gpsimd.local_scatter(
                        out_ap=ebx[:, s * EBX:(s + 1) * EBX],
                        data_ap=cum[:, r * SEGCAP:(r + 1) * SEGCAP],
                        idxs_ap=st[:, 1, :],
                        channels=128, num_elems=EBX, num_idxs=SEGCAP)
                else:
                    nc.vector.memset(ebs[:, s, :], 0)
                    nc.vector.memset(ebx[:, s * EBX:(s + 1) * EBX], 0)

            # ---- E: gates (q-major: sub-chunk pair (q, 4+q) first) ----
            PIECES_Q = [(q * SUB + off, L) for q in range(4) for off, L in PIECES_S]
            for c0, L in PIECES_Q:
                pr = ps.tile([128, 512], f32, tag="ps")
                pz = ps.tile([128, 512], f32, tag="ps")
                pn = ps.tile([128, 512], f32, tag="ps")
                ph = ps.tile([128, 512], f32, tag="ps")
                for g, pst in enumerate([pr, pz, pn, ph]):
                    nc.tensor.matmul(pst[:, :L], whs_s[:, 4 * j + g, :],
                                     hT[:, c0:c0 + L],
                                     start=True, stop=True)
                for g, pst in enumerate([pr, pz, pn]):
                    for half in range(2):
                        pcs = ends_pieces(half * HALF + c0, L)
                        for (ch, eoff, eln) in pcs:
                            # split at the EBCAP boundary within the sub-chunk
                            parts = []
                            if eoff < EBCAP:
                                ln1 = min(eln, EBCAP - eoff)
                                parts.append((ebs, ch * EBCAP + eoff, ln1, eoff))
                            if eoff + eln > EBCAP:
                                o2 = max(eoff, EBCAP)
                                parts.append((ebx, ch * EBX + (o2 - EBCAP),
                                              eoff + eln - o2, o2))
                            for (tile_, toff, tln, nodeoff) in parts:
                                rhs = bass.AP(tile_.tensor,
                                              tile_[:].offset + toff,
                                              [tile_[:].ap[0], (1, tln)])
                                oo = (nodeoff + ch * SUB) - (half * HALF + c0)
                                out = bass.AP(
                                    pst.tensor,
                                    pst[:].offset + 64 * half * pst[:].ap[0][0] + oo,
                                    [(pst[:].ap[0][0], 64), (1, tln)])
                                tp = (0, 64) if half == 1 else None
                                nc.tensor.matmul(out, wfold_s[:, 3 * j + g, :], rhs,
                                                 start=False, stop=False,
                                                 skip_group_check=True,
                                                 tile_position=tp)
                r16 = wk2.tile([128, 512], f16, tag="g_r")
                z16 = wk2.tile([128, 512], f16, tag="g_z")
                nc.scalar.activation(out=r16[:, :L], in_=pr[:, :L], func=AF.Sigmoid,
                                     bias=biases[:, 4 * j:4 * j + 1], scale=1.0)
                nc.scalar.activation(out=z16[:, :L], in_=pz[:, :L], func=AF.Sigmoid,
                                     bias=biases[:, 4 * j + 1:4 * j + 2], scale=1.0)
                t16 = wk2.tile([128, 512], f16, tag="g_t")
                nc.vector.scalar_tensor_tensor(
                    out=t16[:, :L], in0=ph[:, :L],
                    scalar=biases[:, 4 * j + 3:4 * j + 4], in1=r16[:, :L],
                    op0=OP.add, op1=OP.mult)
                u16 = wk2.tile([128, 512], f16, tag="g_u")
                nc.vector.tensor_tensor(out=u16[:, :L], in0=t16[:, :L],
                                        in1=pn[:, :L], op=OP.add)
                n16 = wk2.tile([128, 512], f16, tag="g_n")
                nc.scalar.activation(out=n16[:, :L], in_=u16[:, :L], func=AF.Tanh,
                                     bias=biases[:, 4 * j + 2:4 * j + 3], scale=1.0)
                v16 = wk2.tile([128, 512], f16, tag="g_t")
                nc.vector.tensor_tensor(out=v16[:, :L], in0=hT[:, c0:c0 + L],
                                        in1=n16[:, :L], op=OP.subtract)
                w16 = wk2.tile([128, 512], f16, tag="g_u")
                nc.vector.tensor_tensor(out=w16[:, :L], in0=z16[:, :L],
                                        in1=v16[:, :L], op=OP.mult)
                nc.vector.tensor_tensor(out=hT[:, c0:c0 + L], in0=n16[:, :L],
                                        in1=w16[:, :L], op=OP.add)

        # ================= FINAL: y + pooling =================
        for c0, L in PIECES_H:
            py = ps.tile([2, 512], f32, tag="ps")
            nc.tensor.matmul(py[:, :L], wy[:], hT[:, c0:c0 + L],
                             start=True, stop=True)
            ystg = wk2.tile([2, 512], f32, **GGT)
            nc.vector.tensor_copy(out=ystg[:, :L], in_=py[:, :L])
            nc.sync.dma_start(out=y_dram[:, c0:c0 + L], in_=ystg[:, :L])
        ypool = wk.tile([128, SUB], f32, **BUFA)
        for half in range(2):
            src = bass.AP(y_dram.tensor, y_dram[:].offset + half * HALF,
                          [(SUB, 4), (0, 16), (1, SUB)])
            nc.sync.dma_start(out=ypool[:][64 * half:64 * (half + 1)], in_=src)
        ycum = wk.tile([128, SUB], f32, **BUFB)
        nc.vector.tensor_tensor_scan(out=ycum[:], data0=pmask[:],
                                     data1=ypool[:], initial=0.0,
                                     op0=OP.mult, op1=OP.add)
        pooled = wk2.tile([128, GSLOT], f32, tag="g_r")
        nc.gpsimd.ap_gather(out_ap=pooled[:], in_ap=ycum[:], idxs_ap=pidx[:],
                            channels=128, num_elems=SUB, d=1, num_idxs=GSLOT)
        nc.sync.dma_start(out=out_d[:], in_=pooled[:][0::16])

    nc.compile()
    return nc


# ================= driver =================
_CACHE = {}


def kernel(**inputs):
    inputs = {k: np.asarray(v) for k, v in inputs.items()}
    in_maps_data, meta = host_prep(inputs)
    w = fold_weights_host(inputs)
    EP = meta["EP"]
    if EP not in _CACHE:
        _CACHE[EP] = build(EP)
    nc = _CACHE[EP]
    from concourse.bass_utils import run_bass_kernel_spmd
    in_maps = []
    for c in range(NC):
        m = dict(in_maps_data[c])
        m.update(w)
        in_maps.append(m)
    trace = os.environ.get("KERNEL_PROFILE", "0") == "1"
    br = run_bass_kernel_spmd(nc, in_maps, list(range(NC)), trace=trace)
    if trace and br.exec_time_ns is not None:
        print(f"HW exec time: {br.exec_time_ns} ns")
    got = np.zeros(NG, np.float32)
    for c in range(NC):
        pooled = br.results[c]["pooled"]
        for s in range(8):
            for i, (g, endpos) in enumerate(meta["pool_graphs"][c][s]):
                got[g] = pooled[s, i]
    return got


# revision 8
# speedup vs baseline: 6.3045x; 4.8289x over previous
"""Trainium2 Bass kernel for nn_Net_76622216561354 (gnn_message_passing).

Self-contained: host-side sharding/index prep (numpy) + an 8-core SPMD
Bass/Tile kernel run via run_bass_kernel_spmd. Accepts FULL inputs, returns
the FULL pooled output [8192] float32.

v2: dst-side aggregation uses gpsimd.local_scatter (streaming, ~4cyc/idx)
instead of ap_gather over run-end positions (~70cyc/idx); edge streams are
re-sorted by (src-block, dst sub-chunk) with per-segment padding so each
scatter call covers a static stream window; per-edge scalar tables and
index tensors ship compact ([8,...]) and are partition-replicated x16 on
device via 0-stride DMA, cutting shipped input bytes ~5x.
"""
import numpy as np
import os

NC = 8
N = 131072; E = 524288; F_IN = 16; DIM = 64; DNN = 16; BK = 4; NG = 8192
NL1 = 4; NL2 = 2
SUB = 2112
NPAD = 8 * SUB          # 16896
HALF = NPAD // 2        # 8448
ECH = 512
GSLOT = 192             # pooled graph slots per sub-chunk (padded)
EBCAP = 2046            # local_scatter num_elems cap (num_elems*32 < 2^16)
EBX = SUB - EBCAP       # 66 tail slots per sub-chunk -> cleanup lane
# dst sub-chunk segment order in the edge stream: half-pairs (q, 4+q)
# complete early so gates can start before the full scan pipeline drains
SEG_ORDER = [0, 4, 1, 5, 2, 6, 3, 7]
SEG_RANK = {s: r for r, s in enumerate(SEG_ORDER)}


def host_prep(inputs):
    ei = np.asarray(inputs["edge_index"])
    batch = np.asarray(inputs["batch"]).astype(np.int64)
    src, dst = ei[0].astype(np.int64), ei[1].astype(np.int64)

    # ---- graph spans ----
    gsizes = np.bincount(batch, minlength=NG)
    gstart = np.concatenate([[0], np.cumsum(gsizes)])

    # ---- core cuts at graph boundaries ----
    cuts = [0]
    for c in range(1, NC):
        t = c * (N // NC)
        while t < N and batch[t] == batch[t - 1]:
            t += 1
        cuts.append(t)
    cuts.append(N)
    cuts = np.array(cuts, np.int64)

    # ---- per-core: pack graphs into 8 graph-aligned sub-chunks ----
    g2l = np.full(N, -1, np.int64)      # global node -> local slot (within its core)
    node_core = np.zeros(N, np.int64)
    l2g = [np.full(NPAD, -1, np.int64) for _ in range(NC)]
    pool_graphs = [[[] for _ in range(8)] for _ in range(NC)]
    pool_mask = [np.zeros((8, SUB), np.float32) for _ in range(NC)]

    for c in range(NC):
        lo, hi = cuts[c], cuts[c + 1]
        glo, ghi = batch[lo], (batch[hi - 1] + 1 if hi > lo else batch[lo])
        s = 0; pos = 0
        for g in range(glo, ghi):
            sz = int(gsizes[g])
            if sz == 0:
                continue
            if pos + sz > SUB:
                s += 1; pos = 0
                assert s < 8, f"core {c}: sub-chunk overflow"
                assert sz <= SUB
            nodes = np.arange(gstart[g], gstart[g] + sz)
            slots = s * SUB + pos + np.arange(sz)
            g2l[nodes] = slots
            node_core[nodes] = c
            l2g[c][slots] = nodes
            pool_mask[c][s, pos + 1: pos + sz] = 1.0
            pool_graphs[c][s].append((g, pos + sz - 1))
            pos += sz
        assert hi == lo or batch[hi - 1] + 1 == ghi

    dst_core = node_core[dst]; src_core = node_core[src]
    dstslot = g2l[dst]; srcslot = g2l[src]
    dsub = dstslot // SUB               # dst sub-chunk

    # ---- segment capacity: max edges per (dst core, src block, dst sub) ----
    key = (dst_core * NC + src_core) * 8 + dsub
    segcnt = np.bincount(key, minlength=NC * NC * 8)
    SEGCAP = int(-(-int(segcnt.max()) // 64) * 64)
    EP = 8 * SEGCAP
    nchunk = EP // ECH

    indeg = np.bincount(dst, minlength=N).astype(np.float64)
    inv = 1.0 / np.maximum(indeg, 1.0)
    ea_all = np.asarray(inputs["edge_attr"])

    per_core = []
    for c in range(NC):
        gidx = np.zeros((8, EP), np.int64)       # src slot per stream pos
        craw = np.zeros((8, 6, EP), np.float32)  # inv, ea*4, same-mask
        sidx = np.full((8, 2, EP), -1, np.int64)  # run-end targets: main, clean
        for b in range(NC):
            m = (dst_core == c) & (src_core == b)
            eids = np.nonzero(m)[0]
            order = np.lexsort((dstslot[eids],))
            eids = eids[order]                   # sorted by dstslot => by (dsub, slot)
            dsl = dstslot[eids]
            ds = dsl // SUB
            # position within stream: segment base + rank within segment
            seg_off = np.zeros(len(eids), np.int64)
            for s in range(8):
                sm = ds == s
                k = int(sm.sum())
                assert k <= SEGCAP
                seg_off[sm] = SEG_RANK[s] * SEGCAP + np.arange(k)
            gidx[b, seg_off] = srcslot[eids]
            craw[b, 0, seg_off] = inv[dst[eids]]
            for q in range(BK):
                craw[b, 1 + q, seg_off] = ea_all[eids, q]
            # same-dst continuation mask (within segment; runs never span segs)
            same = np.zeros(len(eids), bool)
            if len(eids) > 0:
                same[1:] = (dsl[1:] == dsl[:-1])
                craw[b, 5, seg_off] = same.astype(np.float32)
                # run ends: last edge of each dst run
                is_end = np.ones(len(eids), bool)
                is_end[:-1] = dsl[1:] != dsl[:-1]
                epos = seg_off[is_end]
                eslot = dsl[is_end] % SUB        # slot within sub-chunk
                main = eslot < EBCAP
                sidx[b, 0, epos[main]] = eslot[main]
                sidx[b, 1, epos[~main]] = eslot[~main] - EBCAP
        per_core.append(dict(gidx=gidx, craw=craw, sidx=sidx))

    # ---- wrap helper: seq -> [16, L/16] with idx[p, s] = seq[s*16+p] ----
    def wrap16(seq):
        L = len(seq)
        assert L % 16 == 0
        return np.asarray(seq).reshape(L // 16, 16).T.copy()

    in_maps = []
    for c in range(NC):
        pc = per_core[c]
        # gather idx: [128, EP/16] int16, wrapped per gather-call window (GCH)
        GCH = 4 * ECH
        gidx_t = np.zeros((128, EP // 16), np.int16)
        for b in range(NC):
            off = 0
            while off < EP:
                L = min(GCH, EP - off)
                seq = pc["gidx"][b, off:off + L]
                gidx_t[16 * b:16 * (b + 1), off // 16:(off + L) // 16] = \
                    wrap16(seq).astype(np.int16)
                off += L
        pidx_t = np.zeros((128, GSLOT // 16), np.int16)
        for s in range(8):
            seq = np.zeros(GSLOT, np.int64)
            gl = pool_graphs[c][s]
            assert len(gl) <= GSLOT, f"GSLOT overflow: {len(gl)}"
            for i, (g, endpos) in enumerate(gl):
                seq[i] = endpos
            pidx_t[16 * s:16 * (s + 1), :] = wrap16(seq).astype(np.int16)
        # x slab transposed [16, NPAD] f16
        xT = np.zeros((16, NPAD), np.float16)
        real = l2g[c] >= 0
        xT[:, real] = np.asarray(inputs["x"])[l2g[c][real]].T.astype(np.float16)
        in_maps.append(dict(
            xT=xT,
            gidx=gidx_t,
            craw=pc["craw"].astype(np.float16),
            sidx=pc["sidx"].astype(np.int16),
            pmask=pool_mask[c].astype(np.float16),
            pidx=pidx_t))

    meta = dict(EP=EP, SEGCAP=SEGCAP, nchunk=nchunk, cuts=cuts,
                pool_graphs=pool_graphs, l2g=l2g)
    return in_maps, meta


def fold_weights_host(inputs):
    """float64 weight folds -> shipped stationaries/biases (per-core identical)."""
    dt = np.float64
    lin0_w = np.asarray(inputs["lin0_w"], dt); lin0_b = np.asarray(inputs["lin0_b"], dt)
    lin1_w = np.asarray(inputs["lin1_w"], dt); lin1_b = np.asarray(inputs["lin1_b"], dt)
    lin2_w = np.asarray(inputs["lin2_w"], dt)
    root_w = np.asarray(inputs["root_w"], dt); conv_b = np.asarray(inputs["conv_b"], dt)
    nn1_w = np.asarray(inputs["nn1_w"], dt); nn1_b = np.asarray(inputs["nn1_b"], dt)
    gw_ih = np.asarray(inputs["gru_w_ih"], dt); gw_hh = np.asarray(inputs["gru_w_hh"], dt)
    gb_ih = np.asarray(inputs["gru_b_ih"], dt); gb_hh = np.asarray(inputs["gru_b_hh"], dt)

    Bm = nn1_b.reshape(DNN, DNN)
    Ak = nn1_w.reshape(BK, DNN, DNN)
    M = np.concatenate([Bm[None], Ak], axis=0)            # [5,16,16]

    w = {}
    # compact stationaries; expanded into block layouts on device at INIT
    w["wlin1s"] = lin1_w.astype(np.float16)               # [64,16]
    w["wMs"] = M.astype(np.float16)                       # [5,16,16]
    whs = np.zeros((NL1, 4, 64, 64), np.float32)
    wfold = np.zeros((NL1, 3, 16, 64), np.float32)
    biases = np.zeros((128, 17), np.float32)
    for j in range(NL1):
        P = lin1_w @ root_w @ gw_ih[j].T                  # [64,192]
        W_rz = P[:, :2 * DIM] + gw_hh[j].T[:, :2 * DIM]
        W_ni = P[:, 2 * DIM:]
        W_nh = gw_hh[j].T[:, 2 * DIM:]
        grp_w = [W_rz[:, :64], W_rz[:, 64:], W_ni, W_nh]
        for g in range(4):
            whs[j, g] = grp_w[g]
        wihT = gw_ih[j].T                                  # [16,192]
        for g in range(3):
            wfold[j, g] = wihT[:, 64 * g:64 * (g + 1)]
        b_base = (lin1_b @ root_w + conv_b) @ gw_ih[j].T   # [192]
        b_rz = b_base[:2 * DIM] + gb_ih[j][:2 * DIM] + gb_hh[j][:2 * DIM]
        b_ni = b_base[2 * DIM:] + gb_ih[j][2 * DIM:]
        b_hn = gb_hh[j][2 * DIM:]
        vec = [b_rz[:64], b_rz[64:], b_ni, b_hn]
        for g in range(4):
            biases[0:64, 4 * j + g] = vec[g]
            biases[64:128, 4 * j + g] = vec[g]
    w["whs"] = whs.astype(np.float16)
    w["wfold"] = wfold.astype(np.float16)
    biases[0:64, 16] = lin0_b
    biases[64:128, 16] = lin0_b
    w["biases"] = biases.astype(np.float32)
    # lin0 stationaries [2, 16, 128] f16
    wlin0 = np.zeros((2, 16, 128), np.float32)
    wlin0[0, :, 0:64] = lin0_w
    wlin0[1, :, 64:128] = lin0_w
    w["wlin0"] = wlin0.astype(np.float16)
    # y stationary [128, 2] f16
    wy = np.zeros((128, 2), np.float32)
    wy[0:64, 0] = lin2_w[:, 0]
    wy[64:128, 1] = lin2_w[:, 0]
    w["wy"] = wy.astype(np.float16)
    return w


# ================= kernel builder =================

import concourse.bass as bass
import concourse.mybir as mybir
import concourse.tile as tile
from concourse import bacc
from contextlib import ExitStack

NITER = 8

f32 = mybir.dt.float32
f16 = mybir.dt.float16
i16 = mybir.dt.int16
AF = mybir.ActivationFunctionType
OP = mybir.AluOpType


def pieces(total, step):
    out = []
    off = 0
    while off < total:
        out.append((off, min(step, total - off)))
        off += step
    return out


def ends_pieces(c0, L):
    """Split node-column window [c0, c0+L) into (sub-chunk, offset, len)."""
    out = []
    while L > 0:
        ch = c0 // SUB
        off = c0 % SUB
        ln = min(L, SUB - off)
        out.append((ch, off, ln))
        c0 += ln
        L -= ln
    return out


def build(EP, fake_collective=False, niter=NITER, skip=()):
    SEGCAP = EP // 8
    nchunk = EP // ECH
    nc = bacc.Bacc("TRN2", target_bir_lowering=False, debug=False, num_devices=NC)

    xT_d = nc.dram_tensor("xT", [16, NPAD], f16, kind="ExternalInput")
    gidx_d = nc.dram_tensor("gidx", [128, EP // 16], i16, kind="ExternalInput")
    craw_d = nc.dram_tensor("craw", [8, 6, EP], f16, kind="ExternalInput")
    sidx_d = nc.dram_tensor("sidx", [8, 2, EP], i16, kind="ExternalInput")
    pmask_d = nc.dram_tensor("pmask", [8, SUB], f16, kind="ExternalInput")
    pidx_d = nc.dram_tensor("pidx", [128, GSLOT // 16], i16, kind="ExternalInput")
    wlin1_d = nc.dram_tensor("wlin1s", [64, 16], f16, kind="ExternalInput")
    wM_d = nc.dram_tensor("wMs", [5, 16, 16], f16, kind="ExternalInput")
    whs_d = nc.dram_tensor("whs", [NL1, 4, 64, 64], f16, kind="ExternalInput")
    wfold_d = nc.dram_tensor("wfold", [NL1, 3, 16, 64], f16, kind="ExternalInput")
    biases_d = nc.dram_tensor("biases", [128, 17], f32, kind="ExternalInput")
    wlin0_d = nc.dram_tensor("wlin0", [2, 16, 128], f16, kind="ExternalInput")
    wy_d = nc.dram_tensor("wy", [128, 2], f16, kind="ExternalInput")
    out_d = nc.dram_tensor("pooled", [8, GSLOT], f32, kind="ExternalOutput")

    PIECES_H = pieces(HALF, 512)
    PIECES_S = pieces(SUB, 512)

    with tile.TileContext(nc) as tc, ExitStack() as ex:
        pp = ex.enter_context(tc.tile_pool(name="persist", bufs=1))
        wk = ex.enter_context(tc.tile_pool(name="work", bufs=2))
        wk2 = ex.enter_context(tc.tile_pool(name="work2", bufs=2))
        ps = ex.enter_context(tc.tile_pool(name="psum", bufs=8, space="PSUM"))
        dr = ex.enter_context(tc.tile_pool(name="dram", bufs=1, space="DRAM"))

        BUFA = dict(tag="bufA")
        BUFB = dict(tag="bufB")
        GGT = dict(tag="gg")

        hT = pp.tile([128, HALF], f16, tag="hT")
        table = pp.tile([128, NPAD, 2], f16, tag="table")
        cum = pp.tile([128, EP], f16, tag="cum")
        nc.vector.memset(cum[:], 0)
        gidx = pp.tile([128, EP // 16], i16, tag="gidx")
        pmask = pp.tile([128, SUB], f16, tag="pmask")
        pidx = pp.tile([128, GSLOT // 16], i16, tag="pidx")
        biases = pp.tile([128, 17], f32, tag="biases")
        wy = pp.tile([128, 2], f16, tag="wy")

        nc.sync.dma_start(out=gidx[:], in_=gidx_d[:])
        nc.sync.dma_start(
            out=pmask[:],
            in_=bass.AP(pmask_d, 0, [(SUB, 8), (0, 16), (1, SUB)]))
        nc.sync.dma_start(out=pidx[:], in_=pidx_d[:])
        nc.sync.dma_start(out=biases[:], in_=biases_d[:])
        nc.sync.dma_start(out=wy[:], in_=wy_d[:])

        # stationaries expanded on device from compact shipped blocks
        wslab_s = pp.tile([128, 8, 128], f16, tag="wslab_s")
        nc.vector.memset(wslab_s[:], 0)
        for b in range(8):
            r0 = 64 * (b // 4)
            nc.sync.dma_start(
                out=wslab_s[r0:r0 + 64, b, 16 * b:16 * (b + 1)],
                in_=bass.AP(wlin1_d, 0, [(16, 64), (1, 16)]))
        wM_s = pp.tile([128, 5, 128], f16, tag="wM_s")
        nc.vector.memset(wM_s[:], 0)
        for p in range(5):
            for b in range(8):
                nc.sync.dma_start(
                    out=wM_s[16 * b:16 * (b + 1), p, 16 * b:16 * (b + 1)],
                    in_=bass.AP(wM_d, p * 256, [(16, 16), (1, 16)]))
        whs_s = pp.tile([128, NL1 * 4, 128], f16, tag="whs_s")
        nc.vector.memset(whs_s[:], 0)
        for j in range(NL1):
            for g in range(4):
                for h_ in range(2):
                    nc.sync.dma_start(
                        out=whs_s[64 * h_:64 * (h_ + 1), 4 * j + g,
                                  64 * h_:64 * (h_ + 1)],
                        in_=bass.AP(whs_d, (j * 4 + g) * 64 * 64,
                                    [(64, 64), (1, 64)]))
        wfold_s = pp.tile([128, NL1 * 3, 64], f16, tag="wfold_s")
        for j in range(NL1):
            for g in range(3):
                nc.sync.dma_start(
                    out=wfold_s[:, 3 * j + g, :],
                    in_=bass.AP(wfold_d, (j * 3 + g) * 16 * 64,
                                [(0, 8), (64, 16), (1, 64)]))

        slab_dram = dr.tile([128, SUB * 2], f16)
        ag_dram = dr.tile([NC, 128, SUB * 2], f16)
        cexp_dram = dr.tile([nchunk, 128, 6 * ECH], f16)
        sidx_dram = dr.tile([128, 2, EP], i16)
        y_dram = dr.tile([2, HALF], f32)

        # ================= INIT =================
        # sidx expand x16 (DRAM -> DRAM), one DMA per lane (3-dim AP limit)
        for t in range(2):
            dstap = bass.AP(sidx_dram.tensor,
                            sidx_dram[:].offset + t * EP,
                            [sidx_dram[:].ap[0], (1, EP)])
            nc.sync.dma_start(
                out=dstap,
                in_=bass.AP(sidx_d, t * EP, [(2 * EP, 8), (0, 16), (1, EP)]))
        # c-expansion: replicate x16 across partitions, fold inv into ea
        for k in range(nchunk):
            cc = wk.tile([128, 6, ECH], f16, **BUFA)
            for j in range(6):
                nc.sync.dma_start(
                    out=cc[:, j, :],
                    in_=bass.AP(craw_d, j * EP + k * ECH,
                                [(6 * EP, 8), (0, 16), (1, ECH)]))
            ce = wk.tile([128, 6, ECH], f16, **BUFB)
            nc.vector.tensor_copy(out=ce[:, 0, :], in_=cc[:, 0, :])
            in0 = bass.AP(cc.tensor, cc[:].offset, [cc[:].ap[0], (0, 4), (1, ECH)])
            nc.vector.tensor_tensor(out=ce[:, 1:5, :], in0=in0,
                                    in1=cc[:, 1:5, :], op=OP.mult)
            nc.vector.tensor_copy(out=ce[:, 5, :], in_=cc[:, 5, :])
            nc.sync.dma_start(out=cexp_dram[k], in_=ce[:])

        # lin0 -> hT
        wlin0 = pp.tile([16, 2, 128], f16, tag="wlin0")
        nc.sync.dma_start(
            out=wlin0[:],
            in_=bass.AP(wlin0_d, 0, [(128, 16), (16 * 128, 2), (1, 128)]))
        for c0, L in PIECES_H:
            xa = wk2.tile([16, 512], f16, **GGT)
            nc.sync.dma_start(out=xa[:, :L], in_=xT_d[:, c0:c0 + L])
            xb = wk2.tile([16, 512], f16, **GGT)
            nc.sync.dma_start(out=xb[:, :L], in_=xT_d[:, HALF + c0:HALF + c0 + L])
            p0 = ps.tile([128, 512], f32, tag="ps")
            nc.tensor.matmul(p0[:, :L], wlin0[:, 0, :], xa[:, :L],
                             start=True, stop=False)
            nc.tensor.matmul(p0[:, :L], wlin0[:, 1, :], xb[:, :L],
                             start=False, stop=True)
            nc.scalar.activation(out=hT[:, c0:c0 + L], in_=p0[:, :L],
                                 func=AF.Relu, bias=biases[:, 16:17], scale=1.0)

        # ================= ITERATIONS =================
        for it in range(niter):
            j = (it // 2) % NL1
            # ---- A: slab + exchange ----
            for c0, L in PIECES_S:
                p0 = ps.tile([128, 512], f32, tag="ps")
                for b in range(8):
                    rc0 = (b % 4) * SUB + c0
                    nc.tensor.matmul(p0[:, :L], wslab_s[:, b, :],
                                     hT[:, rc0:rc0 + L],
                                     start=(b == 0), stop=(b == 7))
                stg = wk2.tile([128, 1024], f16, tag="slabstg")
                for dup in range(2):
                    dst = bass.AP(stg.tensor, stg[:].offset + dup,
                                  [stg[:].ap[0], (2, L)])
                    nc.vector.tensor_copy(out=dst, in_=p0[:, :L])
                nc.sync.dma_start(out=slab_dram[:, c0 * 2:(c0 + L) * 2],
                                  in_=stg[:, :2 * L])
            if fake_collective:
                for cc_ in range(NC):
                    nc.sync.dma_start(out=ag_dram[cc_], in_=slab_dram[:])
            else:
                nc.gpsimd.collective_compute(
                    "AllGather", OP.bypass,
                    replica_groups=[list(range(NC))],
                    ins=[slab_dram[:].opt()], outs=[ag_dram[:].opt()])
            for s in range(8):
                src = bass.AP(ag_dram.tensor,
                              ag_dram[:].offset + (16 * s) * (SUB * 2),
                              [(128 * SUB * 2, 8), (SUB * 2, 16), (1, SUB * 2)])
                dst = bass.AP(table.tensor, table[:].offset + s * SUB * 2,
                              [table[:].ap[0], (1, SUB * 2)])
                nc.sync.dma_start(out=dst, in_=src)

            # ---- C: edge chunks (gathers batched 4x to amortize ap_gather) ----
            GCH = 4 * ECH
            gbuf = None
            for k in range(nchunk):
                cc = wk.tile([128, 6, ECH], f16, **BUFA)
                nc.sync.dma_start(out=cc[:], in_=cexp_dram[k])
                if k % 4 == 0:
                    G0 = k * ECH
                    GL = min(GCH, EP - G0)
                    gbuf = pp.tile([128, GCH, 2], f16, tag="gbuf")
                    if "gather" not in skip:
                        nc.gpsimd.ap_gather(
                            out_ap=gbuf[:, :GL, :], in_ap=table[:],
                            idxs_ap=gidx[:, G0 // 16:(G0 + GL) // 16],
                            channels=128, num_elems=NPAD, d=2, num_idxs=GL)
                    else:
                        nc.vector.memset(gbuf[:, 0:4, :], 0)
                sc = wk.tile([128, 5, ECH], f16, **BUFB)
                g_in0 = bass.AP(gbuf.tensor,
                                gbuf[:].offset + (k % 4) * ECH * 2,
                                [gbuf[:].ap[0], (0, 5), (2, ECH)])
                nc.vector.tensor_tensor(out=sc[:], in0=g_in0, in1=cc[:, 0:5, :],
                                        op=OP.mult)
                msg = ps.tile([128, 512], f32, tag="ps")
                for p in range(5):
                    nc.tensor.matmul(msg[:, :ECH], wM_s[:, p, :], sc[:, p, :],
                                     start=(p == 0), stop=(p == 4))
                if k == 0:
                    init = 0.0
                else:
                    init = bass.AP(cum.tensor, cum[:].offset + (k * ECH - 1),
                                   [cum[:].ap[0], (1, 1)])
                nc.vector.tensor_tensor_scan(out=cum[:, k * ECH:(k + 1) * ECH],
                                             data0=cc[:, 5, :],
                                             data1=msg[:, :ECH], initial=init,
                                             op0=OP.mult, op1=OP.add)

            # ---- D: per-segment local_scatter -> eb tiles ----
            ebx = pp.tile([128, 8 * EBX], f16, tag="ebx")
            ebs = pp.tile([128, 8, EBCAP], f16, tag="ebs")
            for r, s in enumerate(SEG_ORDER):
                st = wk.tile([128, 2, SEGCAP], i16, **BUFB)
                nc.sync.dma_start(
                    out=st[:],
                    in_=bass.AP(sidx_dram.tensor,
                                sidx_dram[:].offset + r * SEGCAP,
                                [sidx_dram[:].ap[0], (EP, 2), (1, SEGCAP)]))
                if "scatter" not in skip:
                    nc.gpsimd.local_scatter(
                        out_ap=ebs[:, s, :],
                        data_ap=cum[:, r * SEGCAP:(r + 1) * SEGCAP],
                        idxs_ap=st[:, 0, :],
                        channels=128, num_elems=EBCAP, num_idxs=SEGCAP)
                    nc.gpsimd.local_scatter(
                        out_ap=ebx[:, s * EBX:(s + 1) * EBX],
                        data_ap=cum[:, r * SEGCAP:(r + 1) * SEGCAP],
                        idxs_ap=st[:, 1, :],
                        channels=128, num_elems=EBX, num_idxs=SEGCAP)
                else:
                    nc.vector.memset(ebs[:, s, :], 0)
                    nc.vector.memset(ebx[:, s * EBX:(s + 1) * EBX], 0)

            # ---- E: gates (q-major: sub-chunk pair (q, 4+q) first) ----
            PIECES_Q = [(q * SUB + off, L) for q in range(4) for off, L in PIECES_S]
            for c0, L in PIECES_Q:
                pr = ps.tile([128, 512], f32, tag="ps")
                pz = ps.tile([128, 512], f32, tag="ps")
                pn = ps.tile([128, 512], f32, tag="ps")
                ph = ps.tile([128, 512], f32, tag="ps")
                for g, pst in enumerate([pr, pz, pn, ph]):
                    nc.tensor.matmul(pst[:, :L], whs_s[:, 4 * j + g, :],
                                     hT[:, c0:c0 + L],
                                     start=True, stop=True)
                for g, pst in enumerate([pr, pz, pn]):
                    for half in range(2):
                        pcs = ends_pieces(half * HALF + c0, L)
                        for (ch, eoff, eln) in pcs:
                            # split at the EBCAP boundary within the sub-chunk
                            parts = []
                            if eoff < EBCAP:
                                ln1 = min(eln, EBCAP - eoff)
                                parts.append((ebs, ch * EBCAP + eoff, ln1, eoff))
                            if eoff + eln > EBCAP:
                                o2 = max(eoff, EBCAP)
                                parts.append((ebx, ch * EBX + (o2 - EBCAP),
                                              eoff + eln - o2, o2))
                            for (tile_, toff, tln, nodeoff) in parts:
                                rhs = bass.AP(tile_.tensor,
                                              tile_[:].offset + toff,
                                              [tile_[:].ap[0], (1, tln)])
                                oo = (nodeoff + ch * SUB) - (half * HALF + c0)
                                out = bass.AP(
                                    pst.tensor,
                                    pst[:].offset + 64 * half * pst[:].ap[0][0] + oo,
                                    [(pst[:].ap[0][0], 64), (1, tln)])
                                tp = (0, 64) if half == 1 else None
                                nc.tensor.matmul(out, wfold_s[:, 3 * j + g, :], rhs,
                                                 start=False, stop=False,
                                                 skip_group_check=True,
                                                 tile_position=tp)
                r16 = wk2.tile([128, 512], f16, tag="g_r")
                z16 = wk2.tile([128, 512], f16, tag="g_z")
                nc.scalar.activation(out=r16[:, :L], in_=pr[:, :L], func=AF.Sigmoid,
                                     bias=biases[:, 4 * j:4 * j + 1], scale=1.0)
                nc.scalar.activation(out=z16[:, :L], in_=pz[:, :L], func=AF.Sigmoid,
                                     bias=biases[:, 4 * j + 1:4 * j + 2], scale=1.0)
                t16 = wk2.tile([128, 512], f16, tag="g_t")
                nc.vector.scalar_tensor_tensor(
                    out=t16[:, :L], in0=ph[:, :L],
                    scalar=biases[:, 4 * j + 3:4 * j + 4], in1=r16[:, :L],
                    op0=OP.add, op1=OP.mult)
                u16 = wk2.tile([128, 512], f16, tag="g_u")
                nc.vector.tensor_tensor(out=u16[:, :L], in0=t16[:, :L],
                                        in1=pn[:, :L], op=OP.add)
                n16 = wk2.tile([128, 512], f16, tag="g_n")
                nc.scalar.activation(out=n16[:, :L], in_=u16[:, :L], func=AF.Tanh,
                                     bias=biases[:, 4 * j + 2:4 * j + 3], scale=1.0)
                v16 = wk2.tile([128, 512], f16, tag="g_t")
                nc.vector.tensor_tensor(out=v16[:, :L], in0=hT[:, c0:c0 + L],
                                        in1=n16[:, :L], op=OP.subtract)
                w16 = wk2.tile([128, 512], f16, tag="g_u")
                nc.vector.tensor_tensor(out=w16[:, :L], in0=z16[:, :L],
                                        in1=v16[:, :L], op=OP.mult)
                nc.vector.tensor_tensor(out=hT[:, c0:c0 + L], in0=n16[:, :L],
                                        in1=w16[:, :L], op=OP.add)

        # ================= FINAL: y + pooling =================
        for c0, L in PIECES_H:
            py = ps.tile([2, 512], f32, tag="ps")
            nc.tensor.matmul(py[:, :L], wy[:], hT[:, c0:c0 + L],
                             start=True, stop=True)
            ystg = wk2.tile([2, 512], f32, **GGT)
            nc.vector.tensor_copy(out=ystg[:, :L], in_=py[:, :L])
            nc.sync.dma_start(out=y_dram[:, c0:c0 + L], in_=ystg[:, :L])
        ypool = wk.tile([128, SUB], f32, **BUFA)
        for half in range(2):
            src = bass.AP(y_dram.tensor, y_dram[:].offset + half * HALF,
                          [(SUB, 4), (0, 16), (1, SUB)])
            nc.sync.dma_start(out=ypool[:][64 * half:64 * (half + 1)], in_=src)
        ycum = wk.tile([128, SUB], f32, **BUFB)
        nc.vector.tensor_tensor_scan(out=ycum[:], data0=pmask[:],
                                     data1=ypool[:], initial=0.0,
                                     op0=OP.mult, op1=OP.add)
        pooled = wk2.tile([128, GSLOT], f32, tag="g_r")
        nc.gpsimd.ap_gather(out_ap=pooled[:], in_ap=ycum[:], idxs_ap=pidx[:],
                            channels=128, num_elems=SUB, d=1, num_idxs=GSLOT)
        nc.sync.dma_start(out=out_d[:], in_=pooled[:][0::16])

    nc.compile()
    return nc


# ================= driver =================
_CACHE = {}


def kernel(**inputs):
    inputs = {k: np.asarray(v) for k, v in inputs.items()}
    in_maps_data, meta = host_prep(inputs)
    w = fold_weights_host(inputs)
    EP = meta["EP"]
    if EP not in _CACHE:
        _CACHE[EP] = build(EP)
    nc = _CACHE[EP]
    from concourse.bass_utils import run_bass_kernel_spmd
    in_maps = []
    for c in range(NC):
        m = dict(in_maps_data[c])
        m.update(w)
        in_maps.append(m)
    trace = os.environ.get("KERNEL_PROFILE", "0") == "1"
    br = run_bass_kernel_spmd(nc, in_maps, list(range(NC)), trace=trace)
    if trace and br.exec_time_ns is not None:
        print(f"HW exec time: {br.exec_time_ns} ns")
    got = np.zeros(NG, np.float32)
    for c in range(NC):
        pooled = br.results[c]["pooled"]
        for s in range(8):
            for i, (g, endpos) in enumerate(meta["pool_graphs"][c][s]):
                got[g] = pooled[s, i]
    return got


# revision 9
# speedup vs baseline: 11.1405x; 1.7671x over previous
"""Trainium2 Bass kernel for nn_Net_76622216561354 (gnn_message_passing).

Self-contained: host-side sharding/index prep (numpy) + an 8-core SPMD
Bass/Tile kernel run via run_bass_kernel_spmd. Accepts FULL inputs, returns
the FULL pooled output [8192] float32.

v2: dst-side aggregation uses gpsimd.local_scatter (streaming, ~4cyc/idx)
instead of ap_gather over run-end positions (~70cyc/idx); edge streams are
re-sorted by (src-block, dst sub-chunk) with per-segment padding so each
scatter call covers a static stream window; per-edge scalar tables and
index tensors ship compact ([8,...]) and are partition-replicated x16 on
device via 0-stride DMA, cutting shipped input bytes ~5x.
"""
import numpy as np
import os

NC = 8
N = 131072; E = 524288; F_IN = 16; DIM = 64; DNN = 16; BK = 4; NG = 8192
NL1 = 4; NL2 = 2
SUB = 2112
NPAD = 8 * SUB          # 16896
HALF = NPAD // 2        # 8448
ECH = 512
GSLOT = 192             # pooled graph slots per sub-chunk (padded)
EBCAP = 2046            # local_scatter num_elems cap (num_elems*32 < 2^16)
EBX = SUB - EBCAP       # 66 tail slots per sub-chunk -> cleanup lane
# dst sub-chunk segment order in the edge stream: half-pairs (q, 4+q)
# complete early so gates can start before the full scan pipeline drains
SEG_ORDER = [0, 4, 1, 5, 2, 6, 3, 7]
SEG_RANK = {s: r for r, s in enumerate(SEG_ORDER)}


def host_prep(inputs):
    ei = np.asarray(inputs["edge_index"])
    batch = np.asarray(inputs["batch"]).astype(np.int64)
    src, dst = ei[0].astype(np.int64), ei[1].astype(np.int64)

    # ---- graph spans ----
    gsizes = np.bincount(batch, minlength=NG)
    gstart = np.concatenate([[0], np.cumsum(gsizes)])

    # ---- core cuts at graph boundaries ----
    cuts = [0]
    for c in range(1, NC):
        t = c * (N // NC)
        while t < N and batch[t] == batch[t - 1]:
            t += 1
        cuts.append(t)
    cuts.append(N)
    cuts = np.array(cuts, np.int64)

    # ---- per-core: pack graphs into 8 graph-aligned sub-chunks ----
    g2l = np.full(N, -1, np.int64)      # global node -> local slot (within its core)
    node_core = np.zeros(N, np.int64)
    l2g = [np.full(NPAD, -1, np.int64) for _ in range(NC)]
    pool_graphs = [[[] for _ in range(8)] for _ in range(NC)]
    pool_mask = [np.zeros((8, SUB), np.float32) for _ in range(NC)]

    for c in range(NC):
        lo, hi = cuts[c], cuts[c + 1]
        glo, ghi = batch[lo], (batch[hi - 1] + 1 if hi > lo else batch[lo])
        s = 0; pos = 0
        for g in range(glo, ghi):
            sz = int(gsizes[g])
            if sz == 0:
                continue
            if pos + sz > SUB:
                s += 1; pos = 0
                assert s < 8, f"core {c}: sub-chunk overflow"
                assert sz <= SUB
            nodes = np.arange(gstart[g], gstart[g] + sz)
            slots = s * SUB + pos + np.arange(sz)
            g2l[nodes] = slots
            node_core[nodes] = c
            l2g[c][slots] = nodes
            pool_mask[c][s, pos + 1: pos + sz] = 1.0
            pool_graphs[c][s].append((g, pos + sz - 1))
            pos += sz
        assert hi == lo or batch[hi - 1] + 1 == ghi

    dst_core = node_core[dst]; src_core = node_core[src]
    dstslot = g2l[dst]; srcslot = g2l[src]
    dsub = dstslot // SUB               # dst sub-chunk

    # ---- segment capacity: max edges per (dst core, src block, dst sub) ----
    key = (dst_core * NC + src_core) * 8 + dsub
    segcnt = np.bincount(key, minlength=NC * NC * 8)
    SEGCAP = int(-(-int(segcnt.max()) // 64) * 64)
    EP = 8 * SEGCAP
    nchunk = EP // ECH

    indeg = np.bincount(dst, minlength=N).astype(np.float64)
    inv = 1.0 / np.maximum(indeg, 1.0)
    ea_all = np.asarray(inputs["edge_attr"])

    per_core = []
    for c in range(NC):
        gidx = np.zeros((8, EP), np.int64)       # src slot per stream pos
        craw = np.zeros((8, 6, EP), np.float32)  # inv, ea*4, same-mask
        sidx = np.full((8, 2, EP), -1, np.int64)  # run-end targets: main, clean
        for b in range(NC):
            m = (dst_core == c) & (src_core == b)
            eids = np.nonzero(m)[0]
            order = np.lexsort((dstslot[eids],))
            eids = eids[order]                   # sorted by dstslot => by (dsub, slot)
            dsl = dstslot[eids]
            ds = dsl // SUB
            # position within stream: segment base + rank within segment
            seg_off = np.zeros(len(eids), np.int64)
            for s in range(8):
                sm = ds == s
                k = int(sm.sum())
                assert k <= SEGCAP
                seg_off[sm] = SEG_RANK[s] * SEGCAP + np.arange(k)
            gidx[b, seg_off] = srcslot[eids]
            craw[b, 0, seg_off] = inv[dst[eids]]
            for q in range(BK):
                craw[b, 1 + q, seg_off] = ea_all[eids, q]
            # same-dst continuation mask (within segment; runs never span segs)
            same = np.zeros(len(eids), bool)
            if len(eids) > 0:
                same[1:] = (dsl[1:] == dsl[:-1])
                craw[b, 5, seg_off] = same.astype(np.float32)
                # run ends: last edge of each dst run
                is_end = np.ones(len(eids), bool)
                is_end[:-1] = dsl[1:] != dsl[:-1]
                epos = seg_off[is_end]
                eslot = dsl[is_end] % SUB        # slot within sub-chunk
                main = eslot < EBCAP
                sidx[b, 0, epos[main]] = eslot[main]
                sidx[b, 1, epos[~main]] = eslot[~main] - EBCAP
        per_core.append(dict(gidx=gidx, craw=craw, sidx=sidx))

    # ---- wrap helper: seq -> [16, L/16] with idx[p, s] = seq[s*16+p] ----
    def wrap16(seq):
        L = len(seq)
        assert L % 16 == 0
        return np.asarray(seq).reshape(L // 16, 16).T.copy()

    in_maps = []
    for c in range(NC):
        pc = per_core[c]
        # gather idx: [128, EP/16] int16, wrapped per gather-call window (GCH)
        GCH = 4 * ECH
        gidx_t = np.zeros((128, EP // 16), np.int16)
        for b in range(NC):
            off = 0
            while off < EP:
                L = min(GCH, EP - off)
                seq = pc["gidx"][b, off:off + L]
                gidx_t[16 * b:16 * (b + 1), off // 16:(off + L) // 16] = \
                    wrap16(seq).astype(np.int16)
                off += L
        pidx_t = np.zeros((128, GSLOT // 16), np.int16)
        for s in range(8):
            seq = np.zeros(GSLOT, np.int64)
            gl = pool_graphs[c][s]
            assert len(gl) <= GSLOT, f"GSLOT overflow: {len(gl)}"
            for i, (g, endpos) in enumerate(gl):
                seq[i] = endpos
            pidx_t[16 * s:16 * (s + 1), :] = wrap16(seq).astype(np.int16)
        # x slab transposed [16, NPAD] f16
        xT = np.zeros((16, NPAD), np.float16)
        real = l2g[c] >= 0
        xT[:, real] = np.asarray(inputs["x"])[l2g[c][real]].T.astype(np.float16)
        in_maps.append(dict(
            xT=xT,
            gidx=gidx_t,
            craw=pc["craw"].astype(np.float16),
            sidx=pc["sidx"].astype(np.int16),
            pmask=pool_mask[c].astype(np.float16),
            pidx=pidx_t))

    meta = dict(EP=EP, SEGCAP=SEGCAP, nchunk=nchunk, cuts=cuts,
                pool_graphs=pool_graphs, l2g=l2g)
    return in_maps, meta


def fold_weights_host(inputs):
    """float64 weight folds -> shipped stationaries/biases (per-core identical)."""
    dt = np.float64
    lin0_w = np.asarray(inputs["lin0_w"], dt); lin0_b = np.asarray(inputs["lin0_b"], dt)
    lin1_w = np.asarray(inputs["lin1_w"], dt); lin1_b = np.asarray(inputs["lin1_b"], dt)
    lin2_w = np.asarray(inputs["lin2_w"], dt)
    root_w = np.asarray(inputs["root_w"], dt); conv_b = np.asarray(inputs["conv_b"], dt)
    nn1_w = np.asarray(inputs["nn1_w"], dt); nn1_b = np.asarray(inputs["nn1_b"], dt)
    gw_ih = np.asarray(inputs["gru_w_ih"], dt); gw_hh = np.asarray(inputs["gru_w_hh"], dt)
    gb_ih = np.asarray(inputs["gru_b_ih"], dt); gb_hh = np.asarray(inputs["gru_b_hh"], dt)

    Bm = nn1_b.reshape(DNN, DNN)
    Ak = nn1_w.reshape(BK, DNN, DNN)
    M = np.concatenate([Bm[None], Ak], axis=0)            # [5,16,16]

    w = {}
    # compact stationaries; expanded into block layouts on device at INIT
    w["wlin1s"] = lin1_w.astype(np.float16)               # [64,16]
    w["wMs"] = M.astype(np.float16)                       # [5,16,16]
    whs = np.zeros((NL1, 4, 64, 64), np.float32)
    wfold = np.zeros((NL1, 3, 16, 64), np.float32)
    biases = np.zeros((128, 17), np.float32)
    for j in range(NL1):
        P = lin1_w @ root_w @ gw_ih[j].T                  # [64,192]
        W_rz = P[:, :2 * DIM] + gw_hh[j].T[:, :2 * DIM]
        W_ni = P[:, 2 * DIM:]
        W_nh = gw_hh[j].T[:, 2 * DIM:]
        grp_w = [W_rz[:, :64], W_rz[:, 64:], W_ni, W_nh]
        for g in range(4):
            whs[j, g] = grp_w[g]
        wihT = gw_ih[j].T                                  # [16,192]
        for g in range(3):
            wfold[j, g] = wihT[:, 64 * g:64 * (g + 1)]
        b_base = (lin1_b @ root_w + conv_b) @ gw_ih[j].T   # [192]
        b_rz = b_base[:2 * DIM] + gb_ih[j][:2 * DIM] + gb_hh[j][:2 * DIM]
        b_ni = b_base[2 * DIM:] + gb_ih[j][2 * DIM:]
        b_hn = gb_hh[j][2 * DIM:]
        vec = [b_rz[:64], b_rz[64:], b_ni, b_hn]
        for g in range(4):
            biases[0:64, 4 * j + g] = vec[g]
            biases[64:128, 4 * j + g] = vec[g]
    w["whs"] = whs.astype(np.float16)
    w["wfold"] = wfold.astype(np.float16)
    biases[0:64, 16] = lin0_b
    biases[64:128, 16] = lin0_b
    w["biases"] = biases.astype(np.float32)
    # lin0 stationaries [2, 16, 128] f16
    wlin0 = np.zeros((2, 16, 128), np.float32)
    wlin0[0, :, 0:64] = lin0_w
    wlin0[1, :, 64:128] = lin0_w
    w["wlin0"] = wlin0.astype(np.float16)
    # y stationary [128, 2] f16
    wy = np.zeros((128, 2), np.float32)
    wy[0:64, 0] = lin2_w[:, 0]
    wy[64:128, 1] = lin2_w[:, 0]
    w["wy"] = wy.astype(np.float16)
    return w


def blob_layout(EP):
    """Fixed packing order of all per-core inputs into one int16 blob.
    Returns (offsets_bytes, total_int16_words)."""
    sizes = [
        ("xT", 16 * NPAD * 2),
        ("gidx", 128 * (EP // 16) * 2),
        ("craw", 8 * 6 * EP * 2),
        ("sidx", 8 * 2 * EP * 2),
        ("pmask", 8 * SUB * 2),
        ("pidx", 128 * (GSLOT // 16) * 2),
        ("wlin1s", 64 * 16 * 2),
        ("wMs", 5 * 16 * 16 * 2),
        ("whs", NL1 * 4 * 64 * 64 * 2),
        ("wfold", NL1 * 3 * 16 * 64 * 2),
        ("wlin0", 2 * 16 * 128 * 2),
        ("wy", 128 * 2 * 2),
        ("biases", 128 * 17 * 4),
    ]
    offs = {}
    off = 0
    for name, nb in sizes:
        off = (off + 3) & ~3          # 4-byte align every section
        offs[name] = off
        off += nb
    total = ((off + 3) & ~3) // 2
    return offs, total


def pack_blob(in_map, w, EP):
    offs, total = blob_layout(EP)
    blob = np.zeros(total, np.int16)
    data = dict(in_map); data.update(w)
    for name, off in offs.items():
        arr = np.ascontiguousarray(data[name])
        nb = arr.nbytes
        view = blob[off // 2:(off + nb) // 2]
        if arr.dtype == np.float32:
            view.view(np.float32)[:] = arr.ravel()
        elif arr.dtype == np.float16:
            view.view(np.float16)[:] = arr.ravel()
        elif arr.dtype == np.int16:
            view[:] = arr.ravel()
        else:
            raise ValueError(f"{name}: {arr.dtype}")
    return blob[None, :]


# ================= kernel builder =================

import concourse.bass as bass
import concourse.mybir as mybir
import concourse.tile as tile
from concourse import bacc
from contextlib import ExitStack

NITER = 8

f32 = mybir.dt.float32
f16 = mybir.dt.float16
i16 = mybir.dt.int16
AF = mybir.ActivationFunctionType
OP = mybir.AluOpType


def pieces(total, step):
    out = []
    off = 0
    while off < total:
        out.append((off, min(step, total - off)))
        off += step
    return out


def ends_pieces(c0, L):
    """Split node-column window [c0, c0+L) into (sub-chunk, offset, len)."""
    out = []
    while L > 0:
        ch = c0 // SUB
        off = c0 % SUB
        ln = min(L, SUB - off)
        out.append((ch, off, ln))
        c0 += ln
        L -= ln
    return out


def build(EP, fake_collective=False, niter=NITER, skip=()):
    SEGCAP = EP // 8
    nchunk = EP // ECH
    nc = bacc.Bacc("TRN2", target_bir_lowering=False, debug=False, num_devices=NC)

    OFFS, TOTW = blob_layout(EP)
    blob_d = nc.dram_tensor("blob", [1, TOTW], i16, kind="ExternalInput")
    blob16 = blob_d.bitcast(f16)
    blob32 = blob_d.bitcast(f32)
    O16 = {k: v // 2 for k, v in OFFS.items()}   # element offsets, 2-byte view
    O32 = {k: v // 4 for k, v in OFFS.items()}   # element offsets, 4-byte view
    out_d = nc.dram_tensor("pooled", [8, GSLOT], f32, kind="ExternalOutput")

    PIECES_H = pieces(HALF, 512)
    PIECES_S = pieces(SUB, 512)

    with tile.TileContext(nc) as tc, ExitStack() as ex:
        pp = ex.enter_context(tc.tile_pool(name="persist", bufs=1))
        wk = ex.enter_context(tc.tile_pool(name="work", bufs=2))
        wk2 = ex.enter_context(tc.tile_pool(name="work2", bufs=2))
        ps = ex.enter_context(tc.tile_pool(name="psum", bufs=8, space="PSUM"))
        dr = ex.enter_context(tc.tile_pool(name="dram", bufs=1, space="DRAM"))

        BUFA = dict(tag="bufA")
        BUFB = dict(tag="bufB")
        GGT = dict(tag="gg")

        hT = pp.tile([128, HALF], f16, tag="hT")
        table = pp.tile([128, NPAD, 2], f16, tag="table")
        cum = pp.tile([128, EP], f16, tag="cum")
        nc.vector.memset(cum[:], 0)
        gidx = pp.tile([128, EP // 16], i16, tag="gidx")
        pmask = pp.tile([128, SUB], f16, tag="pmask")
        pidx = pp.tile([128, GSLOT // 16], i16, tag="pidx")
        biases = pp.tile([128, 17], f32, tag="biases")
        wy = pp.tile([128, 2], f16, tag="wy")

        nc.sync.dma_start(
            out=gidx[:],
            in_=bass.AP(blob_d, O16["gidx"], [(EP // 16, 128), (1, EP // 16)]))
        nc.sync.dma_start(
            out=pmask[:],
            in_=bass.AP(blob16, O16["pmask"], [(SUB, 8), (0, 16), (1, SUB)]))
        nc.sync.dma_start(
            out=pidx[:],
            in_=bass.AP(blob_d, O16["pidx"],
                        [(GSLOT // 16, 128), (1, GSLOT // 16)]))
        nc.sync.dma_start(
            out=biases[:],
            in_=bass.AP(blob32, O32["biases"], [(17, 128), (1, 17)]))
        nc.sync.dma_start(
            out=wy[:], in_=bass.AP(blob16, O16["wy"], [(2, 128), (1, 2)]))

        # stationaries expanded on device from compact shipped blocks
        wslab_s = pp.tile([128, 8, 128], f16, tag="wslab_s")
        nc.vector.memset(wslab_s[:], 0)
        for b in range(8):
            r0 = 64 * (b // 4)
            nc.sync.dma_start(
                out=wslab_s[r0:r0 + 64, b, 16 * b:16 * (b + 1)],
                in_=bass.AP(blob16, O16["wlin1s"], [(16, 64), (1, 16)]))
        wM_s = pp.tile([128, 5, 128], f16, tag="wM_s")
        nc.vector.memset(wM_s[:], 0)
        for p in range(5):
            for b in range(8):
                nc.sync.dma_start(
                    out=wM_s[16 * b:16 * (b + 1), p, 16 * b:16 * (b + 1)],
                    in_=bass.AP(blob16, O16["wMs"] + p * 256, [(16, 16), (1, 16)]))
        whs_s = pp.tile([128, NL1 * 4, 128], f16, tag="whs_s")
        nc.vector.memset(whs_s[:], 0)
        for j in range(NL1):
            for g in range(4):
                for h_ in range(2):
                    nc.sync.dma_start(
                        out=whs_s[64 * h_:64 * (h_ + 1), 4 * j + g,
                                  64 * h_:64 * (h_ + 1)],
                        in_=bass.AP(blob16, O16["whs"] + (j * 4 + g) * 64 * 64,
                                    [(64, 64), (1, 64)]))
        wfold_s = pp.tile([128, NL1 * 3, 64], f16, tag="wfold_s")
        for j in range(NL1):
            for g in range(3):
                nc.sync.dma_start(
                    out=wfold_s[:, 3 * j + g, :],
                    in_=bass.AP(blob16, O16["wfold"] + (j * 3 + g) * 16 * 64,
                                [(0, 8), (64, 16), (1, 64)]))

        slab_dram = dr.tile([128, SUB * 2], f16)
        ag_dram = dr.tile([NC, 128, SUB * 2], f16)
        cexp_dram = dr.tile([nchunk, 128, 6 * ECH], f16)
        sidx_dram = dr.tile([128, 2, EP], i16)
        y_dram = dr.tile([2, HALF], f32)

        # ================= INIT =================
        # sidx expand x16 (DRAM -> DRAM), one DMA per lane (3-dim AP limit)
        for t in range(2):
            dstap = bass.AP(sidx_dram.tensor,
                            sidx_dram[:].offset + t * EP,
                            [sidx_dram[:].ap[0], (1, EP)])
            nc.sync.dma_start(
                out=dstap,
                in_=bass.AP(blob_d, O16["sidx"] + t * EP, [(2 * EP, 8), (0, 16), (1, EP)]))
        # c-expansion: replicate x16 across partitions, fold inv into ea
        for k in range(nchunk):
            cc = wk.tile([128, 6, ECH], f16, **BUFA)
            for j in range(6):
                nc.sync.dma_start(
                    out=cc[:, j, :],
                    in_=bass.AP(blob16, O16["craw"] + j * EP + k * ECH,
                                [(6 * EP, 8), (0, 16), (1, ECH)]))
            ce = wk.tile([128, 6, ECH], f16, **BUFB)
            nc.vector.tensor_copy(out=ce[:, 0, :], in_=cc[:, 0, :])
            in0 = bass.AP(cc.tensor, cc[:].offset, [cc[:].ap[0], (0, 4), (1, ECH)])
            nc.vector.tensor_tensor(out=ce[:, 1:5, :], in0=in0,
                                    in1=cc[:, 1:5, :], op=OP.mult)
            nc.vector.tensor_copy(out=ce[:, 5, :], in_=cc[:, 5, :])
            nc.sync.dma_start(out=cexp_dram[k], in_=ce[:])

        # lin0 -> hT
        wlin0 = pp.tile([16, 2, 128], f16, tag="wlin0")
        nc.sync.dma_start(
            out=wlin0[:],
            in_=bass.AP(blob16, O16["wlin0"], [(128, 16), (16 * 128, 2), (1, 128)]))
        for c0, L in PIECES_H:
            xa = wk2.tile([16, 512], f16, **GGT)
            nc.sync.dma_start(
                out=xa[:, :L],
                in_=bass.AP(blob16, O16["xT"] + c0, [(NPAD, 16), (1, L)]))
            xb = wk2.tile([16, 512], f16, **GGT)
            nc.sync.dma_start(
                out=xb[:, :L],
                in_=bass.AP(blob16, O16["xT"] + HALF + c0, [(NPAD, 16), (1, L)]))
            p0 = ps.tile([128, 512], f32, tag="ps")
            nc.tensor.matmul(p0[:, :L], wlin0[:, 0, :], xa[:, :L],
                             start=True, stop=False)
            nc.tensor.matmul(p0[:, :L], wlin0[:, 1, :], xb[:, :L],
                             start=False, stop=True)
            nc.scalar.activation(out=hT[:, c0:c0 + L], in_=p0[:, :L],
                                 func=AF.Relu, bias=biases[:, 16:17], scale=1.0)

        # ================= ITERATIONS =================
        for it in range(niter):
            j = (it // 2) % NL1
            # ---- A: slab + exchange ----
            for c0, L in PIECES_S:
                p0 = ps.tile([128, 512], f32, tag="ps")
                for b in range(8):
                    rc0 = (b % 4) * SUB + c0
                    nc.tensor.matmul(p0[:, :L], wslab_s[:, b, :],
                                     hT[:, rc0:rc0 + L],
                                     start=(b == 0), stop=(b == 7))
                stg = wk2.tile([128, 1024], f16, tag="slabstg")
                for dup in range(2):
                    dst = bass.AP(stg.tensor, stg[:].offset + dup,
                                  [stg[:].ap[0], (2, L)])
                    nc.vector.tensor_copy(out=dst, in_=p0[:, :L])
                nc.sync.dma_start(out=slab_dram[:, c0 * 2:(c0 + L) * 2],
                                  in_=stg[:, :2 * L])
            if fake_collective:
                for cc_ in range(NC):
                    nc.sync.dma_start(out=ag_dram[cc_], in_=slab_dram[:])
            else:
                nc.gpsimd.collective_compute(
                    "AllGather", OP.bypass,
                    replica_groups=[list(range(NC))],
                    ins=[slab_dram[:].opt()], outs=[ag_dram[:].opt()])
            for s in range(8):
                src = bass.AP(ag_dram.tensor,
                              ag_dram[:].offset + (16 * s) * (SUB * 2),
                              [(128 * SUB * 2, 8), (SUB * 2, 16), (1, SUB * 2)])
                dst = bass.AP(table.tensor, table[:].offset + s * SUB * 2,
                              [table[:].ap[0], (1, SUB * 2)])
                nc.sync.dma_start(out=dst, in_=src)

            # ---- C: edge chunks (gathers batched 4x to amortize ap_gather) ----
            GCH = 4 * ECH
            gbuf = None
            for k in range(nchunk):
                cc = wk.tile([128, 6, ECH], f16, **BUFA)
                nc.sync.dma_start(out=cc[:], in_=cexp_dram[k])
                if k % 4 == 0:
                    G0 = k * ECH
                    GL = min(GCH, EP - G0)
                    gbuf = pp.tile([128, GCH, 2], f16, tag="gbuf")
                    if "gather" not in skip:
                        nc.gpsimd.ap_gather(
                            out_ap=gbuf[:, :GL, :], in_ap=table[:],
                            idxs_ap=gidx[:, G0 // 16:(G0 + GL) // 16],
                            channels=128, num_elems=NPAD, d=2, num_idxs=GL)
                    else:
                        nc.vector.memset(gbuf[:, 0:4, :], 0)
                sc = wk.tile([128, 5, ECH], f16, **BUFB)
                g_in0 = bass.AP(gbuf.tensor,
                                gbuf[:].offset + (k % 4) * ECH * 2,
                                [gbuf[:].ap[0], (0, 5), (2, ECH)])
                nc.vector.tensor_tensor(out=sc[:], in0=g_in0, in1=cc[:, 0:5, :],
                                        op=OP.mult)
                msg = ps.tile([128, 512], f32, tag="ps")
                for p in range(5):
                    nc.tensor.matmul(msg[:, :ECH], wM_s[:, p, :], sc[:, p, :],
                                     start=(p == 0), stop=(p == 4))
                if k == 0:
                    init = 0.0
                else:
                    init = bass.AP(cum.tensor, cum[:].offset + (k * ECH - 1),
                                   [cum[:].ap[0], (1, 1)])
                nc.vector.tensor_tensor_scan(out=cum[:, k * ECH:(k + 1) * ECH],
                                             data0=cc[:, 5, :],
                                             data1=msg[:, :ECH], initial=init,
                                             op0=OP.mult, op1=OP.add)

            # ---- D: per-segment local_scatter -> eb tiles ----
            ebx = pp.tile([128, 8 * EBX], f16, tag="ebx")
            ebs = pp.tile([128, 8, EBCAP], f16, tag="ebs")
            for r, s in enumerate(SEG_ORDER):
                st = wk.tile([128, 2, SEGCAP], i16, **BUFB)
                nc.sync.dma_start(
                    out=st[:],
                    in_=bass.AP(sidx_dram.tensor,
                                sidx_dram[:].offset + r * SEGCAP,
                                [sidx_dram[:].ap[0], (EP, 2), (1, SEGCAP)]))
                if "scatter" not in skip:
                    nc.gpsimd.local_scatter(
                        out_ap=ebs[:, s, :],
                        data_ap=cum[:, r * SEGCAP:(r + 1) * SEGCAP],
                        idxs_ap=st[:, 0, :],
                        channels=128, num_elems=EBCAP, num_idxs=SEGCAP)
                    nc.gpsimd.local_scatter(
                        out_ap=ebx[:, s * EBX:(s + 1) * EBX],
                        data_ap=cum[:, r * SEGCAP:(r + 1) * SEGCAP],
                        idxs_ap=st[:, 1, :],
                        channels=128, num_elems=EBX, num_idxs=SEGCAP)
                else:
                    nc.vector.memset(ebs[:, s, :], 0)
                    nc.vector.memset(ebx[:, s * EBX:(s + 1) * EBX], 0)

            # ---- E: gates (q-major: sub-chunk pair (q, 4+q) first) ----
            PIECES_Q = [(q * SUB + off, L) for q in range(4) for off, L in PIECES_S]
            for c0, L in PIECES_Q:
                pr = ps.tile([128, 512], f32, tag="ps")
                pz = ps.tile([128, 512], f32, tag="ps")
                pn = ps.tile([128, 512], f32, tag="ps")
                ph = ps.tile([128, 512], f32, tag="ps")
                for g, pst in enumerate([pr, pz, pn, ph]):
                    nc.tensor.matmul(pst[:, :L], whs_s[:, 4 * j + g, :],
                                     hT[:, c0:c0 + L],
                                     start=True, stop=True)
                for g, pst in enumerate([pr, pz, pn]):
                    for half in range(2):
                        pcs = ends_pieces(half * HALF + c0, L)
                        for (ch, eoff, eln) in pcs:
                            # split at the EBCAP boundary within the sub-chunk
                            parts = []
                            if eoff < EBCAP:
                                ln1 = min(eln, EBCAP - eoff)
                                parts.append((ebs, ch * EBCAP + eoff, ln1, eoff))
                            if eoff + eln > EBCAP:
                                o2 = max(eoff, EBCAP)
                                parts.append((ebx, ch * EBX + (o2 - EBCAP),
                                              eoff + eln - o2, o2))
                            for (tile_, toff, tln, nodeoff) in parts:
                                rhs = bass.AP(tile_.tensor,
                                              tile_[:].offset + toff,
                                              [tile_[:].ap[0], (1, tln)])
                                oo = (nodeoff + ch * SUB) - (half * HALF + c0)
                                out = bass.AP(
                                    pst.tensor,
                                    pst[:].offset + 64 * half * pst[:].ap[0][0] + oo,
                                    [(pst[:].ap[0][0], 64), (1, tln)])
                                tp = (0, 64) if half == 1 else None
                                nc.tensor.matmul(out, wfold_s[:, 3 * j + g, :], rhs,
                                                 start=False, stop=False,
                                                 skip_group_check=True,
                                                 tile_position=tp)
                r16 = wk2.tile([128, 512], f16, tag="g_r")
                z16 = wk2.tile([128, 512], f16, tag="g_z")
                nc.scalar.activation(out=r16[:, :L], in_=pr[:, :L], func=AF.Sigmoid,
                                     bias=biases[:, 4 * j:4 * j + 1], scale=1.0)
                nc.scalar.activation(out=z16[:, :L], in_=pz[:, :L], func=AF.Sigmoid,
                                     bias=biases[:, 4 * j + 1:4 * j + 2], scale=1.0)
                t16 = wk2.tile([128, 512], f16, tag="g_t")
                nc.vector.scalar_tensor_tensor(
                    out=t16[:, :L], in0=ph[:, :L],
                    scalar=biases[:, 4 * j + 3:4 * j + 4], in1=r16[:, :L],
                    op0=OP.add, op1=OP.mult)
                u16 = wk2.tile([128, 512], f16, tag="g_u")
                nc.vector.tensor_tensor(out=u16[:, :L], in0=t16[:, :L],
                                        in1=pn[:, :L], op=OP.add)
                n16 = wk2.tile([128, 512], f16, tag="g_n")
                nc.scalar.activation(out=n16[:, :L], in_=u16[:, :L], func=AF.Tanh,
                                     bias=biases[:, 4 * j + 2:4 * j + 3], scale=1.0)
                v16 = wk2.tile([128, 512], f16, tag="g_t")
                nc.vector.tensor_tensor(out=v16[:, :L], in0=hT[:, c0:c0 + L],
                                        in1=n16[:, :L], op=OP.subtract)
                w16 = wk2.tile([128, 512], f16, tag="g_u")
                nc.vector.tensor_tensor(out=w16[:, :L], in0=z16[:, :L],
                                        in1=v16[:, :L], op=OP.mult)
                nc.vector.tensor_tensor(out=hT[:, c0:c0 + L], in0=n16[:, :L],
                                        in1=w16[:, :L], op=OP.add)

        # ================= FINAL: y + pooling =================
        for c0, L in PIECES_H:
            py = ps.tile([2, 512], f32, tag="ps")
            nc.tensor.matmul(py[:, :L], wy[:], hT[:, c0:c0 + L],
                             start=True, stop=True)
            ystg = wk2.tile([2, 512], f32, **GGT)
            nc.vector.tensor_copy(out=ystg[:, :L], in_=py[:, :L])
            nc.sync.dma_start(out=y_dram[:, c0:c0 + L], in_=ystg[:, :L])
        ypool = wk.tile([128, SUB], f32, **BUFA)
        for half in range(2):
            src = bass.AP(y_dram.tensor, y_dram[:].offset + half * HALF,
                          [(SUB, 4), (0, 16), (1, SUB)])
            nc.sync.dma_start(out=ypool[:][64 * half:64 * (half + 1)], in_=src)
        ycum = wk.tile([128, SUB], f32, **BUFB)
        nc.vector.tensor_tensor_scan(out=ycum[:], data0=pmask[:],
                                     data1=ypool[:], initial=0.0,
                                     op0=OP.mult, op1=OP.add)
        pooled = wk2.tile([128, GSLOT], f32, tag="g_r")
        nc.gpsimd.ap_gather(out_ap=pooled[:], in_ap=ycum[:], idxs_ap=pidx[:],
                            channels=128, num_elems=SUB, d=1, num_idxs=GSLOT)
        nc.sync.dma_start(out=out_d[:], in_=pooled[:][0::16])

    nc.compile()
    return nc


# ================= driver =================
_CACHE = {}


def kernel(**inputs):
    inputs = {k: np.asarray(v) for k, v in inputs.items()}
    in_maps_data, meta = host_prep(inputs)
    w = fold_weights_host(inputs)
    EP = meta["EP"]
    if EP not in _CACHE:
        _CACHE[EP] = build(EP)
    nc = _CACHE[EP]
    from concourse.bass_utils import run_bass_kernel_spmd
    in_maps = [{"blob": pack_blob(in_maps_data[c], w, EP)} for c in range(NC)]
    trace = os.environ.get("KERNEL_PROFILE", "0") == "1"
    br = run_bass_kernel_spmd(nc, in_maps, list(range(NC)), trace=trace)
    if trace and br.exec_time_ns is not None:
        print(f"HW exec time: {br.exec_time_ns} ns")
    got = np.zeros(NG, np.float32)
    for c in range(NC):
        pooled = br.results[c]["pooled"]
        for s in range(8):
            for i, (g, endpos) in enumerate(meta["pool_graphs"][c][s]):
                got[g] = pooled[s, i]
    return got


# revision 17
# speedup vs baseline: 13.5791x; 1.2189x over previous
"""Trainium2 Bass kernel for nn_Net_76622216561354 (gnn_message_passing).

Self-contained: host-side sharding/index prep (numpy) + an 8-core SPMD
Bass/Tile kernel run via run_bass_kernel_spmd. Accepts FULL inputs, returns
the FULL pooled output [8192] float32.

v2: dst-side aggregation uses gpsimd.local_scatter (streaming, ~4cyc/idx)
instead of ap_gather over run-end positions (~70cyc/idx); edge streams are
re-sorted by (src-block, dst sub-chunk) with per-segment padding so each
scatter call covers a static stream window; per-edge scalar tables and
index tensors ship compact ([8,...]) and are partition-replicated x16 on
device via 0-stride DMA, cutting shipped input bytes ~5x.
"""
import numpy as np
import os

NC = 8
N = 131072; E = 524288; F_IN = 16; DIM = 64; DNN = 16; BK = 4; NG = 8192
NL1 = 4; NL2 = 2
SUB = 2112
NPAD = 8 * SUB          # 16896
HALF = NPAD // 2        # 8448
ECH = 512
GSLOT = 192             # pooled graph slots per sub-chunk (padded)
EBCAP = 2046            # local_scatter num_elems cap (num_elems*32 < 2^16)
EBX = SUB - EBCAP       # 66 tail slots per sub-chunk -> cleanup lane
# dst sub-chunk segment order in the edge stream: half-pairs (q, 4+q)
# complete early so gates can start before the full scan pipeline drains
SEG_ORDER = [0, 4, 1, 5, 2, 6, 3, 7]
SEG_RANK = {s: r for r, s in enumerate(SEG_ORDER)}


def host_prep(inputs):
    ei = np.asarray(inputs["edge_index"])
    batch = np.asarray(inputs["batch"]).astype(np.int64)
    src, dst = ei[0].astype(np.int64), ei[1].astype(np.int64)

    # ---- graph spans ----
    gsizes = np.bincount(batch, minlength=NG)
    gstart = np.concatenate([[0], np.cumsum(gsizes)])

    # ---- core cuts at graph boundaries ----
    cuts = [0]
    for c in range(1, NC):
        t = c * (N // NC)
        while t < N and batch[t] == batch[t - 1]:
            t += 1
        cuts.append(t)
    cuts.append(N)
    cuts = np.array(cuts, np.int64)

    # ---- per-core: pack graphs into 8 graph-aligned sub-chunks ----
    g2l = np.full(N, -1, np.int64)      # global node -> local slot (within its core)
    node_core = np.zeros(N, np.int64)
    l2g = [np.full(NPAD, -1, np.int64) for _ in range(NC)]
    pool_graphs = [[[] for _ in range(8)] for _ in range(NC)]
    pool_mask = [np.zeros((8, SUB), np.float32) for _ in range(NC)]

    for c in range(NC):
        lo, hi = cuts[c], cuts[c + 1]
        glo, ghi = batch[lo], (batch[hi - 1] + 1 if hi > lo else batch[lo])
        s = 0; pos = 0
        for g in range(glo, ghi):
            sz = int(gsizes[g])
            if sz == 0:
                continue
            if pos + sz > SUB:
                s += 1; pos = 0
                assert s < 8, f"core {c}: sub-chunk overflow"
                assert sz <= SUB
            nodes = np.arange(gstart[g], gstart[g] + sz)
            slots = s * SUB + pos + np.arange(sz)
            g2l[nodes] = slots
            node_core[nodes] = c
            l2g[c][slots] = nodes
            pool_mask[c][s, pos + 1: pos + sz] = 1.0
            pool_graphs[c][s].append((g, pos + sz - 1))
            pos += sz
        assert hi == lo or batch[hi - 1] + 1 == ghi

    dst_core = node_core[dst]; src_core = node_core[src]
    dstslot = g2l[dst]; srcslot = g2l[src]
    dsub = dstslot // SUB               # dst sub-chunk

    # ---- segment capacity: max edges per (dst core, src block, dst sub) ----
    key = (dst_core * NC + src_core) * 8 + dsub
    segcnt = np.bincount(key, minlength=NC * NC * 8)
    SEGCAP = int(-(-int(segcnt.max()) // 64) * 64)
    EP = 8 * SEGCAP
    nchunk = EP // ECH

    indeg = np.bincount(dst, minlength=N).astype(np.float64)
    inv = 1.0 / np.maximum(indeg, 1.0)
    ea_all = np.asarray(inputs["edge_attr"])

    per_core = []
    for c in range(NC):
        gidx = np.zeros((8, EP), np.int64)       # src slot per stream pos
        craw = np.zeros((8, 6, EP), np.float32)  # inv, ea*4, same-mask
        sidx = np.full((8, 2, EP), -1, np.int64)  # run-end targets: main, clean
        for b in range(NC):
            m = (dst_core == c) & (src_core == b)
            eids = np.nonzero(m)[0]
            order = np.lexsort((dstslot[eids],))
            eids = eids[order]                   # sorted by dstslot => by (dsub, slot)
            dsl = dstslot[eids]
            ds = dsl // SUB
            # position within stream: segment base + rank within segment
            seg_off = np.zeros(len(eids), np.int64)
            for s in range(8):
                sm = ds == s
                k = int(sm.sum())
                assert k <= SEGCAP
                seg_off[sm] = SEG_RANK[s] * SEGCAP + np.arange(k)
            gidx[b, seg_off] = srcslot[eids]
            craw[b, 0, seg_off] = inv[dst[eids]]
            for q in range(BK):
                craw[b, 1 + q, seg_off] = ea_all[eids, q]
            # same-dst continuation mask (within segment; runs never span segs)
            same = np.zeros(len(eids), bool)
            if len(eids) > 0:
                same[1:] = (dsl[1:] == dsl[:-1])
                craw[b, 5, seg_off] = same.astype(np.float32)
                # run ends: last edge of each dst run
                is_end = np.ones(len(eids), bool)
                is_end[:-1] = dsl[1:] != dsl[:-1]
                epos = seg_off[is_end]
                eslot = dsl[is_end] % SUB        # slot within sub-chunk
                main = eslot < EBCAP
                sidx[b, 0, epos[main]] = eslot[main]
                sidx[b, 1, epos[~main]] = eslot[~main] - EBCAP
        per_core.append(dict(gidx=gidx, craw=craw, sidx=sidx))

    # ---- wrap helper: seq -> [16, L/16] with idx[p, s] = seq[s*16+p] ----
    def wrap16(seq):
        L = len(seq)
        assert L % 16 == 0
        return np.asarray(seq).reshape(L // 16, 16).T.copy()

    in_maps = []
    for c in range(NC):
        pc = per_core[c]
        # gather idx: [128, EP/16] int16, wrapped per gather-call window (GCH)
        GCH = 4 * ECH
        gidx_t = np.zeros((128, EP // 16), np.int16)
        for b in range(NC):
            off = 0
            while off < EP:
                L = min(GCH, EP - off)
                seq = pc["gidx"][b, off:off + L]
                gidx_t[16 * b:16 * (b + 1), off // 16:(off + L) // 16] = \
                    wrap16(seq).astype(np.int16)
                off += L
        pidx_t = np.zeros((128, GSLOT // 16), np.int16)
        for s in range(8):
            seq = np.zeros(GSLOT, np.int64)
            gl = pool_graphs[c][s]
            assert len(gl) <= GSLOT, f"GSLOT overflow: {len(gl)}"
            for i, (g, endpos) in enumerate(gl):
                seq[i] = endpos
            pidx_t[16 * s:16 * (s + 1), :] = wrap16(seq).astype(np.int16)
        # x slab transposed [16, NPAD] f16
        xT = np.zeros((16, NPAD), np.float16)
        real = l2g[c] >= 0
        xT[:, real] = np.asarray(inputs["x"])[l2g[c][real]].T.astype(np.float16)
        in_maps.append(dict(
            xT=xT,
            gidx=gidx_t,
            craw=pc["craw"].astype(np.float16),
            sidx=pc["sidx"].astype(np.int16),
            pmask=pool_mask[c].astype(np.float16),
            pidx=pidx_t))

    meta = dict(EP=EP, SEGCAP=SEGCAP, nchunk=nchunk, cuts=cuts,
                pool_graphs=pool_graphs, l2g=l2g)
    return in_maps, meta


def fold_weights_host(inputs):
    """float64 weight folds -> shipped stationaries/biases (per-core identical)."""
    dt = np.float64
    lin0_w = np.asarray(inputs["lin0_w"], dt); lin0_b = np.asarray(inputs["lin0_b"], dt)
    lin1_w = np.asarray(inputs["lin1_w"], dt); lin1_b = np.asarray(inputs["lin1_b"], dt)
    lin2_w = np.asarray(inputs["lin2_w"], dt)
    root_w = np.asarray(inputs["root_w"], dt); conv_b = np.asarray(inputs["conv_b"], dt)
    nn1_w = np.asarray(inputs["nn1_w"], dt); nn1_b = np.asarray(inputs["nn1_b"], dt)
    gw_ih = np.asarray(inputs["gru_w_ih"], dt); gw_hh = np.asarray(inputs["gru_w_hh"], dt)
    gb_ih = np.asarray(inputs["gru_b_ih"], dt); gb_hh = np.asarray(inputs["gru_b_hh"], dt)

    Bm = nn1_b.reshape(DNN, DNN)
    Ak = nn1_w.reshape(BK, DNN, DNN)
    M = np.concatenate([Bm[None], Ak], axis=0)            # [5,16,16]

    w = {}
    # compact stationaries; expanded into block layouts on device at INIT
    w["wlin1s"] = lin1_w.astype(np.float16)               # [64,16]
    w["wMs"] = M.astype(np.float16)                       # [5,16,16]
    whs = np.zeros((NL1, 4, 64, 64), np.float32)
    wfold = np.zeros((NL1, 3, 16, 64), np.float32)
    biases = np.zeros((128, 17), np.float32)
    for j in range(NL1):
        P = lin1_w @ root_w @ gw_ih[j].T                  # [64,192]
        W_rz = P[:, :2 * DIM] + gw_hh[j].T[:, :2 * DIM]
        W_ni = P[:, 2 * DIM:]
        W_nh = gw_hh[j].T[:, 2 * DIM:]
        grp_w = [W_rz[:, :64], W_rz[:, 64:], W_ni, W_nh]
        for g in range(4):
            whs[j, g] = grp_w[g]
        wihT = gw_ih[j].T                                  # [16,192]
        for g in range(3):
            wfold[j, g] = wihT[:, 64 * g:64 * (g + 1)]
        b_base = (lin1_b @ root_w + conv_b) @ gw_ih[j].T   # [192]
        b_rz = b_base[:2 * DIM] + gb_ih[j][:2 * DIM] + gb_hh[j][:2 * DIM]
        b_ni = b_base[2 * DIM:] + gb_ih[j][2 * DIM:]
        b_hn = gb_hh[j][2 * DIM:]
        vec = [b_rz[:64], b_rz[64:], b_ni, b_hn]
        for g in range(4):
            biases[0:64, 4 * j + g] = vec[g]
            biases[64:128, 4 * j + g] = vec[g]
    w["whs"] = whs.astype(np.float16)
    w["wfold"] = wfold.astype(np.float16)
    biases[0:64, 16] = lin0_b
    biases[64:128, 16] = lin0_b
    w["biases"] = biases.astype(np.float32)
    # lin0 stationaries [2, 16, 128] f16
    wlin0 = np.zeros((2, 16, 128), np.float32)
    wlin0[0, :, 0:64] = lin0_w
    wlin0[1, :, 64:128] = lin0_w
    w["wlin0"] = wlin0.astype(np.float16)
    # y stationary [128, 2] f16
    wy = np.zeros((128, 2), np.float32)
    wy[0:64, 0] = lin2_w[:, 0]
    wy[64:128, 1] = lin2_w[:, 0]
    w["wy"] = wy.astype(np.float16)
    return w


def blob_layout(EP):
    """Fixed packing order of all per-core inputs into one int16 blob.
    Returns (offsets_bytes, total_int16_words)."""
    sizes = [
        ("xT", 16 * NPAD * 2),
        ("gidx", 128 * (EP // 16) * 2),
        ("craw", 8 * 6 * EP * 2),
        ("sidx", 8 * 2 * EP * 2),
        ("pmask", 8 * SUB * 2),
        ("pidx", 128 * (GSLOT // 16) * 2),
        ("wlin1s", 64 * 16 * 2),
        ("wMs", 5 * 16 * 16 * 2),
        ("whs", NL1 * 4 * 64 * 64 * 2),
        ("wfold", NL1 * 3 * 16 * 64 * 2),
        ("wlin0", 2 * 16 * 128 * 2),
        ("wy", 128 * 2 * 2),
        ("biases", 128 * 17 * 4),
    ]
    offs = {}
    off = 0
    for name, nb in sizes:
        off = (off + 3) & ~3          # 4-byte align every section
        offs[name] = off
        off += nb
    total = ((off + 3) & ~3) // 2
    return offs, total


def pack_blob(in_map, w, EP):
    offs, total = blob_layout(EP)
    blob = np.zeros(total, np.int16)
    data = dict(in_map); data.update(w)
    for name, off in offs.items():
        arr = np.ascontiguousarray(data[name])
        nb = arr.nbytes
        view = blob[off // 2:(off + nb) // 2]
        if arr.dtype == np.float32:
            view.view(np.float32)[:] = arr.ravel()
        elif arr.dtype == np.float16:
            view.view(np.float16)[:] = arr.ravel()
        elif arr.dtype == np.int16:
            view[:] = arr.ravel()
        else:
            raise ValueError(f"{name}: {arr.dtype}")
    return blob[None, :]


# ================= kernel builder =================

import concourse.bass as bass
import concourse.mybir as mybir
import concourse.tile as tile
from concourse import bacc
from contextlib import ExitStack

NITER = 8

f32 = mybir.dt.float32
f16 = mybir.dt.float16
i16 = mybir.dt.int16
AF = mybir.ActivationFunctionType
OP = mybir.AluOpType


def pieces(total, step):
    out = []
    off = 0
    while off < total:
        out.append((off, min(step, total - off)))
        off += step
    return out


def ends_pieces(c0, L):
    """Split node-column window [c0, c0+L) into (sub-chunk, offset, len)."""
    out = []
    while L > 0:
        ch = c0 // SUB
        off = c0 % SUB
        ln = min(L, SUB - off)
        out.append((ch, off, ln))
        c0 += ln
        L -= ln
    return out


def build(EP, fake_collective=False, niter=NITER, skip=()):
    SEGCAP = EP // 8
    nchunk = EP // ECH
    nc = bacc.Bacc("TRN2", target_bir_lowering=False, debug=False, num_devices=NC)

    OFFS, TOTW = blob_layout(EP)
    blob_d = nc.dram_tensor("blob", [1, TOTW], i16, kind="ExternalInput")
    blob16 = blob_d.bitcast(f16)
    blob32 = blob_d.bitcast(f32)
    O16 = {k: v // 2 for k, v in OFFS.items()}   # element offsets, 2-byte view
    O32 = {k: v // 4 for k, v in OFFS.items()}   # element offsets, 4-byte view
    out_d = nc.dram_tensor("pooled", [8, GSLOT], f32, kind="ExternalOutput")

    PIECES_H = pieces(HALF, 512)
    PIECES_S = pieces(SUB, 512)

    with tile.TileContext(nc) as tc, ExitStack() as ex:
        pp = ex.enter_context(tc.tile_pool(name="persist", bufs=1))
        wk = ex.enter_context(tc.tile_pool(name="work", bufs=2))
        wk2 = ex.enter_context(tc.tile_pool(name="work2", bufs=2))
        ps = ex.enter_context(tc.tile_pool(name="psum", bufs=8, space="PSUM"))
        dr = ex.enter_context(tc.tile_pool(name="dram", bufs=1, space="DRAM"))

        BUFA = dict(tag="bufA")
        BUFB = dict(tag="bufB")
        GGT = dict(tag="gg")

        hT = pp.tile([128, HALF], f16, tag="hT")
        table = pp.tile([128, NPAD, 2], f16, tag="table")
        if skip:
            nc.vector.memset(table[:], 0)
        cum = pp.tile([128, EP], f16, tag="cum")
        nc.vector.memset(cum[:], 0)
        gidx = pp.tile([128, EP // 16], i16, tag="gidx")
        pmask = pp.tile([128, SUB], f16, tag="pmask")
        pidx = pp.tile([128, GSLOT // 16], i16, tag="pidx")
        biases = pp.tile([128, 17], f32, tag="biases")
        wy = pp.tile([128, 2], f16, tag="wy")

        nc.sync.dma_start(
            out=gidx[:],
            in_=bass.AP(blob_d, O16["gidx"], [(EP // 16, 128), (1, EP // 16)]))
        nc.sync.dma_start(
            out=pmask[:],
            in_=bass.AP(blob16, O16["pmask"], [(SUB, 8), (0, 16), (1, SUB)]))
        nc.sync.dma_start(
            out=pidx[:],
            in_=bass.AP(blob_d, O16["pidx"],
                        [(GSLOT // 16, 128), (1, GSLOT // 16)]))
        nc.sync.dma_start(
            out=biases[:],
            in_=bass.AP(blob32, O32["biases"], [(17, 128), (1, 17)]))
        nc.sync.dma_start(
            out=wy[:], in_=bass.AP(blob16, O16["wy"], [(2, 128), (1, 2)]))

        # stationaries expanded on device from compact shipped blocks
        wslab_s = pp.tile([128, 8, 128], f16, tag="wslab_s")
        nc.vector.memset(wslab_s[:], 0)
        for b in range(8):
            r0 = 64 * (b // 4)
            nc.sync.dma_start(
                out=wslab_s[r0:r0 + 64, b, 16 * b:16 * (b + 1)],
                in_=bass.AP(blob16, O16["wlin1s"], [(16, 64), (1, 16)]))
        wM_s = pp.tile([128, 5, 128], f16, tag="wM_s")
        nc.vector.memset(wM_s[:], 0)
        for p in range(5):
            for b in range(8):
                nc.sync.dma_start(
                    out=wM_s[16 * b:16 * (b + 1), p, 16 * b:16 * (b + 1)],
                    in_=bass.AP(blob16, O16["wMs"] + p * 256, [(16, 16), (1, 16)]))
        whs_s = pp.tile([128, NL1 * 4, 128], f16, tag="whs_s")
        nc.vector.memset(whs_s[:], 0)
        for j in range(NL1):
            for g in range(4):
                for h_ in range(2):
                    nc.sync.dma_start(
                        out=whs_s[64 * h_:64 * (h_ + 1), 4 * j + g,
                                  64 * h_:64 * (h_ + 1)],
                        in_=bass.AP(blob16, O16["whs"] + (j * 4 + g) * 64 * 64,
                                    [(64, 64), (1, 64)]))
        wfold_s = pp.tile([128, NL1 * 3, 64], f16, tag="wfold_s")
        for j in range(NL1):
            for g in range(3):
                nc.sync.dma_start(
                    out=wfold_s[:, 3 * j + g, :],
                    in_=bass.AP(blob16, O16["wfold"] + (j * 3 + g) * 16 * 64,
                                [(0, 8), (64, 16), (1, 64)]))

        slab_dram = dr.tile([128, SUB], f16)
        ag_drams = [dr.tile([NC, 128, SUB], f16, addr_space="Shared",
                            name=f"ag{i}", tag=f"ag{i}")
                    for i in range(niter)]
        cexp_dram = dr.tile([nchunk, 128, 6 * ECH], f16)
        sidx_dram = dr.tile([128, 2, EP], i16)
        y_dram = dr.tile([2, HALF], f32)

        # ================= INIT =================
        # sidx expand x16 (DRAM -> DRAM), one DMA per lane (3-dim AP limit)
        for t in range(2):
            dstap = bass.AP(sidx_dram.tensor,
                            sidx_dram[:].offset + t * EP,
                            [sidx_dram[:].ap[0], (1, EP)])
            nc.sync.dma_start(
                out=dstap,
                in_=bass.AP(blob_d, O16["sidx"] + t * EP, [(2 * EP, 8), (0, 16), (1, EP)]))
        # c-expansion: replicate x16 across partitions, fold inv into ea
        for k in range(nchunk):
            cc = wk.tile([128, 6, ECH], f16, **BUFA)
            for j in range(6):
                nc.sync.dma_start(
                    out=cc[:, j, :],
                    in_=bass.AP(blob16, O16["craw"] + j * EP + k * ECH,
                                [(6 * EP, 8), (0, 16), (1, ECH)]))
            ce = wk.tile([128, 6, ECH], f16, **BUFB)
            nc.vector.tensor_copy(out=ce[:, 0, :], in_=cc[:, 0, :])
            in0 = bass.AP(cc.tensor, cc[:].offset, [cc[:].ap[0], (0, 4), (1, ECH)])
            nc.vector.tensor_tensor(out=ce[:, 1:5, :], in0=in0,
                                    in1=cc[:, 1:5, :], op=OP.mult)
            nc.vector.tensor_copy(out=ce[:, 5, :], in_=cc[:, 5, :])
            nc.sync.dma_start(out=cexp_dram[k], in_=ce[:])

        # lin0 -> hT
        wlin0 = pp.tile([16, 2, 128], f16, tag="wlin0")
        nc.sync.dma_start(
            out=wlin0[:],
            in_=bass.AP(blob16, O16["wlin0"], [(128, 16), (16 * 128, 2), (1, 128)]))
        for c0, L in PIECES_H:
            xa = wk2.tile([16, 512], f16, **GGT)
            nc.sync.dma_start(
                out=xa[:, :L],
                in_=bass.AP(blob16, O16["xT"] + c0, [(NPAD, 16), (1, L)]))
            xb = wk2.tile([16, 512], f16, **GGT)
            nc.sync.dma_start(
                out=xb[:, :L],
                in_=bass.AP(blob16, O16["xT"] + HALF + c0, [(NPAD, 16), (1, L)]))
            p0 = ps.tile([128, 512], f32, tag="ps")
            nc.tensor.matmul(p0[:, :L], wlin0[:, 0, :], xa[:, :L],
                             start=True, stop=False)
            nc.tensor.matmul(p0[:, :L], wlin0[:, 1, :], xb[:, :L],
                             start=False, stop=True)
            nc.scalar.activation(out=hT[:, c0:c0 + L], in_=p0[:, :L],
                                 func=AF.Relu, bias=biases[:, 16:17], scale=1.0)

        # ================= ITERATIONS =================
        for it in range(niter):
            j = (it // 2) % NL1
            # ---- A: slab + exchange ----
            for c0, L in (PIECES_S if "slab" not in skip else []):
                p0 = ps.tile([128, 512], f32, tag="ps")
                for b in range(8):
                    rc0 = (b % 4) * SUB + c0
                    nc.tensor.matmul(p0[:, :L], wslab_s[:, b, :],
                                     hT[:, rc0:rc0 + L],
                                     start=(b == 0), stop=(b == 7))
                stg = wk2.tile([128, 512], f16, tag="slabstg")
                nc.vector.tensor_copy(out=stg[:, :L], in_=p0[:, :L])
                nc.sync.dma_start(out=slab_dram[:, c0:c0 + L],
                                  in_=stg[:, :L])
            ag_dram = ag_drams[it]
            if "exch" not in skip:
                if "coll" not in skip:
                    if fake_collective:
                        for cc_ in range(NC):
                            nc.sync.dma_start(out=ag_dram[cc_], in_=slab_dram[:])
                    else:
                        nc.gpsimd.collective_compute(
                            "AllGather", OP.bypass,
                            replica_groups=[list(range(NC))],
                            ins=[slab_dram[:].opt()], outs=[ag_dram[:].opt()])
                for s in range(8 if "rearr" not in skip else 0):
                    stag = wk2.tile([128, SUB], f16, tag="stag")
                    src = bass.AP(ag_dram.tensor,
                                  ag_dram[:].offset + (16 * s) * SUB,
                                  [(128 * SUB, 8), (SUB, 16), (1, SUB)])
                    nc.sync.dma_start(out=stag[:], in_=src)
                    for dup in range(2):
                        dst = bass.AP(table.tensor,
                                      table[:].offset + s * SUB * 2 + dup,
                                      [table[:].ap[0], (2, SUB)])
                        nc.vector.tensor_copy(out=dst, in_=stag[:])

            # ---- C: edge chunks (gathers batched 4x to amortize ap_gather) ----
            GCH = 4 * ECH
            gbuf = None
            for k in range(nchunk):
                cc = wk.tile([128, 6, ECH], f16, **BUFA)
                nc.sync.dma_start(out=cc[:], in_=cexp_dram[k])
                if k % 4 == 0:
                    G0 = k * ECH
                    GL = min(GCH, EP - G0)
                    gbuf = pp.tile([128, GCH, 2], f16, tag="gbuf")
                    if "gather" not in skip:
                        nc.gpsimd.ap_gather(
                            out_ap=gbuf[:, :GL, :], in_ap=table[:],
                            idxs_ap=gidx[:, G0 // 16:(G0 + GL) // 16],
                            channels=128, num_elems=NPAD, d=2, num_idxs=GL)
                    else:
                        nc.vector.memset(gbuf[:, 0:4, :], 0)
                sc = wk.tile([128, 5, ECH], f16, **BUFB)
                g_in0 = bass.AP(gbuf.tensor,
                                gbuf[:].offset + (k % 4) * ECH * 2,
                                [gbuf[:].ap[0], (0, 5), (2, ECH)])
                nc.vector.tensor_tensor(out=sc[:], in0=g_in0, in1=cc[:, 0:5, :],
                                        op=OP.mult)
                msg = ps.tile([128, 512], f32, tag="ps")
                for p in range(5):
                    nc.tensor.matmul(msg[:, :ECH], wM_s[:, p, :], sc[:, p, :],
                                     start=(p == 0), stop=(p == 4))
                if k == 0:
                    init = 0.0
                else:
                    init = bass.AP(cum.tensor, cum[:].offset + (k * ECH - 1),
                                   [cum[:].ap[0], (1, 1)])
                nc.vector.tensor_tensor_scan(out=cum[:, k * ECH:(k + 1) * ECH],
                                             data0=cc[:, 5, :],
                                             data1=msg[:, :ECH], initial=init,
                                             op0=OP.mult, op1=OP.add)

            # ---- D: per-segment local_scatter -> eb tiles ----
            ebx = pp.tile([128, 8 * EBX], f16, tag="ebx")
            ebs = pp.tile([128, 8, EBCAP], f16, tag="ebs")
            for r, s in enumerate(SEG_ORDER):
                st = wk.tile([128, 2, SEGCAP], i16, **BUFB)
                nc.sync.dma_start(
                    out=st[:],
                    in_=bass.AP(sidx_dram.tensor,
                                sidx_dram[:].offset + r * SEGCAP,
                                [sidx_dram[:].ap[0], (EP, 2), (1, SEGCAP)]))
                if "scatter" not in skip:
                    nc.gpsimd.local_scatter(
                        out_ap=ebs[:, s, :],
                        data_ap=cum[:, r * SEGCAP:(r + 1) * SEGCAP],
                        idxs_ap=st[:, 0, :],
                        channels=128, num_elems=EBCAP, num_idxs=SEGCAP)
                    nc.gpsimd.local_scatter(
                        out_ap=ebx[:, s * EBX:(s + 1) * EBX],
                        data_ap=cum[:, r * SEGCAP:(r + 1) * SEGCAP],
                        idxs_ap=st[:, 1, :],
                        channels=128, num_elems=EBX, num_idxs=SEGCAP)
                else:
                    nc.vector.memset(ebs[:, s, :], 0)
                    nc.vector.memset(ebx[:, s * EBX:(s + 1) * EBX], 0)

            # ---- E: gates (q-major: sub-chunk pair (q, 4+q) first) ----
            PIECES_Q = [(q * SUB + off, L) for q in range(4) for off, L in PIECES_S]
            for c0, L in PIECES_Q:
                pr = ps.tile([128, 512], f32, tag="ps")
                pz = ps.tile([128, 512], f32, tag="ps")
                pn = ps.tile([128, 512], f32, tag="ps")
                ph = ps.tile([128, 512], f32, tag="ps")
                for g, pst in enumerate([pr, pz, pn, ph]):
                    nc.tensor.matmul(pst[:, :L], whs_s[:, 4 * j + g, :],
                                     hT[:, c0:c0 + L],
                                     start=True, stop=True)
                for g, pst in enumerate([pr, pz, pn]):
                    for half in range(2):
                        pcs = ends_pieces(half * HALF + c0, L)
                        for (ch, eoff, eln) in pcs:
                            # split at the EBCAP boundary within the sub-chunk
                            parts = []
                            if eoff < EBCAP:
                                ln1 = min(eln, EBCAP - eoff)
                                parts.append((ebs, ch * EBCAP + eoff, ln1, eoff))
                            if eoff + eln > EBCAP:
                                o2 = max(eoff, EBCAP)
                                parts.append((ebx, ch * EBX + (o2 - EBCAP),
                                              eoff + eln - o2, o2))
                            for (tile_, toff, tln, nodeoff) in parts:
                                rhs = bass.AP(tile_.tensor,
                                              tile_[:].offset + toff,
                                              [tile_[:].ap[0], (1, tln)])
                                oo = (nodeoff + ch * SUB) - (half * HALF + c0)
                                out = bass.AP(
                                    pst.tensor,
                                    pst[:].offset + 64 * half * pst[:].ap[0][0] + oo,
                                    [(pst[:].ap[0][0], 64), (1, tln)])
                                tp = (0, 64) if half == 1 else None
                                nc.tensor.matmul(out, wfold_s[:, 3 * j + g, :], rhs,
                                                 start=False, stop=False,
                                                 skip_group_check=True,
                                                 tile_position=tp)
                r16 = wk2.tile([128, 512], f16, tag="g_r")
                z16 = wk2.tile([128, 512], f16, tag="g_z")
                nc.scalar.activation(out=r16[:, :L], in_=pr[:, :L], func=AF.Sigmoid,
                                     bias=biases[:, 4 * j:4 * j + 1], scale=1.0)
                nc.scalar.activation(out=z16[:, :L], in_=pz[:, :L], func=AF.Sigmoid,
                                     bias=biases[:, 4 * j + 1:4 * j + 2], scale=1.0)
                t16 = wk2.tile([128, 512], f16, tag="g_t")
                nc.vector.scalar_tensor_tensor(
                    out=t16[:, :L], in0=ph[:, :L],
                    scalar=biases[:, 4 * j + 3:4 * j + 4], in1=r16[:, :L],
                    op0=OP.add, op1=OP.mult)
                u16 = wk2.tile([128, 512], f16, tag="g_u")
                nc.vector.tensor_tensor(out=u16[:, :L], in0=t16[:, :L],
                                        in1=pn[:, :L], op=OP.add)
                n16 = wk2.tile([128, 512], f16, tag="g_n")
                nc.scalar.activation(out=n16[:, :L], in_=u16[:, :L], func=AF.Tanh,
                                     bias=biases[:, 4 * j + 2:4 * j + 3], scale=1.0)
                v16 = wk2.tile([128, 512], f16, tag="g_t")
                nc.vector.tensor_tensor(out=v16[:, :L], in0=hT[:, c0:c0 + L],
                                        in1=n16[:, :L], op=OP.subtract)
                w16 = wk2.tile([128, 512], f16, tag="g_u")
                nc.vector.tensor_tensor(out=w16[:, :L], in0=z16[:, :L],
                                        in1=v16[:, :L], op=OP.mult)
                nc.vector.tensor_tensor(out=hT[:, c0:c0 + L], in0=n16[:, :L],
                                        in1=w16[:, :L], op=OP.add)

        # ================= FINAL: y + pooling =================
        for c0, L in PIECES_H:
            py = ps.tile([2, 512], f32, tag="ps")
            nc.tensor.matmul(py[:, :L], wy[:], hT[:, c0:c0 + L],
                             start=True, stop=True)
            ystg = wk2.tile([2, 512], f32, **GGT)
            nc.vector.tensor_copy(out=ystg[:, :L], in_=py[:, :L])
            nc.sync.dma_start(out=y_dram[:, c0:c0 + L], in_=ystg[:, :L])
        ypool = pp.tile([128, SUB], f32, tag="gbuf")
        for half in range(2):
            src = bass.AP(y_dram.tensor, y_dram[:].offset + half * HALF,
                          [(SUB, 4), (0, 16), (1, SUB)])
            nc.sync.dma_start(out=ypool[:][64 * half:64 * (half + 1)], in_=src)
        ycum = pp.tile([128, SUB], f32, tag="cum")
        nc.vector.tensor_tensor_scan(out=ycum[:], data0=pmask[:],
                                     data1=ypool[:], initial=0.0,
                                     op0=OP.mult, op1=OP.add)
        pooled = wk2.tile([128, GSLOT], f32, tag="g_r")
        nc.gpsimd.ap_gather(out_ap=pooled[:], in_ap=ycum[:], idxs_ap=pidx[:],
                            channels=128, num_elems=SUB, d=1, num_idxs=GSLOT)
        nc.sync.dma_start(out=out_d[:], in_=pooled[:][0::16])

    nc.compile()
    return nc


# ================= driver =================
_CACHE = {}


def kernel(**inputs):
    inputs = {k: np.asarray(v) for k, v in inputs.items()}
    in_maps_data, meta = host_prep(inputs)
    w = fold_weights_host(inputs)
    EP = meta["EP"]
    if EP not in _CACHE:
        _CACHE[EP] = build(EP)
    nc = _CACHE[EP]
    from concourse.bass_utils import run_bass_kernel_spmd
    in_maps = [{"blob": pack_blob(in_maps_data[c], w, EP)} for c in range(NC)]
    trace = os.environ.get("KERNEL_PROFILE", "0") == "1"
    br = run_bass_kernel_spmd(nc, in_maps, list(range(NC)), trace=trace)
    if trace and br.exec_time_ns is not None:
        print(f"HW exec time: {br.exec_time_ns} ns")
    got = np.zeros(NG, np.float32)
    for c in range(NC):
        pooled = br.results[c]["pooled"]
        for s in range(8):
            for i, (g, endpos) in enumerate(meta["pool_graphs"][c][s]):
                got[g] = pooled[s, i]
    return got


# revision 21
# speedup vs baseline: 15.5662x; 1.1463x over previous
"""Trainium2 Bass kernel for nn_Net_76622216561354 (gnn_message_passing).

Self-contained: host-side sharding/index prep (numpy) + an 8-core SPMD
Bass/Tile kernel run via run_bass_kernel_spmd. Accepts FULL inputs, returns
the FULL pooled output [8192] float32.

v2: dst-side aggregation uses gpsimd.local_scatter (streaming, ~4cyc/idx)
instead of ap_gather over run-end positions (~70cyc/idx); edge streams are
re-sorted by (src-block, dst sub-chunk) with per-segment padding so each
scatter call covers a static stream window; per-edge scalar tables and
index tensors ship compact ([8,...]) and are partition-replicated x16 on
device via 0-stride DMA, cutting shipped input bytes ~5x.
"""
import numpy as np
import os

NC = 8
N = 131072; E = 524288; F_IN = 16; DIM = 64; DNN = 16; BK = 4; NG = 8192
NL1 = 4; NL2 = 2
SUB = 2112
NPAD = 8 * SUB          # 16896
HALF = NPAD // 2        # 8448
ECH = 512
GSLOT = 192             # pooled graph slots per sub-chunk (padded)
EBCAP = 2046            # local_scatter num_elems cap (num_elems*32 < 2^16)
EBX = SUB - EBCAP       # 66 tail slots per sub-chunk -> cleanup lane
# dst sub-chunk segment order in the edge stream: half-pairs (q, 4+q)
# complete early so gates can start before the full scan pipeline drains
SEG_ORDER = [0, 4, 1, 5, 2, 6, 3, 7]
SEG_RANK = {s: r for r, s in enumerate(SEG_ORDER)}


def host_prep(inputs):
    ei = np.asarray(inputs["edge_index"])
    batch = np.asarray(inputs["batch"]).astype(np.int64)
    src, dst = ei[0].astype(np.int64), ei[1].astype(np.int64)

    # ---- graph spans ----
    gsizes = np.bincount(batch, minlength=NG)
    gstart = np.concatenate([[0], np.cumsum(gsizes)])

    # ---- core cuts at graph boundaries ----
    cuts = [0]
    for c in range(1, NC):
        t = c * (N // NC)
        while t < N and batch[t] == batch[t - 1]:
            t += 1
        cuts.append(t)
    cuts.append(N)
    cuts = np.array(cuts, np.int64)

    # ---- per-core: pack graphs into 8 graph-aligned sub-chunks ----
    g2l = np.full(N, -1, np.int64)      # global node -> local slot (within its core)
    node_core = np.zeros(N, np.int64)
    l2g = [np.full(NPAD, -1, np.int64) for _ in range(NC)]
    pool_graphs = [[[] for _ in range(8)] for _ in range(NC)]
    pool_mask = [np.zeros((8, SUB), np.float32) for _ in range(NC)]

    for c in range(NC):
        lo, hi = cuts[c], cuts[c + 1]
        glo, ghi = batch[lo], (batch[hi - 1] + 1 if hi > lo else batch[lo])
        s = 0; pos = 0
        for g in range(glo, ghi):
            sz = int(gsizes[g])
            if sz == 0:
                continue
            if pos + sz > SUB:
                s += 1; pos = 0
                assert s < 8, f"core {c}: sub-chunk overflow"
                assert sz <= SUB
            nodes = np.arange(gstart[g], gstart[g] + sz)
            slots = s * SUB + pos + np.arange(sz)
            g2l[nodes] = slots
            node_core[nodes] = c
            l2g[c][slots] = nodes
            pool_mask[c][s, pos + 1: pos + sz] = 1.0
            pool_graphs[c][s].append((g, pos + sz - 1))
            pos += sz
        assert hi == lo or batch[hi - 1] + 1 == ghi

    dst_core = node_core[dst]; src_core = node_core[src]
    dstslot = g2l[dst]; srcslot = g2l[src]
    dsub = dstslot // SUB               # dst sub-chunk

    # ---- segment capacity: max edges per (dst core, src block, dst sub) ----
    key = (dst_core * NC + src_core) * 8 + dsub
    segcnt = np.bincount(key, minlength=NC * NC * 8)
    SEGCAP = int(-(-int(segcnt.max()) // 64) * 64)
    EP = 8 * SEGCAP
    nchunk = EP // ECH

    indeg = np.bincount(dst, minlength=N).astype(np.float64)
    inv = 1.0 / np.maximum(indeg, 1.0)
    ea_all = np.asarray(inputs["edge_attr"])

    per_core = []
    for c in range(NC):
        gidx = np.zeros((8, EP), np.int64)       # src slot per stream pos
        craw = np.zeros((8, 6, EP), np.float32)  # inv, ea*4, same-mask
        sidx = np.full((8, 2, EP), -1, np.int64)  # run-end targets: main, clean
        for b in range(NC):
            m = (dst_core == c) & (src_core == b)
            eids = np.nonzero(m)[0]
            order = np.lexsort((dstslot[eids],))
            eids = eids[order]                   # sorted by dstslot => by (dsub, slot)
            dsl = dstslot[eids]
            ds = dsl // SUB
            # position within stream: segment base + rank within segment
            seg_off = np.zeros(len(eids), np.int64)
            for s in range(8):
                sm = ds == s
                k = int(sm.sum())
                assert k <= SEGCAP
                seg_off[sm] = SEG_RANK[s] * SEGCAP + np.arange(k)
            gidx[b, seg_off] = srcslot[eids]
            craw[b, 0, seg_off] = inv[dst[eids]]
            for q in range(BK):
                craw[b, 1 + q, seg_off] = ea_all[eids, q]
            # same-dst continuation mask (within segment; runs never span segs)
            same = np.zeros(len(eids), bool)
            if len(eids) > 0:
                same[1:] = (dsl[1:] == dsl[:-1])
                craw[b, 5, seg_off] = same.astype(np.float32)
                # run ends: last edge of each dst run
                is_end = np.ones(len(eids), bool)
                is_end[:-1] = dsl[1:] != dsl[:-1]
                epos = seg_off[is_end]
                eslot = dsl[is_end] % SUB        # slot within sub-chunk
                main = eslot < EBCAP
                sidx[b, 0, epos[main]] = eslot[main]
                sidx[b, 1, epos[~main]] = eslot[~main] - EBCAP
        per_core.append(dict(gidx=gidx, craw=craw, sidx=sidx))

    # ---- wrap helper: seq -> [16, L/16] with idx[p, s] = seq[s*16+p] ----
    def wrap16(seq):
        L = len(seq)
        assert L % 16 == 0
        return np.asarray(seq).reshape(L // 16, 16).T.copy()

    in_maps = []
    for c in range(NC):
        pc = per_core[c]
        # gather idx: [128, EP/16] int16, wrapped per gather-call window (GCH)
        GCH = 4 * ECH
        gidx_t = np.zeros((128, EP // 16), np.int16)
        for b in range(NC):
            off = 0
            while off < EP:
                L = min(GCH, EP - off)
                seq = pc["gidx"][b, off:off + L]
                gidx_t[16 * b:16 * (b + 1), off // 16:(off + L) // 16] = \
                    wrap16(seq).astype(np.int16)
                off += L
        pidx_t = np.zeros((128, GSLOT // 16), np.int16)
        for s in range(8):
            seq = np.zeros(GSLOT, np.int64)
            gl = pool_graphs[c][s]
            assert len(gl) <= GSLOT, f"GSLOT overflow: {len(gl)}"
            for i, (g, endpos) in enumerate(gl):
                seq[i] = endpos
            pidx_t[16 * s:16 * (s + 1), :] = wrap16(seq).astype(np.int16)
        # x slab transposed [16, NPAD] f16
        xT = np.zeros((16, NPAD), np.float16)
        real = l2g[c] >= 0
        xT[:, real] = np.asarray(inputs["x"])[l2g[c][real]].T.astype(np.float16)
        in_maps.append(dict(
            xT=xT,
            gidx=gidx_t,
            craw=pc["craw"].astype(np.float16),
            sidx=pc["sidx"].astype(np.int16),
            pmask=pool_mask[c].astype(np.float16),
            pidx=pidx_t))

    meta = dict(EP=EP, SEGCAP=SEGCAP, nchunk=nchunk, cuts=cuts,
                pool_graphs=pool_graphs, l2g=l2g)
    return in_maps, meta


def fold_weights_host(inputs):
    """float64 weight folds -> shipped stationaries/biases (per-core identical)."""
    dt = np.float64
    lin0_w = np.asarray(inputs["lin0_w"], dt); lin0_b = np.asarray(inputs["lin0_b"], dt)
    lin1_w = np.asarray(inputs["lin1_w"], dt); lin1_b = np.asarray(inputs["lin1_b"], dt)
    lin2_w = np.asarray(inputs["lin2_w"], dt)
    root_w = np.asarray(inputs["root_w"], dt); conv_b = np.asarray(inputs["conv_b"], dt)
    nn1_w = np.asarray(inputs["nn1_w"], dt); nn1_b = np.asarray(inputs["nn1_b"], dt)
    gw_ih = np.asarray(inputs["gru_w_ih"], dt); gw_hh = np.asarray(inputs["gru_w_hh"], dt)
    gb_ih = np.asarray(inputs["gru_b_ih"], dt); gb_hh = np.asarray(inputs["gru_b_hh"], dt)

    Bm = nn1_b.reshape(DNN, DNN)
    Ak = nn1_w.reshape(BK, DNN, DNN)
    M = np.concatenate([Bm[None], Ak], axis=0)            # [5,16,16]

    w = {}
    # compact stationaries; expanded into block layouts on device at INIT
    w["wlin1s"] = lin1_w.astype(np.float16)               # [64,16]
    w["wMs"] = M.astype(np.float16)                       # [5,16,16]
    whs = np.zeros((NL1, 4, 64, 64), np.float32)
    wfold = np.zeros((NL1, 3, 16, 64), np.float32)
    biases = np.zeros((128, 17), np.float32)
    for j in range(NL1):
        P = lin1_w @ root_w @ gw_ih[j].T                  # [64,192]
        W_rz = P[:, :2 * DIM] + gw_hh[j].T[:, :2 * DIM]
        W_ni = P[:, 2 * DIM:]
        W_nh = gw_hh[j].T[:, 2 * DIM:]
        grp_w = [W_rz[:, :64], W_rz[:, 64:], W_ni, W_nh]
        for g in range(4):
            whs[j, g] = grp_w[g]
        wihT = gw_ih[j].T                                  # [16,192]
        for g in range(3):
            wfold[j, g] = wihT[:, 64 * g:64 * (g + 1)]
        b_base = (lin1_b @ root_w + conv_b) @ gw_ih[j].T   # [192]
        b_rz = b_base[:2 * DIM] + gb_ih[j][:2 * DIM] + gb_hh[j][:2 * DIM]
        b_ni = b_base[2 * DIM:] + gb_ih[j][2 * DIM:]
        b_hn = gb_hh[j][2 * DIM:]
        vec = [b_rz[:64], b_rz[64:], b_ni, b_hn]
        for g in range(4):
            biases[0:64, 4 * j + g] = vec[g]
            biases[64:128, 4 * j + g] = vec[g]
    w["whs"] = whs.astype(np.float16)
    w["wfold"] = wfold.astype(np.float16)
    biases[0:64, 16] = lin0_b
    biases[64:128, 16] = lin0_b
    w["biases"] = biases.astype(np.float32)
    # lin0 stationaries [2, 16, 128] f16
    wlin0 = np.zeros((2, 16, 128), np.float32)
    wlin0[0, :, 0:64] = lin0_w
    wlin0[1, :, 64:128] = lin0_w
    w["wlin0"] = wlin0.astype(np.float16)
    # y stationary [128, 2] f16
    wy = np.zeros((128, 2), np.float32)
    wy[0:64, 0] = lin2_w[:, 0]
    wy[64:128, 1] = lin2_w[:, 0]
    w["wy"] = wy.astype(np.float16)
    return w


def blob_layout(EP):
    """Fixed packing order of all per-core inputs into one int16 blob.
    Returns (offsets_bytes, total_int16_words)."""
    sizes = [
        ("xT", 16 * NPAD * 2),
        ("gidx", 128 * (EP // 16) * 2),
        ("craw", 8 * 6 * EP * 2),
        ("sidx", 8 * 2 * EP * 2),
        ("pmask", 8 * SUB * 2),
        ("pidx", 128 * (GSLOT // 16) * 2),
        ("wlin1s", 64 * 16 * 2),
        ("wMs", 5 * 16 * 16 * 2),
        ("whs", NL1 * 4 * 64 * 64 * 2),
        ("wfold", NL1 * 3 * 16 * 64 * 2),
        ("wlin0", 2 * 16 * 128 * 2),
        ("wy", 128 * 2 * 2),
        ("biases", 128 * 17 * 4),
    ]
    offs = {}
    off = 0
    for name, nb in sizes:
        off = (off + 3) & ~3          # 4-byte align every section
        offs[name] = off
        off += nb
    total = ((off + 3) & ~3) // 2
    return offs, total


def pack_blob(in_map, w, EP):
    offs, total = blob_layout(EP)
    blob = np.zeros(total, np.int16)
    data = dict(in_map); data.update(w)
    for name, off in offs.items():
        arr = np.ascontiguousarray(data[name])
        nb = arr.nbytes
        view = blob[off // 2:(off + nb) // 2]
        if arr.dtype == np.float32:
            view.view(np.float32)[:] = arr.ravel()
        elif arr.dtype == np.float16:
            view.view(np.float16)[:] = arr.ravel()
        elif arr.dtype == np.int16:
            view[:] = arr.ravel()
        else:
            raise ValueError(f"{name}: {arr.dtype}")
    return blob[None, :]


# ================= kernel builder =================

import concourse.bass as bass
import concourse.mybir as mybir
import concourse.tile as tile
from concourse import bacc
from contextlib import ExitStack

NITER = 8

f32 = mybir.dt.float32
f16 = mybir.dt.float16
i16 = mybir.dt.int16
AF = mybir.ActivationFunctionType
OP = mybir.AluOpType


def pieces(total, step):
    out = []
    off = 0
    while off < total:
        out.append((off, min(step, total - off)))
        off += step
    return out


def ends_pieces(c0, L):
    """Split node-column window [c0, c0+L) into (sub-chunk, offset, len)."""
    out = []
    while L > 0:
        ch = c0 // SUB
        off = c0 % SUB
        ln = min(L, SUB - off)
        out.append((ch, off, ln))
        c0 += ln
        L -= ln
    return out


def build(EP, fake_collective=False, niter=NITER, skip=()):
    SEGCAP = EP // 8
    nchunk = EP // ECH
    nc = bacc.Bacc("TRN2", target_bir_lowering=False, debug=False, num_devices=NC)

    OFFS, TOTW = blob_layout(EP)
    blob_d = nc.dram_tensor("blob", [1, TOTW], i16, kind="ExternalInput")
    blob16 = blob_d.bitcast(f16)
    blob32 = blob_d.bitcast(f32)
    O16 = {k: v // 2 for k, v in OFFS.items()}   # element offsets, 2-byte view
    O32 = {k: v // 4 for k, v in OFFS.items()}   # element offsets, 4-byte view
    out_d = nc.dram_tensor("pooled", [8, GSLOT], f32, kind="ExternalOutput")

    PIECES_H = pieces(HALF, 512)
    PIECES_S = pieces(SUB, 512)

    with tile.TileContext(nc) as tc, ExitStack() as ex:
        pp = ex.enter_context(tc.tile_pool(name="persist", bufs=1))
        wk = ex.enter_context(tc.tile_pool(name="work", bufs=2))
        wk2 = ex.enter_context(tc.tile_pool(name="work2", bufs=2))
        ps = ex.enter_context(tc.tile_pool(name="psum", bufs=8, space="PSUM"))
        dr = ex.enter_context(tc.tile_pool(name="dram", bufs=1, space="DRAM"))

        BUFA = dict(tag="bufA")
        BUFB = dict(tag="bufB")
        GGT = dict(tag="gg")

        hT = pp.tile([128, HALF], f16, tag="hT")
        table = pp.tile([128, NPAD, 2], f16, tag="table")
        if skip:
            nc.vector.memset(table[:], 0)
        cum = pp.tile([128, EP], f16, tag="cum")
        nc.vector.memset(cum[:], 0)
        gidx = pp.tile([128, EP // 16], i16, tag="gidx")
        pmask = pp.tile([128, SUB], f16, tag="pmask")
        pidx = pp.tile([128, GSLOT // 16], i16, tag="pidx")
        biases = pp.tile([128, 17], f32, tag="biases")
        wy = pp.tile([128, 2], f16, tag="wy")

        nc.sync.dma_start(
            out=gidx[:],
            in_=bass.AP(blob_d, O16["gidx"], [(EP // 16, 128), (1, EP // 16)]))
        nc.sync.dma_start(
            out=pmask[:],
            in_=bass.AP(blob16, O16["pmask"], [(SUB, 8), (0, 16), (1, SUB)]))
        nc.sync.dma_start(
            out=pidx[:],
            in_=bass.AP(blob_d, O16["pidx"],
                        [(GSLOT // 16, 128), (1, GSLOT // 16)]))
        nc.sync.dma_start(
            out=biases[:],
            in_=bass.AP(blob32, O32["biases"], [(17, 128), (1, 17)]))
        nc.sync.dma_start(
            out=wy[:], in_=bass.AP(blob16, O16["wy"], [(2, 128), (1, 2)]))

        # stationaries expanded on device from compact shipped blocks
        wslab_s = pp.tile([128, 8, 128], f16, tag="wslab_s")
        nc.vector.memset(wslab_s[:], 0)
        for b in range(8):
            r0 = 64 * (b // 4)
            nc.sync.dma_start(
                out=wslab_s[r0:r0 + 64, b, 16 * b:16 * (b + 1)],
                in_=bass.AP(blob16, O16["wlin1s"], [(16, 64), (1, 16)]))
        wM_s = pp.tile([128, 5, 128], f16, tag="wM_s")
        nc.vector.memset(wM_s[:], 0)
        for p in range(5):
            for b in range(8):
                nc.sync.dma_start(
                    out=wM_s[16 * b:16 * (b + 1), p, 16 * b:16 * (b + 1)],
                    in_=bass.AP(blob16, O16["wMs"] + p * 256, [(16, 16), (1, 16)]))
        whs_s = pp.tile([128, NL1 * 4, 128], f16, tag="whs_s")
        nc.vector.memset(whs_s[:], 0)
        for j in range(NL1):
            for g in range(4):
                for h_ in range(2):
                    nc.sync.dma_start(
                        out=whs_s[64 * h_:64 * (h_ + 1), 4 * j + g,
                                  64 * h_:64 * (h_ + 1)],
                        in_=bass.AP(blob16, O16["whs"] + (j * 4 + g) * 64 * 64,
                                    [(64, 64), (1, 64)]))
        wfold_s = pp.tile([128, NL1 * 3, 64], f16, tag="wfold_s")
        for j in range(NL1):
            for g in range(3):
                nc.sync.dma_start(
                    out=wfold_s[:, 3 * j + g, :],
                    in_=bass.AP(blob16, O16["wfold"] + (j * 3 + g) * 16 * 64,
                                [(0, 8), (64, 16), (1, 64)]))

        slab_dram = dr.tile([128, SUB], f16)
        ag_dram = dr.tile([NC, 128, SUB], f16)
        cexp_dram = dr.tile([nchunk, 128, 6 * ECH], f16)
        y_dram = dr.tile([2, HALF], f32)

        # ================= INIT =================
        # c-expansion: replicate x16 across partitions, fold inv into ea
        for k in range(nchunk):
            cc = wk.tile([128, 6, ECH], f16, **BUFA)
            for j in range(6):
                nc.sync.dma_start(
                    out=cc[:, j, :],
                    in_=bass.AP(blob16, O16["craw"] + j * EP + k * ECH,
                                [(6 * EP, 8), (0, 16), (1, ECH)]))
            ce = wk.tile([128, 6, ECH], f16, **BUFB)
            nc.vector.tensor_copy(out=ce[:, 0, :], in_=cc[:, 0, :])
            in0 = bass.AP(cc.tensor, cc[:].offset, [cc[:].ap[0], (0, 4), (1, ECH)])
            nc.vector.tensor_tensor(out=ce[:, 1:5, :], in0=in0,
                                    in1=cc[:, 1:5, :], op=OP.mult)
            nc.vector.tensor_copy(out=ce[:, 5, :], in_=cc[:, 5, :])
            nc.sync.dma_start(out=cexp_dram[k], in_=ce[:])

        # lin0 -> hT
        wlin0 = pp.tile([16, 2, 128], f16, tag="wlin0")
        nc.sync.dma_start(
            out=wlin0[:],
            in_=bass.AP(blob16, O16["wlin0"], [(128, 16), (16 * 128, 2), (1, 128)]))
        for c0, L in PIECES_H:
            xa = wk2.tile([16, 512], f16, **GGT)
            nc.sync.dma_start(
                out=xa[:, :L],
                in_=bass.AP(blob16, O16["xT"] + c0, [(NPAD, 16), (1, L)]))
            xb = wk2.tile([16, 512], f16, **GGT)
            nc.sync.dma_start(
                out=xb[:, :L],
                in_=bass.AP(blob16, O16["xT"] + HALF + c0, [(NPAD, 16), (1, L)]))
            p0 = ps.tile([128, 512], f32, tag="ps")
            nc.tensor.matmul(p0[:, :L], wlin0[:, 0, :], xa[:, :L],
                             start=True, stop=False)
            nc.tensor.matmul(p0[:, :L], wlin0[:, 1, :], xb[:, :L],
                             start=False, stop=True)
            nc.scalar.activation(out=hT[:, c0:c0 + L], in_=p0[:, :L],
                                 func=AF.Relu, bias=biases[:, 16:17], scale=1.0)

        # ================= ITERATIONS =================
        for it in range(niter):
            j = (it // 2) % NL1
            # ---- A: slab + exchange ----
            for c0, L in (PIECES_S if "slab" not in skip else []):
                p0 = ps.tile([128, 512], f32, tag="ps")
                for b in range(8):
                    rc0 = (b % 4) * SUB + c0
                    nc.tensor.matmul(p0[:, :L], wslab_s[:, b, :],
                                     hT[:, rc0:rc0 + L],
                                     start=(b == 0), stop=(b == 7))
                stg = wk2.tile([128, 512], f16, tag="slabstg")
                nc.vector.tensor_copy(out=stg[:, :L], in_=p0[:, :L])
                nc.sync.dma_start(out=slab_dram[:, c0:c0 + L],
                                  in_=stg[:, :L])
            if "exch" not in skip:
                if "coll" not in skip:
                    if fake_collective:
                        for cc_ in range(NC):
                            nc.sync.dma_start(out=ag_dram[cc_], in_=slab_dram[:])
                    else:
                        nc.gpsimd.collective_compute(
                            "AllGather", OP.bypass,
                            replica_groups=[list(range(NC))],
                            ins=[slab_dram[:].opt()], outs=[ag_dram[:].opt()])
                for s in range(8 if "rearr" not in skip else 0):
                    stag = wk2.tile([128, SUB], f16, tag="stag")
                    src = bass.AP(ag_dram.tensor,
                                  ag_dram[:].offset + (16 * s) * SUB,
                                  [(128 * SUB, 8), (SUB, 16), (1, SUB)])
                    nc.sync.dma_start(out=stag[:], in_=src)
                    for dup in range(2):
                        dst = bass.AP(table.tensor,
                                      table[:].offset + s * SUB * 2 + dup,
                                      [table[:].ap[0], (2, SUB)])
                        nc.vector.tensor_copy(out=dst, in_=stag[:])

            # ---- C: edge chunks (gathers batched 4x to amortize ap_gather) ----
            GCH = 4 * ECH
            gbuf = None
            for k in range(nchunk):
                cc = wk.tile([128, 6, ECH], f16, **BUFA)
                nc.sync.dma_start(out=cc[:], in_=cexp_dram[k])
                if k % 4 == 0:
                    G0 = k * ECH
                    GL = min(GCH, EP - G0)
                    gbuf = pp.tile([128, GCH, 2], f16, tag="gbuf")
                    if "gather" not in skip:
                        nc.gpsimd.ap_gather(
                            out_ap=gbuf[:, :GL, :], in_ap=table[:],
                            idxs_ap=gidx[:, G0 // 16:(G0 + GL) // 16],
                            channels=128, num_elems=NPAD, d=2, num_idxs=GL)
                    else:
                        nc.vector.memset(gbuf[:, 0:4, :], 0)
                sc = wk.tile([128, 5, ECH], f16, **BUFB)
                g_in0 = bass.AP(gbuf.tensor,
                                gbuf[:].offset + (k % 4) * ECH * 2,
                                [gbuf[:].ap[0], (0, 5), (2, ECH)])
                nc.vector.tensor_tensor(out=sc[:], in0=g_in0, in1=cc[:, 0:5, :],
                                        op=OP.mult)
                msg = ps.tile([128, 512], f32, tag="ps")
                for p in range(5):
                    nc.tensor.matmul(msg[:, :ECH], wM_s[:, p, :], sc[:, p, :],
                                     start=(p == 0), stop=(p == 4))
                if k == 0:
                    init = 0.0
                else:
                    init = bass.AP(cum.tensor, cum[:].offset + (k * ECH - 1),
                                   [cum[:].ap[0], (1, 1)])
                nc.vector.tensor_tensor_scan(out=cum[:, k * ECH:(k + 1) * ECH],
                                             data0=cc[:, 5, :],
                                             data1=msg[:, :ECH], initial=init,
                                             op0=OP.mult, op1=OP.add)

            # ---- D: per-segment local_scatter -> eb tiles ----
            ebx = pp.tile([128, 8 * EBX], f16, tag="ebx")
            ebs = pp.tile([128, 8, EBCAP], f16, tag="ebs")
            for r, s in enumerate(SEG_ORDER):
                st = wk.tile([128, 2, SEGCAP], i16, **BUFB)
                for t in range(2):
                    nc.sync.dma_start(
                        out=st[:, t, :],
                        in_=bass.AP(blob_d,
                                    O16["sidx"] + t * EP + r * SEGCAP,
                                    [(2 * EP, 8), (0, 16), (1, SEGCAP)]))
                if "scatter" not in skip:
                    nc.gpsimd.local_scatter(
                        out_ap=ebs[:, s, :],
                        data_ap=cum[:, r * SEGCAP:(r + 1) * SEGCAP],
                        idxs_ap=st[:, 0, :],
                        channels=128, num_elems=EBCAP, num_idxs=SEGCAP)
                    nc.gpsimd.local_scatter(
                        out_ap=ebx[:, s * EBX:(s + 1) * EBX],
                        data_ap=cum[:, r * SEGCAP:(r + 1) * SEGCAP],
                        idxs_ap=st[:, 1, :],
                        channels=128, num_elems=EBX, num_idxs=SEGCAP)
                else:
                    nc.vector.memset(ebs[:, s, :], 0)
                    nc.vector.memset(ebx[:, s * EBX:(s + 1) * EBX], 0)

            # ---- E: gates (q-major: sub-chunk pair (q, 4+q) first) ----
            PIECES_Q = [(q * SUB + off, L) for q in range(4) for off, L in PIECES_S]
            for c0, L in PIECES_Q:
                pr = ps.tile([128, 512], f32, tag="ps")
                pz = ps.tile([128, 512], f32, tag="ps")
                pn = ps.tile([128, 512], f32, tag="ps")
                ph = ps.tile([128, 512], f32, tag="ps")
                for g, pst in enumerate([pr, pz, pn, ph]):
                    nc.tensor.matmul(pst[:, :L], whs_s[:, 4 * j + g, :],
                                     hT[:, c0:c0 + L],
                                     start=True, stop=True)
                for g, pst in enumerate([pr, pz, pn]):
                    for half in range(2):
                        pcs = ends_pieces(half * HALF + c0, L)
                        for (ch, eoff, eln) in pcs:
                            # split at the EBCAP boundary within the sub-chunk
                            parts = []
                            if eoff < EBCAP:
                                ln1 = min(eln, EBCAP - eoff)
                                parts.append((ebs, ch * EBCAP + eoff, ln1, eoff))
                            if eoff + eln > EBCAP:
                                o2 = max(eoff, EBCAP)
                                parts.append((ebx, ch * EBX + (o2 - EBCAP),
                                              eoff + eln - o2, o2))
                            for (tile_, toff, tln, nodeoff) in parts:
                                rhs = bass.AP(tile_.tensor,
                                              tile_[:].offset + toff,
                                              [tile_[:].ap[0], (1, tln)])
                                oo = (nodeoff + ch * SUB) - (half * HALF + c0)
                                out = bass.AP(
                                    pst.tensor,
                                    pst[:].offset + 64 * half * pst[:].ap[0][0] + oo,
                                    [(pst[:].ap[0][0], 64), (1, tln)])
                                tp = (0, 64) if half == 1 else None
                                nc.tensor.matmul(out, wfold_s[:, 3 * j + g, :], rhs,
                                                 start=False, stop=False,
                                                 skip_group_check=True,
                                                 tile_position=tp)
                r16 = wk2.tile([128, 512], f16, tag="g_r")
                z16 = wk2.tile([128, 512], f16, tag="g_z")
                nc.scalar.activation(out=r16[:, :L], in_=pr[:, :L], func=AF.Sigmoid,
                                     bias=biases[:, 4 * j:4 * j + 1], scale=1.0)
                nc.scalar.activation(out=z16[:, :L], in_=pz[:, :L], func=AF.Sigmoid,
                                     bias=biases[:, 4 * j + 1:4 * j + 2], scale=1.0)
                t16 = wk2.tile([128, 512], f16, tag="g_t")
                nc.vector.scalar_tensor_tensor(
                    out=t16[:, :L], in0=ph[:, :L],
                    scalar=biases[:, 4 * j + 3:4 * j + 4], in1=r16[:, :L],
                    op0=OP.add, op1=OP.mult)
                u16 = wk2.tile([128, 512], f16, tag="g_u")
                nc.vector.tensor_tensor(out=u16[:, :L], in0=t16[:, :L],
                                        in1=pn[:, :L], op=OP.add)
                n16 = wk2.tile([128, 512], f16, tag="g_n")
                nc.scalar.activation(out=n16[:, :L], in_=u16[:, :L], func=AF.Tanh,
                                     bias=biases[:, 4 * j + 2:4 * j + 3], scale=1.0)
                v16 = wk2.tile([128, 512], f16, tag="g_t")
                nc.vector.tensor_tensor(out=v16[:, :L], in0=hT[:, c0:c0 + L],
                                        in1=n16[:, :L], op=OP.subtract)
                w16 = wk2.tile([128, 512], f16, tag="g_u")
                nc.vector.tensor_tensor(out=w16[:, :L], in0=z16[:, :L],
                                        in1=v16[:, :L], op=OP.mult)
                nc.vector.tensor_tensor(out=hT[:, c0:c0 + L], in0=n16[:, :L],
                                        in1=w16[:, :L], op=OP.add)

        # ================= FINAL: y + pooling =================
        for c0, L in PIECES_H:
            py = ps.tile([2, 512], f32, tag="ps")
            nc.tensor.matmul(py[:, :L], wy[:], hT[:, c0:c0 + L],
                             start=True, stop=True)
            ystg = wk2.tile([2, 512], f32, **GGT)
            nc.vector.tensor_copy(out=ystg[:, :L], in_=py[:, :L])
            nc.sync.dma_start(out=y_dram[:, c0:c0 + L], in_=ystg[:, :L])
        ypool = pp.tile([128, SUB], f32, tag="gbuf")
        for half in range(2):
            src = bass.AP(y_dram.tensor, y_dram[:].offset + half * HALF,
                          [(SUB, 4), (0, 16), (1, SUB)])
            nc.sync.dma_start(out=ypool[:][64 * half:64 * (half + 1)], in_=src)
        ycum = pp.tile([128, SUB], f32, tag="cum")
        nc.vector.tensor_tensor_scan(out=ycum[:], data0=pmask[:],
                                     data1=ypool[:], initial=0.0,
                                     op0=OP.mult, op1=OP.add)
        pooled = wk2.tile([128, GSLOT], f32, tag="g_r")
        nc.gpsimd.ap_gather(out_ap=pooled[:], in_ap=ycum[:], idxs_ap=pidx[:],
                            channels=128, num_elems=SUB, d=1, num_idxs=GSLOT)
        nc.sync.dma_start(out=out_d[:], in_=pooled[:][0::16])

    nc.compile()
    return nc


# ================= driver =================
_CACHE = {}


def kernel(**inputs):
    inputs = {k: np.asarray(v) for k, v in inputs.items()}
    in_maps_data, meta = host_prep(inputs)
    w = fold_weights_host(inputs)
    EP = meta["EP"]
    if EP not in _CACHE:
        _CACHE[EP] = build(EP)
    nc = _CACHE[EP]
    from concourse.bass_utils import run_bass_kernel_spmd
    in_maps = [{"blob": pack_blob(in_maps_data[c], w, EP)} for c in range(NC)]
    trace = os.environ.get("KERNEL_PROFILE", "0") == "1"
    br = run_bass_kernel_spmd(nc, in_maps, list(range(NC)), trace=trace)
    if trace and br.exec_time_ns is not None:
        print(f"HW exec time: {br.exec_time_ns} ns")
    got = np.zeros(NG, np.float32)
    for c in range(NC):
        pooled = br.results[c]["pooled"]
        for s in range(8):
            for i, (g, endpos) in enumerate(meta["pool_graphs"][c][s]):
                got[g] = pooled[s, i]
    return got
